# revision 1
# baseline (speedup 1.0000x reference)
"""DIEN forward on 8 Trainium2 NeuronCores (Bass/Tile).

Data-parallel with ragged packing:
 - Host sorts batch rows by descending hist_valid_lens, deals them to the 8
   cores round-robin over the sorted order, and builds a shared per-timestep
   active-column schedule N_t (max over cores, padded to x4).  All per-step
   shapes are compile-time constants; rows whose sequence ended simply fall
   out of the processed column prefix (no masking ops in the scans).
 - Feature-major layout: interests/h at SBUF partitions 64:128 of one packed
   buffer BUF, gathered hist embeddings x^T at partitions 0:64, with x(t)
   stored at the columns of h(t-1) so each GRU step runs K=128-stacked
   matmuls over [x_t ; h_{t-1}] with a single rhs AP.
 - Attention runs over packed ragged columns; L3 scatters scores straight
   into a batch-major [r, t] PSUM via per-region column matmuls; softmax is
   batch-major; att is transposed back (PE) and streamed to the AUGRU as
   [1, N] rows broadcast via K=1 ones-matmuls.
 - BatchNorm (training stats over the full 2048 batch) via a tiny AllReduce.

kernel(**inputs) takes FULL unsharded inputs, returns [B, 1] float32.
"""

import hashlib
import numpy as np

import concourse.bass as bass
import concourse.bacc as bacc
import concourse.tile as tile
from concourse import mybir
from concourse.bass_utils import run_bass_kernel_spmd
from concourse.masks import make_identity

F32 = mybir.dt.float32
I32 = mybir.dt.int32
AF = mybir.ActivationFunctionType
OP = mybir.AluOpType
AX = mybir.AxisListType

B, T, E, NF, SL, DL, VOCAB = 2048, 200, 32, 2, 8, 16, 100000
D = E * NF          # 64
NCORES = 8
BC = B // NCORES    # 256
GCH = 16            # gather chunks (128 cols each) per indirect DMA


# --------------------------------------------------------------------------
def _make_schedule(lens):
    order = np.argsort(-lens, kind="stable")
    core_lens = lens[order].reshape(-1, NCORES)       # [BC, 8]
    tmax = int(lens.max())
    nts = []
    for t in range(tmax):
        n = int((core_lens > t).sum(axis=0).max())
        n = min(BC, ((n + 3) // 4) * 4)
        nts.append(max(n, 4))
    nts = np.array(nts, np.int32)
    offs = np.zeros(tmax + 1, np.int64)
    offs[1:] = np.cumsum(nts)
    m_total = int(offs[tmax])
    n0 = int(nts[0])
    xcols = np.zeros(tmax, np.int64)
    xcols[1:] = n0 + offs[:tmax - 1]                  # x(t) at h(t-1) cols
    hcols = n0 + offs[:tmax]
    xspan = int(n0 + offs[tmax - 1]) if tmax > 1 else int(nts[0])
    xspan = max(xspan, int(xcols[tmax - 1] + nts[tmax - 1]))
    xspan_pad = ((xspan + 128 * GCH - 1) // (128 * GCH)) * (128 * GCH)
    buf_cols = max(n0 + m_total, ((xspan + 127) // 128) * 128 + 128)
    return dict(order=order, tmax=tmax, nts=nts, offs=offs, m_total=m_total,
                n0=n0, xcols=xcols, hcols=hcols, xspan=xspan,
                xspan_pad=xspan_pad, buf_cols=buf_cols)


def _att_chunks(sch):
    chunks, cur, w = [], [], 0
    for t in range(sch["tmax"]):
        n = int(sch["nts"][t])
        if w + n > 512 and cur:
            chunks.append(cur)
            cur, w = [], 0
        cur.append(t)
        w += n
    if cur:
        chunks.append(cur)
    return chunks


# --------------------------------------------------------------------------
def _build(sch):
    nc = bacc.Bacc("TRN2", target_bir_lowering=False, debug=False,
                   num_devices=NCORES)
    tmax, nts = sch["tmax"], sch["nts"]
    hcols, xcols = sch["hcols"], sch["xcols"]
    n0, buf_cols, xspan_pad = sch["n0"], sch["buf_cols"], sch["xspan_pad"]
    nsc = xspan_pad // (128 * GCH)
    chunks = _att_chunks(sch)

    xgrid = nc.dram_tensor("xgrid", [64, buf_cols], F32, kind="ExternalInput")
    qTin = nc.dram_tensor("qTin", [64, BC], F32, kind="ExternalInput")
    spTin = nc.dram_tensor("spTin", [2, 128, BC], F32, kind="ExternalInput")
    densT = nc.dram_tensor("densT", [DL, BC], F32, kind="ExternalInput")
    maskA = nc.dram_tensor("maskA", [128, T], F32, kind="ExternalInput")
    maskB = nc.dram_tensor("maskB", [128, T], F32, kind="ExternalInput")
    w_rz_g = nc.dram_tensor("w_rz_g", [128, 128], F32, kind="ExternalInput")
    w_n_g = nc.dram_tensor("w_n_g", [128, 128], F32, kind="ExternalInput")
    w_rz_a = nc.dram_tensor("w_rz_a", [128, 128], F32, kind="ExternalInput")
    w_n_a = nc.dram_tensor("w_n_a", [128, 128], F32, kind="ExternalInput")
    gvecs = nc.dram_tensor("gvecs", [128, 6], F32, kind="ExternalInput")
    w_att = nc.dram_tensor("w_att", [128, 3 * D], F32, kind="ExternalInput")
    w2_att = nc.dram_tensor("w2_att", [D, 16], F32, kind="ExternalInput")
    w3_att = nc.dram_tensor("w3_att", [16, 1], F32, kind="ExternalInput")
    attb = nc.dram_tensor("attb", [64, 2], F32, kind="ExternalInput")
    dnn_w1 = nc.dram_tensor("dnn_w1", [384, 256], F32, kind="ExternalInput")
    dnn_w2 = nc.dram_tensor("dnn_w2", [256, 128], F32, kind="ExternalInput")
    out_w = nc.dram_tensor("out_w", [128, 1], F32, kind="ExternalInput")
    bn_g = nc.dram_tensor("bn_g", [128, 6], F32, kind="ExternalInput")
    dnn_b = nc.dram_tensor("dnn_b", [128, 3], F32, kind="ExternalInput")
    out_bias = nc.dram_tensor("out_bias", [1, 1], F32, kind="ExternalInput")
    out = nc.dram_tensor("out", [1, BC], F32, kind="ExternalOutput")
    import os
    KDEBUG = bool(os.environ.get("KDEBUG"))
    if KDEBUG:
        dbg_x = nc.dram_tensor("dbg_x", [64, 1024], F32, kind="ExternalOutput")
        dbg_h = nc.dram_tensor("dbg_h", [64, 1024], F32, kind="ExternalOutput")
        dbg_q = nc.dram_tensor("dbg_q", [64, BC], F32, kind="ExternalOutput")
        dbg_sp = nc.dram_tensor("dbg_sp", [128, BC], F32, kind="ExternalOutput")
        dbg_att = nc.dram_tensor("dbg_att", [128, 256], F32, kind="ExternalOutput")
        dbg_hist = nc.dram_tensor("dbg_hist", [64, BC], F32, kind="ExternalOutput")
        dbg_st = nc.dram_tensor("dbg_st", [128, 6], F32, kind="ExternalOutput")

    with tile.TileContext(nc) as tc:
        with tc.tile_pool(name="big", bufs=1) as big, \
             tc.tile_pool(name="w", bufs=1) as w, \
             tc.tile_pool(name="stage", bufs=2) as stage, \
             tc.tile_pool(name="idx", bufs=2) as idxp, \
             tc.tile_pool(name="s", bufs=2) as sp, \
             tc.tile_pool(name="s2", bufs=2) as sp2, \
             tc.tile_pool(name="dram", bufs=1, space="DRAM") as dramp:

            BUF = big.tile([128, buf_cols], F32)
            RH2 = big.tile([128, BC], F32)
            qT = big.tile([128, BC], F32)
            spT = [big.tile([128, BC], F32, tag=f"spT{i}", name=f"spT{i}") for i in range(2)]
            attB = [big.tile([128, 256], F32, tag=f"attB{i}", name=f"attB{i}") for i in range(2)]
            attT = [big.tile([128, 256], F32, tag=f"attT{i}", name=f"attT{i}") for i in range(2)]

            ident = w.tile([128, 128], F32)
            make_identity(nc, ident[:])
            ones1 = w.tile([1, 64], F32)
            nc.vector.memset(ones1[:], 1.0)
            wrzg = w.tile([128, 128], F32)
            wng = w.tile([128, 128], F32)
            wrza = w.tile([128, 128], F32)
            wna = w.tile([128, 128], F32)
            gv = w.tile([128, 6], F32)
            watt = w.tile([128, 3 * D], F32)
            w2a = w.tile([D, 16], F32)
            w3a = w.tile([16, 1], F32)
            ab = w.tile([64, 2], F32)
            for dst, src in ((wrzg, w_rz_g), (wng, w_n_g), (wrza, w_rz_a),
                             (wna, w_n_a), (gv, gvecs), (watt, w_att),
                             (w2a, w2_att), (w3a, w3_att), (ab, attb)):
                nc.sync.dma_start(out=dst[:], in_=src[:])

            nc.vector.memset(BUF[64:128, 0:n0], 0.0)

            # ---------------- load pre-gathered activations -----------------
            nc.sync.dma_start(out=BUF[0:64, :], in_=xgrid[:])
            nc.sync.dma_start(out=qT[64:128, :], in_=qTin[:])
            nc.sync.dma_start(out=spT[0][:], in_=spTin[0, :, :])
            nc.sync.dma_start(out=spT[1][:], in_=spTin[1, :, :])

            # ---------------- scan step ---------------------------------
            def scan_step(pool, t, rhs_buf, rhs_col, wrz, wn, vo, out_buf,
                          out_col, att_rhs=None):
                n = int(nts[t])
                pA = pool.tile([128, 256], F32, tag="pA")
                pB = pool.tile([128, 256], F32, tag="pB")
                rhs = rhs_buf[:, rhs_col:rhs_col + n]
                nc.tensor.matmul(out=pA[:, 0:n], lhsT=wrz[:], rhs=rhs,
                                 start=True, stop=True)
                nc.tensor.matmul(out=pB[:, 0:n], lhsT=wn[:], rhs=rhs,
                                 start=True, stop=True)
                srz = sp.tile([128, 256], F32, tag="srz")
                nc.scalar.activation(out=srz[:, 0:n], in_=pA[:, 0:n],
                                     func=AF.Sigmoid,
                                     bias=gv[:, vo:vo + 1], scale=1.0)
                t1 = sp.tile([128, 256], F32, tag="t1")
                nc.vector.scalar_tensor_tensor(
                    out=t1[64:128, 0:n], in0=pB[64:128, 0:n],
                    scalar=gv[64:128, vo + 1:vo + 2],
                    in1=srz[64:128, 0:n], op0=OP.add, op1=OP.mult)
                t2 = sp.tile([128, 256], F32, tag="t2")
                nc.vector.tensor_tensor(out=t2[64:128, 0:n],
                                        in0=t1[64:128, 0:n],
                                        in1=pB[0:64, 0:n], op=OP.add)
                nt = sp.tile([128, 256], F32, tag="nt")
                nc.scalar.activation(out=nt[64:128, 0:n], in_=t2[64:128, 0:n],
                                     func=AF.Tanh,
                                     bias=gv[64:128, vo + 2:vo + 3], scale=1.0)
                pD = pool.tile([64, 256], F32, tag="pD")
                h_prev = rhs_buf[64:128, rhs_col:rhs_col + n]
                et = sp2.tile([128, 256], F32, tag="et")
                if att_rhs is None:
                    # GRU: h' = n + z*(h - n)
                    nc.vector.tensor_tensor(out=pD[0:64, 0:n], in0=h_prev,
                                            in1=nt[64:128, 0:n],
                                            op=OP.subtract)
                    nc.vector.tensor_tensor(out=et[64:128, 0:n],
                                            in0=pD[0:64, 0:n],
                                            in1=srz[0:64, 0:n], op=OP.mult)
                    nc.vector.tensor_tensor(
                        out=out_buf[64:128, out_col:out_col + n],
                        in0=et[64:128, 0:n], in1=nt[64:128, 0:n], op=OP.add)
                else:
                    # AUGRU: h' = h + att*z*(n - h)
                    nc.vector.tensor_tensor(out=pD[0:64, 0:n],
                                            in0=nt[64:128, 0:n],
                                            in1=h_prev, op=OP.subtract)
                    pAtt = pool.tile([64, 256], F32, tag="pAtt")
                    nc.tensor.matmul(out=pAtt[:, 0:n], lhsT=ones1[:],
                                     rhs=att_rhs, start=True, stop=True)
                    zt = sp2.tile([128, 256], F32, tag="zt")
                    nc.vector.tensor_tensor(out=zt[0:64, 0:n],
                                            in0=pAtt[0:64, 0:n],
                                            in1=srz[0:64, 0:n], op=OP.mult)
                    nc.vector.tensor_tensor(out=et[64:128, 0:n],
                                            in0=pD[0:64, 0:n],
                                            in1=zt[0:64, 0:n], op=OP.mult)
                    nc.vector.tensor_tensor(
                        out=out_buf[64:128, out_col:out_col + n],
                        in0=et[64:128, 0:n],
                        in1=rhs_buf[64:128, rhs_col:rhs_col + n], op=OP.add)

            # ---------------- GRU scan ----------------------------------
            with tc.tile_pool(name="sps", bufs=2, space="PSUM") as sps:
                for t in range(tmax):
                    scan_step(sps, t, BUF, int(xcols[t]), wrzg, wng, 0,
                              BUF, int(hcols[t]))

            # ---------------- attention ---------------------------------
            with tc.tile_pool(name="apsB", bufs=1, space="PSUM") as apsB, \
                 tc.tile_pool(name="aps", bufs=2, space="PSUM") as aps:
                psB = [apsB.tile([128, T], F32, tag=f"psB{i}", name=f"psB{i}") for i in range(2)]
                nc.vector.memset(psB[0][:], 0.0)
                nc.vector.memset(psB[1][:], 0.0)

                for ch in chunks:
                    wch = int(sum(int(nts[t]) for t in ch))
                    qk = sp.tile([128, 512], F32, tag="qk")
                    col = 0
                    for t in ch:
                        n = int(nts[t])
                        hc = int(hcols[t])
                        nc.vector.tensor_tensor(
                            out=qk[64:128, col:col + n],
                            in0=BUF[64:128, hc:hc + n],
                            in1=qT[64:128, 0:n], op=OP.mult)
                        col += n
                    pL1 = aps.tile([64, 512], F32, tag="pL1")
                    col = 0
                    for t in ch:
                        n = int(nts[t])
                        hc = int(hcols[t])
                        nc.tensor.matmul(out=pL1[:, col:col + n],
                                         lhsT=watt[64:128, 0:64],
                                         rhs=BUF[64:128, hc:hc + n],
                                         start=True, stop=False)
                        nc.tensor.matmul(out=pL1[:, col:col + n],
                                         lhsT=watt[64:128, 64:128],
                                         rhs=qk[64:128, col:col + n],
                                         start=False, stop=False)
                        nc.tensor.matmul(out=pL1[:, col:col + n],
                                         lhsT=watt[64:128, 128:192],
                                         rhs=qT[64:128, 0:n],
                                         start=False, stop=True)
                        col += n
                    h1 = sp.tile([64, 512], F32, tag="h1")
                    nc.scalar.activation(out=h1[:, 0:wch], in_=pL1[:, 0:wch],
                                         func=AF.Relu, bias=ab[:, 0:1],
                                         scale=1.0)
                    pL2 = aps.tile([16, 512], F32, tag="pL2")
                    nc.tensor.matmul(out=pL2[:, 0:wch], lhsT=w2a[:],
                                     rhs=h1[:, 0:wch], start=True, stop=True)
                    h2 = sp.tile([16, 512], F32, tag="h2")
                    nc.scalar.activation(out=h2[:, 0:wch], in_=pL2[:, 0:wch],
                                         func=AF.Relu, bias=ab[0:16, 1:2],
                                         scale=1.0)
                    col = 0
                    for t in ch:
                        n = int(nts[t])
                        for piece in range(2):
                            lo = piece * 128
                            if lo >= n:
                                break
                            pw = min(128, n - lo)
                            nc.tensor.matmul(
                                out=psB[piece][0:pw, t:t + 1],
                                lhsT=h2[:, col + lo:col + lo + pw],
                                rhs=w3a[:], start=True, stop=True)
                        col += n

                # softmax (batch-major)
                for i, mk in ((0, maskA), (1, maskB)):
                    msk = sp.tile([128, T], F32, tag="msk")
                    nc.sync.dma_start(out=msk[:], in_=mk[:])
                    sc_t = sp.tile([128, T], F32, tag="sct")
                    nc.vector.tensor_tensor(out=sc_t[:], in0=psB[i][:],
                                            in1=msk[:], op=OP.add)
                    mx = sp.tile([128, 1], F32, tag="mx")
                    nc.vector.tensor_reduce(out=mx[:], in_=sc_t[:],
                                            axis=AX.X, op=OP.max)
                    nmx = sp.tile([128, 1], F32, tag="nmx")
                    nc.vector.tensor_scalar_mul(nmx[:], mx[:], -1.0)
                    ex = sp.tile([128, 256], F32, tag="ex")
                    nc.vector.memset(ex[:], 0.0)
                    nc.scalar.activation(out=ex[:, 0:T], in_=sc_t[:],
                                         func=AF.Exp, bias=nmx[:], scale=1.0)
                    sm = sp.tile([128, 1], F32, tag="sm")
                    nc.vector.tensor_reduce(out=sm[:], in_=ex[:, 0:T],
                                            axis=AX.X, op=OP.add)
                    rs = sp.tile([128, 1], F32, tag="rs")
                    nc.vector.reciprocal(out=rs[:], in_=sm[:])
                    nc.vector.memset(attB[i][:], 0.0)
                    nc.vector.tensor_scalar(
                        out=attB[i][:, 0:T], in0=ex[:, 0:T], scalar1=rs[:],
                        scalar2=None, op0=OP.mult)

                # transpose attB -> attT (rows = t, cols = r)
                for th in range(2):
                    tw = 128 if th == 0 else T - 128
                    for rh in range(2):
                        pat = aps.tile([128, 128], F32, tag="pAT")
                        nc.tensor.transpose(
                            out=pat[0:tw, :],
                            in_=attB[rh][:, th * 128:th * 128 + tw],
                            identity=ident[:])
                        nc.vector.tensor_copy(
                            out=attT[th][0:tw, rh * 128:(rh + 1) * 128],
                            in_=pat[0:tw, :])

            # ---------------- AUGRU scan --------------------------------
            nc.vector.memset(RH2[:], 0.0)
            with tc.tile_pool(name="aups", bufs=2, space="PSUM") as aups, \
                 tc.tile_pool(name="strp", bufs=2) as strp:
                nstrip = (tmax + 7) // 8
                for s in range(nstrip):
                    t0 = s * 8
                    t1s = min(t0 + 8, tmax)
                    rows = t1s - t0
                    strip = strp.tile([1, 8 * 256], F32, tag="strip")
                    th = t0 // 128
                    r0 = t0 - th * 128
                    nc.sync.dma_start(
                        out=strip[0:1, 0:rows * 256].rearrange(
                            "o (t r) -> o t r", t=rows),
                        in_=attT[th][r0:r0 + rows, :])
                    for t in range(t0, t1s):
                        n = int(nts[t])
                        hc = int(hcols[t])
                        nc.gpsimd.tensor_copy(out=RH2[0:64, 0:n],
                                              in_=BUF[64:128, hc:hc + n])
                        arhs = strip[0:1, (t - t0) * 256:(t - t0) * 256 + n]
                        scan_step(aups, t, RH2, 0, wrza, wna, 3, RH2, 0,
                                  att_rhs=arhs)

            # ---------------- DNN head ----------------------------------
            with tc.tile_pool(name="mps", bufs=2, space="PSUM") as mps:
                densTt = big.tile([128, BC], F32, tag="densTt")
                nc.vector.memset(densTt[:], 0.0)
                nc.sync.dma_start(out=densTt[0:DL, :], in_=densT[:])
                nc.vector.tensor_copy(out=densTt[64:128, :],
                                      in_=RH2[64:128, :])

                groups = [spT[0], spT[1], densTt]
                gwidth = [128, 128, 128]
                stats = sp.tile([128, 6], F32, tag="stats")
                nc.vector.memset(stats[:], 0.0)
                scratch = sp.tile([128, BC], F32, tag="scr")
                for gi_, (g, wd) in enumerate(zip(groups, gwidth)):
                    nc.vector.tensor_reduce(out=stats[0:wd, gi_:gi_ + 1],
                                            in_=g[0:wd, :], axis=AX.X,
                                            op=OP.add)
                    nc.vector.scalar_tensor_tensor(
                        out=scratch[0:wd, :], in0=g[0:wd, :], scalar=0.0,
                        in1=g[0:wd, :], op0=OP.add, op1=OP.mult,
                        accum_out=stats[0:wd, 3 + gi_:4 + gi_])

                cc_in = dramp.tile([128, 6], F32)
                cc_out = dramp.tile([128, 6], F32)
                nc.sync.dma_start(out=cc_in[:], in_=stats[:])
                nc.gpsimd.collective_compute(
                    "AllReduce", OP.add,
                    replica_groups=[list(range(NCORES))],
                    ins=[cc_in.opt()], outs=[cc_out.opt()])
                gstats = sp.tile([128, 6], F32, tag="gstats")
                nc.sync.dma_start(out=gstats[:], in_=cc_out[:])

                bn_gt = w.tile([128, 6], F32)
                nc.sync.dma_start(out=bn_gt[:], in_=bn_g[:])
                mu = sp.tile([128, 3], F32, tag="mu")
                nc.vector.tensor_scalar_mul(mu[:], gstats[:, 0:3], 1.0 / B)
                ex2 = sp.tile([128, 3], F32, tag="ex2")
                nc.vector.tensor_scalar_mul(ex2[:], gstats[:, 3:6], 1.0 / B)
                var = sp.tile([128, 3], F32, tag="var")
                nc.vector.tensor_tensor(out=var[:], in0=mu[:], in1=mu[:],
                                        op=OP.mult)
                nc.vector.tensor_tensor(out=var[:], in0=ex2[:], in1=var[:],
                                        op=OP.subtract)
                epst = sp.tile([128, 1], F32, tag="epst")
                nc.vector.memset(epst[:], 1e-5)
                sdv = sp.tile([128, 3], F32, tag="sdv")
                nc.scalar.activation(out=sdv[:], in_=var[:], func=AF.Sqrt,
                                     bias=epst[:], scale=1.0)
                rst = sp.tile([128, 3], F32, tag="rst")
                nc.vector.reciprocal(out=rst[:], in_=sdv[:])
                scl = sp.tile([128, 3], F32, tag="scl")
                nc.vector.tensor_tensor(out=scl[:], in0=bn_gt[:, 0:3],
                                        in1=rst[:], op=OP.mult)
                shf = sp.tile([128, 3], F32, tag="shf")
                nc.vector.tensor_tensor(out=shf[:], in0=mu[:], in1=scl[:],
                                        op=OP.mult)
                nc.vector.tensor_tensor(out=shf[:], in0=bn_gt[:, 3:6],
                                        in1=shf[:], op=OP.subtract)

                for gi_, (g, wd) in enumerate(zip(groups, gwidth)):
                    nc.vector.tensor_scalar(
                        out=g[0:wd, :], in0=g[0:wd, :],
                        scalar1=scl[0:wd, gi_:gi_ + 1],
                        scalar2=shf[0:wd, gi_:gi_ + 1],
                        op0=OP.mult, op1=OP.add)

                w1t = [w.tile([128, 256], F32, tag=f"w1t{i}", name=f"w1t{i}") for i in range(3)]
                for gi_, wt in enumerate(w1t):
                    nc.sync.dma_start(
                        out=wt[:], in_=dnn_w1[gi_ * 128:(gi_ + 1) * 128, :])
                w2t = [w.tile([128, 128], F32, tag=f"w2t{i}", name=f"w2t{i}") for i in range(2)]
                for gi_, wt in enumerate(w2t):
                    nc.sync.dma_start(out=wt[:],
                                      in_=dnn_w2[gi_ * 128:(gi_ + 1) * 128, :])
                owt = w.tile([128, 1], F32)
                nc.sync.dma_start(out=owt[:], in_=out_w[:])
                dbt = w.tile([128, 3], F32)
                nc.sync.dma_start(out=dbt[:], in_=dnn_b[:])
                obt = w.tile([1, 1], F32)
                nc.sync.dma_start(out=obt[:], in_=out_bias[:])

                h1d = [sp.tile([128, BC], F32, tag=f"h1d{i}", name=f"h1d{i}") for i in range(2)]
                for mh in range(2):
                    pm = mps.tile([128, BC], F32, tag="pm1")
                    for gi_, (g, wd) in enumerate(zip(groups, gwidth)):
                        nc.tensor.matmul(
                            out=pm[:],
                            lhsT=w1t[gi_][0:wd, mh * 128:(mh + 1) * 128],
                            rhs=g[0:wd, :], start=(gi_ == 0), stop=(gi_ == 2))
                    nc.scalar.activation(out=h1d[mh][:], in_=pm[:],
                                         func=AF.Relu,
                                         bias=dbt[:, mh:mh + 1], scale=1.0)
                pm2 = mps.tile([128, BC], F32, tag="pm2")
                for mh in range(2):
                    nc.tensor.matmul(out=pm2[:], lhsT=w2t[mh][:],
                                     rhs=h1d[mh][:], start=(mh == 0),
                                     stop=(mh == 1))
                h2d = sp.tile([128, BC], F32, tag="h2d")
                nc.scalar.activation(out=h2d[:], in_=pm2[:], func=AF.Relu,
                                     bias=dbt[:, 2:3], scale=1.0)
                if KDEBUG:
                    nc.sync.dma_start(out=dbg_x[:], in_=BUF[0:64, 0:1024])
                    nc.sync.dma_start(out=dbg_h[:], in_=BUF[64:128, n0:n0 + 1024])
                    nc.sync.dma_start(out=dbg_q[:], in_=qT[64:128, :])
                    nc.sync.dma_start(out=dbg_sp[:], in_=spT[0][:])
                    nc.sync.dma_start(out=dbg_att[:], in_=attB[0][:])
                    nc.sync.dma_start(out=dbg_hist[:], in_=RH2[64:128, :])
                    nc.sync.dma_start(out=dbg_st[:], in_=gstats[:])
                pmo = mps.tile([1, BC], F32, tag="pmo")
                nc.tensor.matmul(out=pmo[:], lhsT=owt[:], rhs=h2d[:],
                                 start=True, stop=True)
                res = sp.tile([1, BC], F32, tag="res")
                nc.vector.tensor_scalar(
                    out=res[:], in0=pmo[:], scalar1=obt[0:1, 0:1],
                    scalar2=None, op0=OP.add)
                nc.sync.dma_start(out=out[:], in_=res[:])

    nc.compile()
    return nc


# --------------------------------------------------------------------------
def _host_prep(inputs, sch):
    lens = np.asarray(inputs["hist_valid_lens"]).astype(np.int64)
    order = sch["order"]
    tmax, nts, xcols = sch["tmax"], sch["nts"], sch["xcols"]
    xspan_pad = sch["xspan_pad"]
    nsc = xspan_pad // (128 * GCH)

    emb = np.ascontiguousarray(np.asarray(inputs["emb"], np.float32))
    hist_item = np.asarray(inputs["hist_item"]).astype(np.int32)
    tgt = np.asarray(inputs["target_item"]).astype(np.int32)
    spf = np.asarray(inputs["sparse_feature"]).astype(np.int32)
    dense = np.asarray(inputs["dense_feature"], np.float32)

    gw = {k: np.asarray(inputs[k], np.float32) for k in
          ("gru_wih", "gru_whh", "gru_bih", "gru_bhh",
           "augru_wih", "augru_whh", "augru_bih", "augru_bhh",
           "att_w1", "att_b1", "att_w2", "att_b2", "att_w3", "att_b3",
           "bn_gamma", "bn_beta", "dnn_w1", "dnn_b1", "dnn_w2", "dnn_b2",
           "out_w", "out_b")}

    def stack_rz(wih, whh):
        m = np.zeros((128, 128), np.float32)
        m[0:64, 0:64] = wih[64:128].T      # z, x-side
        m[64:128, 0:64] = whh[64:128].T    # z, h-side
        m[0:64, 64:128] = wih[0:64].T      # r, x-side
        m[64:128, 64:128] = whh[0:64].T    # r, h-side
        return m

    def block_n(wih, whh):
        m = np.zeros((128, 128), np.float32)
        m[0:64, 0:64] = wih[128:192].T     # i_n (-> M 0:64)
        m[64:128, 64:128] = whh[128:192].T  # h_n (-> M 64:128)
        return m

    def vecs(bih, bhh):
        brz = np.zeros(128, np.float32)
        brz[0:64] = bih[64:128] + bhh[64:128]   # z
        brz[64:128] = bih[0:64] + bhh[0:64]     # r
        bhhn = np.zeros(128, np.float32)
        bhhn[64:128] = bhh[128:192]
        bihn = np.zeros(128, np.float32)
        bihn[64:128] = bih[128:192]
        return brz, bhhn, bihn

    gvecs = np.zeros((128, 6), np.float32)
    gvecs[:, 0], gvecs[:, 1], gvecs[:, 2] = vecs(gw["gru_bih"], gw["gru_bhh"])
    gvecs[:, 3], gvecs[:, 4], gvecs[:, 5] = vecs(gw["augru_bih"],
                                                 gw["augru_bhh"])

    w1 = gw["att_w1"]
    w_att = np.zeros((128, 3 * D), np.float32)
    w_att[64:128, 0:64] = w1[64:128] - w1[128:192]   # k-term
    w_att[64:128, 64:128] = w1[192:256]              # q*k-term
    w_att[64:128, 128:192] = w1[0:64] + w1[128:192]  # q-term
    attb = np.zeros((64, 2), np.float32)
    attb[:, 0] = gw["att_b1"]
    attb[0:16, 1] = gw["att_b2"]

    bn_g = np.zeros((128, 6), np.float32)
    bn_g[:, 0:3] = 1.0
    for g in range(2):
        bn_g[:, g] = gw["bn_gamma"][g * 128:(g + 1) * 128]
        bn_g[:, 3 + g] = gw["bn_beta"][g * 128:(g + 1) * 128]
    bn_g[0:DL, 2] = gw["bn_gamma"][256:272]
    bn_g[0:DL, 5] = gw["bn_beta"][256:272]
    bn_g[64:128, 2] = gw["bn_gamma"][272:336]
    bn_g[64:128, 5] = gw["bn_beta"][272:336]
    dnn_w1p = np.zeros((384, 256), np.float32)
    dnn_w1p[0:256] = gw["dnn_w1"][0:256]
    dnn_w1p[256:272] = gw["dnn_w1"][256:272]
    dnn_w1p[320:384] = gw["dnn_w1"][272:336]
    dnn_b = np.zeros((128, 3), np.float32)
    dnn_b[:, 0] = gw["dnn_b1"][0:128]
    dnn_b[:, 1] = gw["dnn_b1"][128:256]
    dnn_b[:, 2] = gw["dnn_b2"]

    shared = dict(
        w_rz_g=stack_rz(gw["gru_wih"], gw["gru_whh"]),
        w_n_g=block_n(gw["gru_wih"], gw["gru_whh"]),
        w_rz_a=stack_rz(gw["augru_wih"], gw["augru_whh"]),
        w_n_a=block_n(gw["augru_wih"], gw["augru_whh"]),
        gvecs=gvecs, w_att=w_att, w2_att=gw["att_w2"], w3_att=gw["att_w3"],
        attb=attb, dnn_w1=dnn_w1p, dnn_w2=gw["dnn_w2"],
        out_w=gw["out_w"], bn_g=bn_g, dnn_b=dnn_b,
        out_bias=gw["out_b"].reshape(1, 1).astype(np.float32))

    dcol_t = np.zeros(xspan_pad, np.int64)
    dcol_r = np.zeros(xspan_pad, np.int64)
    dcol_valid = np.zeros(xspan_pad, bool)
    for t in range(tmax):
        c0, n = int(xcols[t]), int(nts[t])
        dcol_t[c0:c0 + n] = t
        dcol_r[c0:c0 + n] = np.arange(n)
        dcol_valid[c0:c0 + n] = True

    buf_cols = sch["buf_cols"]
    dval = np.nonzero(dcol_valid[:buf_cols])[0]
    tt_ = dcol_t[dval]
    rr_ = dcol_r[dval]

    in_maps = []
    for c in range(NCORES):
        rows = order[c::NCORES]
        ids = hist_item[rows[rr_], tt_, :]            # [nval, 2]
        xgrid = np.zeros((64, buf_cols), np.float32)
        xgrid[0:32, dval] = emb[ids[:, 0]].T
        xgrid[32:64, dval] = emb[ids[:, 1]].T

        qTin = emb[tgt[rows]].reshape(BC, 64).T.astype(np.float32)
        qTin = np.ascontiguousarray(qTin)

        spfull = emb[spf[rows]].reshape(BC, 256).T    # [256 feat, 256 rows]
        spTin = np.ascontiguousarray(
            spfull.reshape(2, 128, BC).astype(np.float32))

        densT = np.ascontiguousarray(dense[rows, :].T)
        core_lens = lens[rows]
        msk = np.where(np.arange(T)[None, :] < core_lens[:, None],
                       0.0, -1e9).astype(np.float32)

        im = dict(shared)
        im.update(xgrid=xgrid, qTin=qTin, spTin=spTin, densT=densT,
                  maskA=np.ascontiguousarray(msk[0:128]),
                  maskB=np.ascontiguousarray(msk[128:256]))
        in_maps.append(im)
    return in_maps, order


_CACHE = {}


def kernel(**inputs):
    lens = np.asarray(inputs["hist_valid_lens"]).astype(np.int64)
    key = hashlib.sha1(lens.tobytes()).hexdigest()
    sch = _make_schedule(lens)
    if key not in _CACHE:
        _CACHE[key] = _build(sch)
    nc = _CACHE[key]
    in_maps, order = _host_prep(inputs, sch)
    import os, time
    trace = bool(os.environ.get("KTRACE"))
    t0 = time.perf_counter()
    res = run_bass_kernel_spmd(nc, in_maps, core_ids=list(range(NCORES)),
                               trace=trace)
    kernel.last_spmd_s = time.perf_counter() - t0
    if trace and res.exec_time_ns is not None:
        print(f"HW exec time: {res.exec_time_ns} ns")
    kernel.last_res = res
    kernel.last_sch = sch
    kernel.last_maps = in_maps
    out = np.zeros((B, 1), np.float32)
    for c in range(NCORES):
        rows = order[c::NCORES]
        out[rows, 0] = res.results[c]["out"][0]
    return out



# revision 11
# speedup vs baseline: 2.3886x; 2.3886x over previous
"""DIEN forward on 8 Trainium2 NeuronCores (Bass/Tile).

Data-parallel with ragged packing; device-side embedding gather.

The axon RPC wall time is dominated by input upload (~55 MB/s tunnel), so
the kernel minimizes bytes shipped per call:
 - The 100k x 32 embedding table is converted to fp16 on the host, row-
   sharded 1/8th per core (0.8 MB each), AllGathered on-device into DRAM,
   and all embedding lookups (history, target, sparse) run as on-device
   indirect-DMA gathers + PE transposes into the feature-major layout.
 - All network weights are packed fp16 into the same sharded blob
   (64 KB/core) and sliced out of the AllGathered copy.
 - Attention masks are built on-device from iota + per-row lengths.
 - Host ships only: blob shard (fp16), gather offsets (int32), dense
   features (f32) -- ~1.1 MB/core vs ~7.6 MB/core for pre-gathered f32.

Compute layout (unchanged from the packed-ragged design):
 - Host sorts batch rows by descending hist_valid_lens, deals them to the 8
   cores round-robin over the sorted order, and builds a shared per-timestep
   active-column schedule N_t (max over cores, padded to x4).  All per-step
   shapes are compile-time constants.
 - Feature-major layout: interests/h at SBUF partitions 64:128 of one packed
   buffer BUF, gathered hist embeddings x^T at partitions 0:64, with x(t)
   stored at the columns of h(t-1) so each GRU step runs K=128-stacked
   matmuls over [x_t ; h_{t-1}] with a single rhs AP.
 - Attention runs over packed ragged columns; L3 scatters scores straight
   into a batch-major [r, t] PSUM via per-region column matmuls; softmax is
   batch-major; att is transposed back (PE) and streamed to the AUGRU as
   [1, N] rows broadcast via K=1 ones-matmuls.
 - BatchNorm (training stats over the full 2048 batch) via a tiny AllReduce.

kernel(**inputs) takes FULL unsharded inputs, returns [B, 1] float32.
"""

import hashlib
import numpy as np

import concourse.bass as bass
import concourse.bacc as bacc
import concourse.tile as tile
from concourse import mybir
from concourse.bass_utils import run_bass_kernel_spmd
from concourse.masks import make_identity

F32 = mybir.dt.float32
F16 = mybir.dt.float16
I32 = mybir.dt.int32
AF = mybir.ActivationFunctionType
OP = mybir.AluOpType
AX = mybir.AxisListType

B, T, E, NF, SL, DL, VOCAB = 2048, 200, 32, 2, 8, 16, 100000
D = E * NF          # 64
NCORES = 8
BC = B // NCORES    # 256

# sharded fp16 blob: [emb rows | weight row], per core
VSH = VOCAB // NCORES            # 12500 emb rows per shard
EMB_ELEMS = VSH * E              # 400000
WROW = 32768                     # weight elems per shard row
SHW = EMB_ELEMS + WROW           # 432768 fp16 elems per shard
SH_F = SHW // 128                # 3381 (shard shipped as [128, SH_F])
WB = EMB_ELEMS                   # weight-area base inside a row
ZOFF = (7 * SHW + WB) // E       # guaranteed-zero blob row (row 7 pad area)
GRP = 16                         # gather chunks (128 cols) per indirect DMA

# weight-area element offsets (row 6 smalls)
O_GV = 24576
O_W2A = O_GV + 768
O_W3A = O_W2A + 1024
O_AB = O_W3A + 16
O_BNG = O_AB + 128
O_DBT = O_BNG + 768
O_OWT = O_DBT + 384
O_OBT = O_OWT + 128


# --------------------------------------------------------------------------
def _make_schedule(lens):
    order = np.argsort(-lens, kind="stable")
    core_lens = lens[order].reshape(-1, NCORES)       # [BC, 8]
    tmax = int(lens.max())
    nts = []
    for t in range(tmax):
        n = int((core_lens > t).sum(axis=0).max())
        n = min(BC, ((n + 3) // 4) * 4)
        nts.append(max(n, 4))
    nts = np.array(nts, np.int32)
    offs = np.zeros(tmax + 1, np.int64)
    offs[1:] = np.cumsum(nts)
    m_total = int(offs[tmax])
    n0 = int(nts[0])
    xcols = np.zeros(tmax, np.int64)
    xcols[1:] = n0 + offs[:tmax - 1]                  # x(t) at h(t-1) cols
    hcols = n0 + offs[:tmax]
    xspan = int(n0 + offs[tmax - 1]) if tmax > 1 else int(nts[0])
    xspan = max(xspan, int(xcols[tmax - 1] + nts[tmax - 1]))
    nch = (xspan + 127) // 128                        # 128-col gather chunks
    buf_cols = max(n0 + m_total, nch * 128 + 128)
    ni = 2 * nch + 22                                 # idx tensor width
    return dict(order=order, tmax=tmax, nts=nts, offs=offs, m_total=m_total,
                n0=n0, xcols=xcols, hcols=hcols, xspan=xspan,
                nch=nch, ni=ni, buf_cols=buf_cols)


def _att_chunks(sch):
    chunks, cur, w = [], [], 0
    for t in range(sch["tmax"]):
        n = int(sch["nts"][t])
        if w + n > 512 and cur:
            chunks.append(cur)
            cur, w = [], 0
        cur.append(t)
        w += n
    if cur:
        chunks.append(cur)
    return chunks


# --------------------------------------------------------------------------
def _build(sch):
    nc = bacc.Bacc("TRN2", target_bir_lowering=False, debug=False,
                   num_devices=NCORES)
    tmax, nts = sch["tmax"], sch["nts"]
    hcols, xcols = sch["hcols"], sch["xcols"]
    n0, buf_cols = sch["n0"], sch["buf_cols"]
    nch, ni = sch["nch"], sch["ni"]
    chunks = _att_chunks(sch)

    shard = nc.dram_tensor("shard", [128, SH_F], F16, kind="ExternalInput")
    idx = nc.dram_tensor("idx", [128, ni], I32, kind="ExternalInput")
    densT = nc.dram_tensor("densT", [DL, BC], F32, kind="ExternalInput")
    out = nc.dram_tensor("out", [1, BC], F32, kind="ExternalOutput")
    import os
    KDEBUG = bool(os.environ.get("KDEBUG"))
    if KDEBUG:
        dbg_gath = nc.dram_tensor("dbg_gath", [8, 512], F32,
                                  kind="ExternalOutput")
        dbg_x = nc.dram_tensor("dbg_x", [64, 1024], F32, kind="ExternalOutput")
        dbg_q = nc.dram_tensor("dbg_q", [64, BC], F32, kind="ExternalOutput")
        dbg_sp = nc.dram_tensor("dbg_sp", [128, BC], F32,
                                kind="ExternalOutput")
        dbg_msk = nc.dram_tensor("dbg_msk", [128, 2 * T], F32,
                                 kind="ExternalOutput")
        dbg_h = nc.dram_tensor("dbg_h", [64, 512], F32, kind="ExternalOutput")
        dbg_wt = nc.dram_tensor("dbg_wt", [128, 256], F32,
                                kind="ExternalOutput")
        dbg_att = nc.dram_tensor("dbg_att", [128, 256], F32,
                                 kind="ExternalOutput")

    with tile.TileContext(nc) as tc:
        with tc.tile_pool(name="big", bufs=1) as big, \
             tc.tile_pool(name="w", bufs=1) as w, \
             tc.tile_pool(name="stage", bufs=2) as stage, \
             tc.tile_pool(name="gp", bufs=2) as gpool, \
             tc.tile_pool(name="idx", bufs=1) as idxp, \
             tc.tile_pool(name="s", bufs=2) as sp, \
             tc.tile_pool(name="s2", bufs=2) as sp2, \
             tc.tile_pool(name="dram", bufs=1, space="DRAM") as dramp:

            BUF = big.tile([128, buf_cols], F32)
            RH2 = big.tile([128, BC], F32)
            qT = big.tile([128, BC], F32)
            spT = [big.tile([128, BC], F32, tag=f"spT{i}", name=f"spT{i}")
                   for i in range(2)]
            attB = [big.tile([128, 256], F32, tag=f"attB{i}", name=f"attB{i}")
                    for i in range(2)]
            attT = [big.tile([128, 256], F32, tag=f"attT{i}", name=f"attT{i}")
                    for i in range(2)]

            ident = w.tile([128, 128], F32)
            make_identity(nc, ident[:])
            ident16 = w.tile([128, 128], F16)
            make_identity(nc, ident16[:])
            ones1 = w.tile([1, 64], F32)
            nc.vector.memset(ones1[:], 1.0)

            # ---------------- shard upload + AllGather -------------------
            stg = w.tile([128, SH_F], F16)
            nc.sync.dma_start(out=stg[:], in_=shard[:])
            cc_in = dramp.tile([128, SH_F], F16)
            nc.sync.dma_start(out=cc_in[:], in_=stg[:])
            gath = dramp.tile([NCORES, SHW], F16)
            nc.gpsimd.collective_compute(
                "AllGather", OP.bypass,
                replica_groups=[list(range(NCORES))],
                ins=[cc_in[:]], outs=[gath[:]])

            idxT = idxp.tile([128, ni], I32)
            nc.sync.dma_start(out=idxT[:], in_=idx[:])

            # ---------------- weight slices from blob --------------------
            def wload(shape, row, off, tag):
                p, f = shape
                st = stage.tile(shape, F16, tag=f"st_{tag}", name=f"st_{tag}")
                nc.sync.dma_start(
                    out=st[:],
                    in_=gath[row:row + 1, WB + off:WB + off + p * f].rearrange(
                        "a (p f) -> (a p) f", p=p))
                ft = w.tile(shape, F32, tag=f"w_{tag}", name=f"w_{tag}")
                nc.vector.tensor_copy(out=ft[:], in_=st[:])
                return ft

            w1t = [wload([128, 256], i, 0, f"w1t{i}") for i in range(3)]
            w2all = wload([128, 256], 3, 0, "w2all")
            wrzng = wload([128, 256], 4, 0, "wrzng")
            wrzna = wload([128, 256], 5, 0, "wrzna")
            watt = wload([128, 192], 6, 0, "watt")
            gv = wload([128, 6], 6, O_GV, "gv")
            w2a = wload([64, 16], 6, O_W2A, "w2a")
            w3a = wload([16, 1], 6, O_W3A, "w3a")
            ab = wload([64, 2], 6, O_AB, "ab")
            bn_gt = wload([128, 6], 6, O_BNG, "bng")
            dbt = wload([128, 3], 6, O_DBT, "dbt")
            owt = wload([128, 1], 6, O_OWT, "owt")
            obt = wload([1, 1], 6, O_OBT, "obt")

            nc.vector.memset(BUF[64:128, 0:n0], 0.0)

            if KDEBUG:
                dg16 = w.tile([8, 512], F16, tag="dg16")
                for r in range(8):
                    nc.sync.dma_start(out=dg16[r:r + 1, :],
                                      in_=gath[r:r + 1, 0:512])
                dg32 = w.tile([8, 512], F32, tag="dg32")
                nc.vector.tensor_copy(out=dg32[:], in_=dg16[:])
                nc.sync.dma_start(out=dbg_gath[:], in_=dg32[:])
                nc.sync.dma_start(out=dbg_wt[:], in_=wrzng[:])

            # ---------------- device-side embedding gathers --------------
            # HW SWDGE only handles the canonical indirect shape: offset AP
            # [128, 1], dest [128, rowlen] (one gathered row per partition
            # per instruction).  Offsets are row indices into the blob
            # viewed as [(8*SHW)//E, E].
            in_rows = gath[:, :].rearrange("r (q e) -> (r q) e", e=E)

            def gather1(dst, col):
                nc.gpsimd.indirect_dma_start(
                    out=dst, out_offset=None, in_=in_rows,
                    in_offset=bass.IndirectOffsetOnAxis(
                        ap=idxT[:, col:col + 1], axis=0))

            with tc.tile_pool(name="tps", bufs=3, space="PSUM") as tps:
                ngrp = (nch + GRP - 1) // GRP
                for g in range(ngrp):
                    c0 = g * GRP
                    gw = min(GRP, nch - c0)
                    gt = gpool.tile([128, GRP * 64], F16, tag="gt")
                    for c in range(gw):
                        for k in range(2):
                            gather1(gt[:, c * 64 + k * 32:c * 64 + k * 32 + 32],
                                    2 * (c0 + c) + k)
                    for c in range(gw):
                        j = c0 + c
                        pt = tps.tile([64, 128], F16, tag="pt")
                        nc.tensor.transpose(out=pt[:],
                                            in_=gt[:, c * 64:c * 64 + 64],
                                            identity=ident16[:])
                        nc.vector.tensor_copy(
                            out=BUF[0:64, j * 128:j * 128 + 128], in_=pt[:])

                # target-item embeddings -> qT[64:128, :]
                gq = gpool.tile([128, 128], F16, tag="gq")
                for c in range(2):
                    for k in range(2):
                        gather1(gq[:, c * 64 + k * 32:c * 64 + k * 32 + 32],
                                2 * nch + 2 * c + k)
                for c in range(2):
                    pt = tps.tile([64, 128], F16, tag="pt")
                    nc.tensor.transpose(out=pt[:], in_=gq[:, c * 64:c * 64 + 64],
                                        identity=ident16[:])
                    nc.vector.tensor_copy(
                        out=qT[64:128, c * 128:c * 128 + 128], in_=pt[:])

                # sparse-feature embeddings -> spT[0], spT[1]
                gsp = gpool.tile([128, 512], F16, tag="gsp")
                for j in range(16):
                    gather1(gsp[:, j * 32:(j + 1) * 32], 2 * nch + 4 + j)
                for rh in range(2):
                    for gf in range(2):
                        pt2 = tps.tile([128, 128], F16, tag="pt2")
                        base = (rh * 2 + gf) * 128
                        nc.tensor.transpose(out=pt2[:],
                                            in_=gsp[:, base:base + 128],
                                            identity=ident16[:])
                        nc.vector.tensor_copy(
                            out=spT[gf][:, rh * 128:rh * 128 + 128],
                            in_=pt2[:])

            # ---------------- scan step ---------------------------------
            def scan_step(pool, t, rhs_buf, rhs_col, wrz, wn, vo, out_buf,
                          out_col, att_rhs=None):
                n = int(nts[t])
                pA = pool.tile([128, 256], F32, tag="pA")
                pB = pool.tile([128, 256], F32, tag="pB")
                rhs = rhs_buf[:, rhs_col:rhs_col + n]
                nc.tensor.matmul(out=pA[:, 0:n], lhsT=wrz, rhs=rhs,
                                 start=True, stop=True)
                nc.tensor.matmul(out=pB[:, 0:n], lhsT=wn, rhs=rhs,
                                 start=True, stop=True)
                srz = sp.tile([128, 256], F32, tag="srz")
                nc.scalar.activation(out=srz[:, 0:n], in_=pA[:, 0:n],
                                     func=AF.Sigmoid,
                                     bias=gv[:, vo:vo + 1], scale=1.0)
                t1 = sp.tile([128, 256], F32, tag="t1")
                nc.vector.scalar_tensor_tensor(
                    out=t1[64:128, 0:n], in0=pB[64:128, 0:n],
                    scalar=gv[64:128, vo + 1:vo + 2],
                    in1=srz[64:128, 0:n], op0=OP.add, op1=OP.mult)
                t2 = sp.tile([128, 256], F32, tag="t2")
                nc.vector.tensor_tensor(out=t2[64:128, 0:n],
                                        in0=t1[64:128, 0:n],
                                        in1=pB[0:64, 0:n], op=OP.add)
                nt = sp.tile([128, 256], F32, tag="nt")
                nc.scalar.activation(out=nt[64:128, 0:n], in_=t2[64:128, 0:n],
                                     func=AF.Tanh,
                                     bias=gv[64:128, vo + 2:vo + 3], scale=1.0)
                pD = pool.tile([64, 256], F32, tag="pD")
                h_prev = rhs_buf[64:128, rhs_col:rhs_col + n]
                et = sp2.tile([128, 256], F32, tag="et")
                if att_rhs is None:
                    # GRU: h' = n + z*(h - n)
                    nc.vector.tensor_tensor(out=pD[0:64, 0:n], in0=h_prev,
                                            in1=nt[64:128, 0:n],
                                            op=OP.subtract)
                    nc.vector.tensor_tensor(out=et[64:128, 0:n],
                                            in0=pD[0:64, 0:n],
                                            in1=srz[0:64, 0:n], op=OP.mult)
                    nc.vector.tensor_tensor(
                        out=out_buf[64:128, out_col:out_col + n],
                        in0=et[64:128, 0:n], in1=nt[64:128, 0:n], op=OP.add)
                else:
                    # AUGRU: h' = h + att*z*(n - h)
                    nc.vector.tensor_tensor(out=pD[0:64, 0:n],
                                            in0=nt[64:128, 0:n],
                                            in1=h_prev, op=OP.subtract)
                    pAtt = pool.tile([64, 256], F32, tag="pAtt")
                    nc.tensor.matmul(out=pAtt[:, 0:n], lhsT=ones1[:],
                                     rhs=att_rhs, start=True, stop=True)
                    zt = sp2.tile([128, 256], F32, tag="zt")
                    nc.vector.tensor_tensor(out=zt[0:64, 0:n],
                                            in0=pAtt[0:64, 0:n],
                                            in1=srz[0:64, 0:n], op=OP.mult)
                    nc.vector.tensor_tensor(out=et[64:128, 0:n],
                                            in0=pD[0:64, 0:n],
                                            in1=zt[0:64, 0:n], op=OP.mult)
                    nc.vector.tensor_tensor(
                        out=out_buf[64:128, out_col:out_col + n],
                        in0=et[64:128, 0:n],
                        in1=rhs_buf[64:128, rhs_col:rhs_col + n], op=OP.add)

            if KDEBUG:
                nc.sync.dma_start(out=dbg_x[:], in_=BUF[0:64, 0:1024])
                nc.sync.dma_start(out=dbg_q[:], in_=qT[64:128, :])
                nc.sync.dma_start(out=dbg_sp[:], in_=spT[0][:])

            # ---------------- GRU scan ----------------------------------
            with tc.tile_pool(name="sps", bufs=2, space="PSUM") as sps:
                for t in range(tmax):
                    scan_step(sps, t, BUF, int(xcols[t]),
                              wrzng[:, 0:128], wrzng[:, 128:256], 0,
                              BUF, int(hcols[t]))
            if KDEBUG:
                nc.sync.dma_start(out=dbg_h[:], in_=BUF[64:128, n0:n0 + 512])

            # ---------------- attention ---------------------------------
            with tc.tile_pool(name="apsB", bufs=1, space="PSUM") as apsB, \
                 tc.tile_pool(name="aps", bufs=2, space="PSUM") as aps:
                psB = [apsB.tile([128, T], F32, tag=f"psB{i}", name=f"psB{i}")
                       for i in range(2)]
                nc.vector.memset(psB[0][:], 0.0)
                nc.vector.memset(psB[1][:], 0.0)

                for ch in chunks:
                    wch = int(sum(int(nts[t]) for t in ch))
                    qk = sp.tile([128, 512], F32, tag="qk")
                    col = 0
                    for t in ch:
                        n = int(nts[t])
                        hc = int(hcols[t])
                        nc.vector.tensor_tensor(
                            out=qk[64:128, col:col + n],
                            in0=BUF[64:128, hc:hc + n],
                            in1=qT[64:128, 0:n], op=OP.mult)
                        col += n
                    pL1 = aps.tile([64, 512], F32, tag="pL1")
                    col = 0
                    for t in ch:
                        n = int(nts[t])
                        hc = int(hcols[t])
                        nc.tensor.matmul(out=pL1[:, col:col + n],
                                         lhsT=watt[64:128, 0:64],
                                         rhs=BUF[64:128, hc:hc + n],
                                         start=True, stop=False)
                        nc.tensor.matmul(out=pL1[:, col:col + n],
                                         lhsT=watt[64:128, 64:128],
                                         rhs=qk[64:128, col:col + n],
                                         start=False, stop=False)
                        nc.tensor.matmul(out=pL1[:, col:col + n],
                                         lhsT=watt[64:128, 128:192],
                                         rhs=qT[64:128, 0:n],
                                         start=False, stop=True)
                        col += n
                    h1 = sp.tile([64, 512], F32, tag="h1")
                    nc.scalar.activation(out=h1[:, 0:wch], in_=pL1[:, 0:wch],
                                         func=AF.Relu, bias=ab[:, 0:1],
                                         scale=1.0)
                    pL2 = aps.tile([16, 512], F32, tag="pL2")
                    nc.tensor.matmul(out=pL2[:, 0:wch], lhsT=w2a[:],
                                     rhs=h1[:, 0:wch], start=True, stop=True)
                    h2 = sp.tile([16, 512], F32, tag="h2")
                    nc.scalar.activation(out=h2[:, 0:wch], in_=pL2[:, 0:wch],
                                         func=AF.Relu, bias=ab[0:16, 1:2],
                                         scale=1.0)
                    col = 0
                    for t in ch:
                        n = int(nts[t])
                        for piece in range(2):
                            lo = piece * 128
                            if lo >= n:
                                break
                            pw = min(128, n - lo)
                            nc.tensor.matmul(
                                out=psB[piece][0:pw, t:t + 1],
                                lhsT=h2[:, col + lo:col + lo + pw],
                                rhs=w3a[:], start=True, stop=True)
                        col += n

                # masks from iota + lens, then softmax (batch-major)
                ar_i = idxp.tile([128, T], I32)
                nc.gpsimd.iota(out=ar_i[:], pattern=[[1, T]], base=0,
                               channel_multiplier=0)
                arB = sp.tile([128, T], F32, tag="arB")
                nc.vector.tensor_copy(out=arB[:], in_=ar_i[:])
                lensf = sp.tile([128, 2], F32, tag="lensf")
                nc.vector.tensor_copy(out=lensf[:], in_=idxT[:, ni - 2:ni])
                for i in range(2):
                    m1 = sp.tile([128, T], F32, tag="m1")
                    nc.vector.tensor_scalar(
                        out=m1[:], in0=arB[:], scalar1=lensf[:, i:i + 1],
                        scalar2=1.0, op0=OP.subtract, op1=OP.add)
                    msk = sp.tile([128, T], F32, tag="msk")
                    nc.vector.tensor_scalar(
                        out=msk[:], in0=m1[:], scalar1=0.0, scalar2=-1e9,
                        op0=OP.max, op1=OP.mult)
                    if KDEBUG:
                        nc.sync.dma_start(out=dbg_msk[:, i * T:(i + 1) * T],
                                          in_=msk[:])
                    sc_t = sp.tile([128, T], F32, tag="sct")
                    nc.vector.tensor_tensor(out=sc_t[:], in0=psB[i][:],
                                            in1=msk[:], op=OP.add)
                    mx = sp.tile([128, 1], F32, tag="mx")
                    nc.vector.tensor_reduce(out=mx[:], in_=sc_t[:],
                                            axis=AX.X, op=OP.max)
                    nmx = sp.tile([128, 1], F32, tag="nmx")
                    nc.vector.tensor_scalar_mul(nmx[:], mx[:], -1.0)
                    ex = sp.tile([128, 256], F32, tag="ex")
                    nc.vector.memset(ex[:], 0.0)
                    nc.scalar.activation(out=ex[:, 0:T], in_=sc_t[:],
                                         func=AF.Exp, bias=nmx[:], scale=1.0)
                    sm = sp.tile([128, 1], F32, tag="sm")
                    nc.vector.tensor_reduce(out=sm[:], in_=ex[:, 0:T],
                                            axis=AX.X, op=OP.add)
                    rs = sp.tile([128, 1], F32, tag="rs")
                    nc.vector.reciprocal(out=rs[:], in_=sm[:])
                    nc.vector.memset(attB[i][:], 0.0)
                    nc.vector.tensor_scalar(
                        out=attB[i][:, 0:T], in0=ex[:, 0:T], scalar1=rs[:],
                        scalar2=None, op0=OP.mult)
                if KDEBUG:
                    nc.sync.dma_start(out=dbg_att[:], in_=attB[0][:])

                # transpose attB -> attT (rows = t, cols = r)
                for th in range(2):
                    tw = 128 if th == 0 else T - 128
                    for rh in range(2):
                        pat = aps.tile([128, 128], F32, tag="pAT")
                        nc.tensor.transpose(
                            out=pat[0:tw, :],
                            in_=attB[rh][:, th * 128:th * 128 + tw],
                            identity=ident[:])
                        nc.vector.tensor_copy(
                            out=attT[th][0:tw, rh * 128:(rh + 1) * 128],
                            in_=pat[0:tw, :])

            # ---------------- AUGRU scan --------------------------------
            nc.vector.memset(RH2[:], 0.0)
            with tc.tile_pool(name="aups", bufs=2, space="PSUM") as aups, \
                 tc.tile_pool(name="strp", bufs=2) as strp:
                nstrip = (tmax + 7) // 8
                for s in range(nstrip):
                    t0 = s * 8
                    t1s = min(t0 + 8, tmax)
                    rows = t1s - t0
                    strip = strp.tile([1, 8 * 256], F32, tag="strip")
                    th = t0 // 128
                    r0 = t0 - th * 128
                    nc.sync.dma_start(
                        out=strip[0:1, 0:rows * 256].rearrange(
                            "o (t r) -> o t r", t=rows),
                        in_=attT[th][r0:r0 + rows, :])
                    for t in range(t0, t1s):
                        n = int(nts[t])
                        hc = int(hcols[t])
                        nc.gpsimd.tensor_copy(out=RH2[0:64, 0:n],
                                              in_=BUF[64:128, hc:hc + n])
                        arhs = strip[0:1, (t - t0) * 256:(t - t0) * 256 + n]
                        scan_step(aups, t, RH2, 0,
                                  wrzna[:, 0:128], wrzna[:, 128:256], 3,
                                  RH2, 0, att_rhs=arhs)

            # ---------------- DNN head ----------------------------------
            with tc.tile_pool(name="mps", bufs=2, space="PSUM") as mps:
                densTt = big.tile([128, BC], F32, tag="densTt")
                nc.vector.memset(densTt[:], 0.0)
                nc.sync.dma_start(out=densTt[0:DL, :], in_=densT[:])
                nc.vector.tensor_copy(out=densTt[64:128, :],
                                      in_=RH2[64:128, :])

                groups = [spT[0], spT[1], densTt]
                gwidth = [128, 128, 128]
                stats = sp.tile([128, 6], F32, tag="stats")
                nc.vector.memset(stats[:], 0.0)
                scratch = sp.tile([128, BC], F32, tag="scr")
                for gi_, (g, wd) in enumerate(zip(groups, gwidth)):
                    nc.vector.tensor_reduce(out=stats[0:wd, gi_:gi_ + 1],
                                            in_=g[0:wd, :], axis=AX.X,
                                            op=OP.add)
                    nc.vector.scalar_tensor_tensor(
                        out=scratch[0:wd, :], in0=g[0:wd, :], scalar=0.0,
                        in1=g[0:wd, :], op0=OP.add, op1=OP.mult,
                        accum_out=stats[0:wd, 3 + gi_:4 + gi_])

                cc_in2 = dramp.tile([128, 6], F32)
                cc_out2 = dramp.tile([128, 6], F32)
                nc.sync.dma_start(out=cc_in2[:], in_=stats[:])
                nc.gpsimd.collective_compute(
                    "AllReduce", OP.add,
                    replica_groups=[list(range(NCORES))],
                    ins=[cc_in2.opt()], outs=[cc_out2.opt()])
                gstats = sp.tile([128, 6], F32, tag="gstats")
                nc.sync.dma_start(out=gstats[:], in_=cc_out2[:])

                mu = sp.tile([128, 3], F32, tag="mu")
                nc.vector.tensor_scalar_mul(mu[:], gstats[:, 0:3], 1.0 / B)
                ex2 = sp.tile([128, 3], F32, tag="ex2")
                nc.vector.tensor_scalar_mul(ex2[:], gstats[:, 3:6], 1.0 / B)
                var = sp.tile([128, 3], F32, tag="var")
                nc.vector.tensor_tensor(out=var[:], in0=mu[:], in1=mu[:],
                                        op=OP.mult)
                nc.vector.tensor_tensor(out=var[:], in0=ex2[:], in1=var[:],
                                        op=OP.subtract)
                epst = sp.tile([128, 1], F32, tag="epst")
                nc.vector.memset(epst[:], 1e-5)
                sdv = sp.tile([128, 3], F32, tag="sdv")
                nc.scalar.activation(out=sdv[:], in_=var[:], func=AF.Sqrt,
                                     bias=epst[:], scale=1.0)
                rst = sp.tile([128, 3], F32, tag="rst")
                nc.vector.reciprocal(out=rst[:], in_=sdv[:])
                scl = sp.tile([128, 3], F32, tag="scl")
                nc.vector.tensor_tensor(out=scl[:], in0=bn_gt[:, 0:3],
                                        in1=rst[:], op=OP.mult)
                shf = sp.tile([128, 3], F32, tag="shf")
                nc.vector.tensor_tensor(out=shf[:], in0=mu[:], in1=scl[:],
                                        op=OP.mult)
                nc.vector.tensor_tensor(out=shf[:], in0=bn_gt[:, 3:6],
                                        in1=shf[:], op=OP.subtract)

                for gi_, (g, wd) in enumerate(zip(groups, gwidth)):
                    nc.vector.tensor_scalar(
                        out=g[0:wd, :], in0=g[0:wd, :],
                        scalar1=scl[0:wd, gi_:gi_ + 1],
                        scalar2=shf[0:wd, gi_:gi_ + 1],
                        op0=OP.mult, op1=OP.add)

                h1d = [sp.tile([128, BC], F32, tag=f"h1d{i}", name=f"h1d{i}")
                       for i in range(2)]
                for mh in range(2):
                    pm = mps.tile([128, BC], F32, tag="pm1")
                    for gi_, (g, wd) in enumerate(zip(groups, gwidth)):
                        nc.tensor.matmul(
                            out=pm[:],
                            lhsT=w1t[gi_][0:wd, mh * 128:(mh + 1) * 128],
                            rhs=g[0:wd, :], start=(gi_ == 0), stop=(gi_ == 2))
                    nc.scalar.activation(out=h1d[mh][:], in_=pm[:],
                                         func=AF.Relu,
                                         bias=dbt[:, mh:mh + 1], scale=1.0)
                pm2 = mps.tile([128, BC], F32, tag="pm2")
                for mh in range(2):
                    nc.tensor.matmul(out=pm2[:],
                                     lhsT=w2all[:, mh * 128:(mh + 1) * 128],
                                     rhs=h1d[mh][:], start=(mh == 0),
                                     stop=(mh == 1))
                h2d = sp.tile([128, BC], F32, tag="h2d")
                nc.scalar.activation(out=h2d[:], in_=pm2[:], func=AF.Relu,
                                     bias=dbt[:, 2:3], scale=1.0)
                pmo = mps.tile([1, BC], F32, tag="pmo")
                nc.tensor.matmul(out=pmo[:], lhsT=owt[:], rhs=h2d[:],
                                 start=True, stop=True)
                res = sp.tile([1, BC], F32, tag="res")
                nc.vector.tensor_scalar(
                    out=res[:], in0=pmo[:], scalar1=obt[0:1, 0:1],
                    scalar2=None, op0=OP.add)
                nc.sync.dma_start(out=out[:], in_=res[:])

    nc.compile()
    return nc


# --------------------------------------------------------------------------
def _pack_weights(gw):
    """Pack all network weights into 8 fp16 rows of WROW elems each."""

    def stack_rz(wih, whh):
        m = np.zeros((128, 128), np.float32)
        m[0:64, 0:64] = wih[64:128].T      # z, x-side
        m[64:128, 0:64] = whh[64:128].T    # z, h-side
        m[0:64, 64:128] = wih[0:64].T      # r, x-side
        m[64:128, 64:128] = whh[0:64].T    # r, h-side
        return m

    def block_n(wih, whh):
        m = np.zeros((128, 128), np.float32)
        m[0:64, 0:64] = wih[128:192].T     # i_n (-> M 0:64)
        m[64:128, 64:128] = whh[128:192].T  # h_n (-> M 64:128)
        return m

    def vecs(bih, bhh):
        brz = np.zeros(128, np.float32)
        brz[0:64] = bih[64:128] + bhh[64:128]   # z
        brz[64:128] = bih[0:64] + bhh[0:64]     # r
        bhhn = np.zeros(128, np.float32)
        bhhn[64:128] = bhh[128:192]
        bihn = np.zeros(128, np.float32)
        bihn[64:128] = bih[128:192]
        return brz, bhhn, bihn

    gvecs = np.zeros((128, 6), np.float32)
    gvecs[:, 0], gvecs[:, 1], gvecs[:, 2] = vecs(gw["gru_bih"], gw["gru_bhh"])
    gvecs[:, 3], gvecs[:, 4], gvecs[:, 5] = vecs(gw["augru_bih"],
                                                 gw["augru_bhh"])

    w1 = gw["att_w1"]
    w_att = np.zeros((128, 192), np.float32)
    w_att[64:128, 0:64] = w1[64:128] - w1[128:192]   # k-term
    w_att[64:128, 64:128] = w1[192:256]              # q*k-term
    w_att[64:128, 128:192] = w1[0:64] + w1[128:192]  # q-term
    attb = np.zeros((64, 2), np.float32)
    attb[:, 0] = gw["att_b1"]
    attb[0:16, 1] = gw["att_b2"]

    bn_g = np.zeros((128, 6), np.float32)
    for g in range(2):
        bn_g[:, g] = gw["bn_gamma"][g * 128:(g + 1) * 128]
        bn_g[:, 3 + g] = gw["bn_beta"][g * 128:(g + 1) * 128]
    bn_g[0:DL, 2] = gw["bn_gamma"][256:272]
    bn_g[0:DL, 5] = gw["bn_beta"][256:272]
    bn_g[64:128, 2] = gw["bn_gamma"][272:336]
    bn_g[64:128, 5] = gw["bn_beta"][272:336]
    dnn_w1p = np.zeros((384, 256), np.float32)
    dnn_w1p[0:256] = gw["dnn_w1"][0:256]
    dnn_w1p[256:272] = gw["dnn_w1"][256:272]
    dnn_w1p[320:384] = gw["dnn_w1"][272:336]
    dnn_b = np.zeros((128, 3), np.float32)
    dnn_b[:, 0] = gw["dnn_b1"][0:128]
    dnn_b[:, 1] = gw["dnn_b1"][128:256]
    dnn_b[:, 2] = gw["dnn_b2"]
    w2all = np.zeros((128, 256), np.float32)
    w2all[:, 0:128] = gw["dnn_w2"][0:128]
    w2all[:, 128:256] = gw["dnn_w2"][128:256]

    rows = np.zeros((8, WROW), np.float16)
    for i in range(3):
        rows[i, 0:32768] = dnn_w1p[128 * i:128 * (i + 1)].ravel()
    rows[3, 0:32768] = w2all.ravel()
    rows[4] = np.concatenate([stack_rz(gw["gru_wih"], gw["gru_whh"]),
                              block_n(gw["gru_wih"], gw["gru_whh"])],
                             axis=1).ravel()
    rows[5] = np.concatenate([stack_rz(gw["augru_wih"], gw["augru_whh"]),
                              block_n(gw["augru_wih"], gw["augru_whh"])],
                             axis=1).ravel()
    r6 = np.zeros(WROW, np.float32)
    r6[0:24576] = w_att.ravel()
    r6[O_GV:O_GV + 768] = gvecs.ravel()
    r6[O_W2A:O_W2A + 1024] = gw["att_w2"].ravel()
    r6[O_W3A:O_W3A + 16] = gw["att_w3"].ravel()
    r6[O_AB:O_AB + 128] = attb.ravel()
    r6[O_BNG:O_BNG + 768] = bn_g.ravel()
    r6[O_DBT:O_DBT + 384] = dnn_b.ravel()
    r6[O_OWT:O_OWT + 128] = gw["out_w"].ravel()
    r6[O_OBT] = float(np.asarray(gw["out_b"]).ravel()[0])
    rows[6] = r6.astype(np.float16)
    # row 7 stays all-zero: ZOFF pad gathers read from here
    return rows


def _host_prep(inputs, sch):
    lens = np.asarray(inputs["hist_valid_lens"]).astype(np.int64)
    order = sch["order"]
    tmax, nts, xcols = sch["tmax"], sch["nts"], sch["xcols"]
    nch, ni, xspan = sch["nch"], sch["ni"], sch["xspan"]

    emb16 = np.asarray(inputs["emb"], np.float32).astype(np.float16)
    hist_item = np.asarray(inputs["hist_item"]).astype(np.int64)
    tgt = np.asarray(inputs["target_item"]).astype(np.int64)
    spf = np.asarray(inputs["sparse_feature"]).astype(np.int64)
    dense = np.asarray(inputs["dense_feature"], np.float32)

    gw = {k: np.asarray(inputs[k], np.float32) for k in
          ("gru_wih", "gru_whh", "gru_bih", "gru_bhh",
           "augru_wih", "augru_whh", "augru_bih", "augru_bhh",
           "att_w1", "att_b1", "att_w2", "att_b2", "att_w3", "att_b3",
           "bn_gamma", "bn_beta", "dnn_w1", "dnn_b1", "dnn_w2", "dnn_b2",
           "out_w", "out_b")}
    wrows = _pack_weights(gw)

    def off(ids):
        # row index into the blob viewed as [(8*SHW)//E, E]
        return ((ids // VSH) * (SHW // E) + (ids % VSH)).astype(np.int32)

    # schedule column -> (t, r)
    dcol_t = np.zeros(xspan, np.int64)
    dcol_r = np.zeros(xspan, np.int64)
    for t in range(tmax):
        c0, n = int(xcols[t]), int(nts[t])
        dcol_t[c0:c0 + n] = t
        dcol_r[c0:c0 + n] = np.arange(n)

    cols = np.arange(xspan)
    chs = cols // 128
    ps = cols % 128

    in_maps = []
    for c in range(NCORES):
        rows = order[c::NCORES]
        shard = np.concatenate(
            [emb16[VSH * c:VSH * (c + 1)].ravel(), wrows[c]]).reshape(128,
                                                                      SH_F)
        idx = np.full((128, ni), ZOFF, np.int32)
        ids = hist_item[rows[dcol_r], dcol_t, :]          # [xspan, 2]
        offs = off(ids)
        idx[ps, 2 * chs] = offs[:, 0]
        idx[ps, 2 * chs + 1] = offs[:, 1]
        qoff = off(tgt[rows])                             # [256, 2]
        for half in range(2):
            idx[:, 2 * nch + 2 * half] = qoff[128 * half:128 * (half + 1), 0]
            idx[:, 2 * nch + 2 * half + 1] = qoff[128 * half:128 * (half + 1), 1]
        spoff = off(spf[rows])                            # [256, 8]
        for rh in range(2):
            for gf in range(2):
                for j in range(4):
                    idx[:, 2 * nch + 4 + (rh * 2 + gf) * 4 + j] = \
                        spoff[128 * rh:128 * (rh + 1), 4 * gf + j]
        idx[:, ni - 2] = lens[rows[0:128]]
        idx[:, ni - 1] = lens[rows[128:256]]

        densT = np.ascontiguousarray(dense[rows, :].T)
        in_maps.append(dict(shard=shard, idx=idx, densT=densT))
    return in_maps, order


_CACHE = {}


def kernel(**inputs):
    lens = np.asarray(inputs["hist_valid_lens"]).astype(np.int64)
    key = hashlib.sha1(lens.tobytes()).hexdigest()
    sch = _make_schedule(lens)
    if key not in _CACHE:
        _CACHE[key] = _build(sch)
    nc = _CACHE[key]
    in_maps, order = _host_prep(inputs, sch)
    import os, time
    trace = bool(os.environ.get("KTRACE"))
    t0 = time.perf_counter()
    res = run_bass_kernel_spmd(nc, in_maps, core_ids=list(range(NCORES)),
                               trace=trace)
    kernel.last_spmd_s = time.perf_counter() - t0
    if trace and res.exec_time_ns is not None:
        print(f"HW exec time: {res.exec_time_ns} ns")
    kernel.last_res = res
    kernel.last_sch = sch
    kernel.last_maps = in_maps
    out = np.zeros((B, 1), np.float32)
    for c in range(NCORES):
        rows = order[c::NCORES]
        out[rows, 0] = res.results[c]["out"][0]
    return out


# revision 12
# speedup vs baseline: 8.6963x; 3.6408x over previous
"""DIEN forward on 8 Trainium2 NeuronCores (Bass/Tile).

Data-parallel with ragged packing; device-side embedding gather.

The axon RPC wall time is dominated by input upload (~55 MB/s tunnel), so
the kernel minimizes bytes shipped per call:
 - The 100k x 32 embedding table is converted to fp16 on the host, row-
   sharded 1/8th per core (0.8 MB each), AllGathered on-device into DRAM,
   and all embedding lookups (history, target, sparse) run as on-device
   indirect-DMA gathers + PE transposes into the feature-major layout.
 - All network weights are packed fp16 into the same sharded blob
   (64 KB/core) and sliced out of the AllGathered copy.
 - Attention masks are built on-device from iota + per-row lengths.
 - Host ships only: blob shard (fp16), gather offsets (int32), dense
   features (f32) -- ~1.1 MB/core vs ~7.6 MB/core for pre-gathered f32.

Compute layout (unchanged from the packed-ragged design):
 - Host sorts batch rows by descending hist_valid_lens, deals them to the 8
   cores round-robin over the sorted order, and builds a shared per-timestep
   active-column schedule N_t (max over cores, padded to x4).  All per-step
   shapes are compile-time constants.
 - Feature-major layout: interests/h at SBUF partitions 64:128 of one packed
   buffer BUF, gathered hist embeddings x^T at partitions 0:64, with x(t)
   stored at the columns of h(t-1) so each GRU step runs K=128-stacked
   matmuls over [x_t ; h_{t-1}] with a single rhs AP.
 - Attention runs over packed ragged columns; L3 scatters scores straight
   into a batch-major [r, t] PSUM via per-region column matmuls; softmax is
   batch-major; att is transposed back (PE) and streamed to the AUGRU as
   [1, N] rows broadcast via K=1 ones-matmuls.
 - BatchNorm (training stats over the full 2048 batch) via a tiny AllReduce.

kernel(**inputs) takes FULL unsharded inputs, returns [B, 1] float32.
"""

import hashlib
import numpy as np

import concourse.bass as bass
import concourse.bacc as bacc
import concourse.tile as tile
from concourse import mybir
from concourse.bass_utils import run_bass_kernel_spmd
from concourse.masks import make_identity

F32 = mybir.dt.float32
F16 = mybir.dt.float16
I32 = mybir.dt.int32
AF = mybir.ActivationFunctionType
OP = mybir.AluOpType
AX = mybir.AxisListType

B, T, E, NF, SL, DL, VOCAB = 2048, 200, 32, 2, 8, 16, 100000
D = E * NF          # 64
NCORES = 8
BC = B // NCORES    # 256

# sharded fp16 blob: [emb rows | weight row], per core
VSH = VOCAB // NCORES            # 12500 emb rows per shard
EMB_ELEMS = VSH * E              # 400000
WROW = 32768                     # weight elems per shard row
SHW = EMB_ELEMS + WROW           # 432768 fp16 elems per shard
SH_F = SHW // 128                # 3381 (shard shipped as [128, SH_F])
WB = EMB_ELEMS                   # weight-area base inside a row
ZOFF = (7 * SHW + WB) // E       # guaranteed-zero blob row (row 7 pad area)
GRP = 16                         # gather chunks (128 cols) per indirect DMA

# weight-area element offsets (row 6 smalls)
O_GV = 24576
O_W2A = O_GV + 768
O_W3A = O_W2A + 1024
O_AB = O_W3A + 16
O_BNG = O_AB + 128
O_DBT = O_BNG + 768
O_OWT = O_DBT + 384
O_OBT = O_OWT + 128


# --------------------------------------------------------------------------
def _make_schedule(lens):
    order = np.argsort(-lens, kind="stable")
    core_lens = lens[order].reshape(-1, NCORES)       # [BC, 8]
    tmax = int(lens.max())
    nts = []
    for t in range(tmax):
        n = int((core_lens > t).sum(axis=0).max())
        n = min(BC, ((n + 3) // 4) * 4)
        nts.append(max(n, 4))
    nts = np.array(nts, np.int32)
    offs = np.zeros(tmax + 1, np.int64)
    offs[1:] = np.cumsum(nts)
    m_total = int(offs[tmax])
    n0 = int(nts[0])
    xcols = np.zeros(tmax, np.int64)
    xcols[1:] = n0 + offs[:tmax - 1]                  # x(t) at h(t-1) cols
    hcols = n0 + offs[:tmax]
    xspan = int(n0 + offs[tmax - 1]) if tmax > 1 else int(nts[0])
    xspan = max(xspan, int(xcols[tmax - 1] + nts[tmax - 1]))
    nch = (xspan + 127) // 128                        # 128-col gather chunks
    buf_cols = max(n0 + m_total, nch * 128 + 128)
    ni = 2 * nch + 22                                 # idx tensor width
    return dict(order=order, tmax=tmax, nts=nts, offs=offs, m_total=m_total,
                n0=n0, xcols=xcols, hcols=hcols, xspan=xspan,
                nch=nch, ni=ni, buf_cols=buf_cols)


def _att_chunks(sch):
    chunks, cur, w = [], [], 0
    for t in range(sch["tmax"]):
        n = int(sch["nts"][t])
        if w + n > 512 and cur:
            chunks.append(cur)
            cur, w = [], 0
        cur.append(t)
        w += n
    if cur:
        chunks.append(cur)
    return chunks


# --------------------------------------------------------------------------
def _build(sch):
    nc = bacc.Bacc("TRN2", target_bir_lowering=False, debug=False,
                   num_devices=NCORES)
    tmax, nts = sch["tmax"], sch["nts"]
    hcols, xcols = sch["hcols"], sch["xcols"]
    n0, buf_cols = sch["n0"], sch["buf_cols"]
    nch, ni = sch["nch"], sch["ni"]
    chunks = _att_chunks(sch)

    shard = nc.dram_tensor("shard", [128, SH_F], F16, kind="ExternalInput")
    idx = nc.dram_tensor("idx", [128, ni], I32, kind="ExternalInput")
    densT = nc.dram_tensor("densT", [DL, BC], F32, kind="ExternalInput")
    out = nc.dram_tensor("out", [1, BC], F32, kind="ExternalOutput")
    import os
    KDEBUG = bool(os.environ.get("KDEBUG"))
    if KDEBUG:
        dbg_gath = nc.dram_tensor("dbg_gath", [8, 512], F32,
                                  kind="ExternalOutput")
        dbg_x = nc.dram_tensor("dbg_x", [64, 1024], F32, kind="ExternalOutput")
        dbg_q = nc.dram_tensor("dbg_q", [64, BC], F32, kind="ExternalOutput")
        dbg_sp = nc.dram_tensor("dbg_sp", [128, BC], F32,
                                kind="ExternalOutput")
        dbg_msk = nc.dram_tensor("dbg_msk", [128, 2 * T], F32,
                                 kind="ExternalOutput")
        dbg_h = nc.dram_tensor("dbg_h", [64, 512], F32, kind="ExternalOutput")
        dbg_wt = nc.dram_tensor("dbg_wt", [128, 256], F32,
                                kind="ExternalOutput")
        dbg_att = nc.dram_tensor("dbg_att", [128, 256], F32,
                                 kind="ExternalOutput")

    with tile.TileContext(nc) as tc:
        with tc.tile_pool(name="big", bufs=1) as big, \
             tc.tile_pool(name="w", bufs=1) as w, \
             tc.tile_pool(name="stage", bufs=2) as stage, \
             tc.tile_pool(name="gp", bufs=2) as gpool, \
             tc.tile_pool(name="idx", bufs=1) as idxp, \
             tc.tile_pool(name="s", bufs=2) as sp, \
             tc.tile_pool(name="s2", bufs=2) as sp2, \
             tc.tile_pool(name="dram", bufs=1, space="DRAM") as dramp:

            BUF = big.tile([128, buf_cols], F32)
            RH2 = big.tile([128, BC], F32)
            qT = big.tile([128, BC], F32)
            spT = [big.tile([128, BC], F32, tag=f"spT{i}", name=f"spT{i}")
                   for i in range(2)]
            attB = [big.tile([128, 256], F32, tag=f"attB{i}", name=f"attB{i}")
                    for i in range(2)]
            attT = [big.tile([128, 256], F32, tag=f"attT{i}", name=f"attT{i}")
                    for i in range(2)]

            ident = w.tile([128, 128], F32)
            make_identity(nc, ident[:])
            ident16 = w.tile([128, 128], F16)
            make_identity(nc, ident16[:])
            ones1 = w.tile([1, 64], F32)
            nc.vector.memset(ones1[:], 1.0)

            # ---------------- shard upload + AllGather -------------------
            stg = w.tile([128, SH_F], F16)
            nc.sync.dma_start(out=stg[:], in_=shard[:])
            cc_in = dramp.tile([128, SH_F], F16)
            nc.sync.dma_start(out=cc_in[:], in_=stg[:])
            gath = dramp.tile([NCORES, SHW], F16)
            nc.gpsimd.collective_compute(
                "AllGather", OP.bypass,
                replica_groups=[list(range(NCORES))],
                ins=[cc_in[:]], outs=[gath[:]])

            idxT = idxp.tile([128, ni], I32)
            nc.sync.dma_start(out=idxT[:], in_=idx[:])

            # ---------------- weight slices from blob --------------------
            def wload(shape, row, off, tag):
                p, f = shape
                st = stage.tile(shape, F16, tag=f"st_{tag}", name=f"st_{tag}")
                nc.sync.dma_start(
                    out=st[:],
                    in_=gath[row:row + 1, WB + off:WB + off + p * f].rearrange(
                        "a (p f) -> (a p) f", p=p))
                ft = w.tile(shape, F32, tag=f"w_{tag}", name=f"w_{tag}")
                nc.vector.tensor_copy(out=ft[:], in_=st[:])
                return ft

            w1t = [wload([128, 256], i, 0, f"w1t{i}") for i in range(3)]
            w2all = wload([128, 256], 3, 0, "w2all")
            wrzng = wload([128, 256], 4, 0, "wrzng")
            wrzna = wload([128, 256], 5, 0, "wrzna")
            watt = wload([128, 192], 6, 0, "watt")
            gv = wload([128, 6], 6, O_GV, "gv")
            w2a = wload([64, 16], 6, O_W2A, "w2a")
            w3a = wload([16, 1], 6, O_W3A, "w3a")
            ab = wload([64, 2], 6, O_AB, "ab")
            bn_gt = wload([128, 6], 6, O_BNG, "bng")
            dbt = wload([128, 3], 6, O_DBT, "dbt")
            owt = wload([128, 1], 6, O_OWT, "owt")
            obt = wload([1, 1], 6, O_OBT, "obt")

            nc.vector.memset(BUF[64:128, 0:n0], 0.0)

            if KDEBUG:
                dg16 = w.tile([8, 512], F16, tag="dg16")
                for r in range(8):
                    nc.sync.dma_start(out=dg16[r:r + 1, :],
                                      in_=gath[r:r + 1, 0:512])
                dg32 = w.tile([8, 512], F32, tag="dg32")
                nc.vector.tensor_copy(out=dg32[:], in_=dg16[:])
                nc.sync.dma_start(out=dbg_gath[:], in_=dg32[:])
                nc.sync.dma_start(out=dbg_wt[:], in_=wrzng[:])

            # ---------------- device-side embedding gathers --------------
            # HW SWDGE only handles the canonical indirect shape: offset AP
            # [128, 1], dest [128, rowlen] (one gathered row per partition
            # per instruction).  Offsets are row indices into the blob
            # viewed as [(8*SHW)//E, E].
            in_rows = gath[:, :].rearrange("r (q e) -> (r q) e", e=E)

            def gather1(dst, col):
                nc.gpsimd.indirect_dma_start(
                    out=dst, out_offset=None, in_=in_rows,
                    in_offset=bass.IndirectOffsetOnAxis(
                        ap=idxT[:, col:col + 1], axis=0))

            with tc.tile_pool(name="tps", bufs=3, space="PSUM") as tps:
                ngrp = (nch + GRP - 1) // GRP
                for g in range(ngrp):
                    c0 = g * GRP
                    gw = min(GRP, nch - c0)
                    gt = gpool.tile([128, GRP * 64], F16, tag="gt")
                    for c in range(gw):
                        for k in range(2):
                            gather1(gt[:, c * 64 + k * 32:c * 64 + k * 32 + 32],
                                    2 * (c0 + c) + k)
                    for c in range(gw):
                        j = c0 + c
                        pt = tps.tile([64, 128], F16, tag="pt")
                        nc.tensor.transpose(out=pt[:],
                                            in_=gt[:, c * 64:c * 64 + 64],
                                            identity=ident16[:])
                        nc.vector.tensor_copy(
                            out=BUF[0:64, j * 128:j * 128 + 128], in_=pt[:])

                # target-item embeddings -> qT[64:128, :]
                gq = gpool.tile([128, 128], F16, tag="gq")
                for c in range(2):
                    for k in range(2):
                        gather1(gq[:, c * 64 + k * 32:c * 64 + k * 32 + 32],
                                2 * nch + 2 * c + k)
                for c in range(2):
                    pt = tps.tile([64, 128], F16, tag="pt")
                    nc.tensor.transpose(out=pt[:], in_=gq[:, c * 64:c * 64 + 64],
                                        identity=ident16[:])
                    nc.vector.tensor_copy(
                        out=qT[64:128, c * 128:c * 128 + 128], in_=pt[:])

                # sparse-feature embeddings -> spT[0], spT[1]
                gsp = gpool.tile([128, 512], F16, tag="gsp")
                for j in range(16):
                    gather1(gsp[:, j * 32:(j + 1) * 32], 2 * nch + 4 + j)
                for rh in range(2):
                    for gf in range(2):
                        pt2 = tps.tile([128, 128], F16, tag="pt2")
                        base = (rh * 2 + gf) * 128
                        nc.tensor.transpose(out=pt2[:],
                                            in_=gsp[:, base:base + 128],
                                            identity=ident16[:])
                        nc.vector.tensor_copy(
                            out=spT[gf][:, rh * 128:rh * 128 + 128],
                            in_=pt2[:])

            # ---------------- scan step ---------------------------------
            def scan_step(pool, t, rhs_buf, rhs_col, wrz, wn, vo, out_buf,
                          out_col, att_rhs=None):
                n = int(nts[t])
                pA = pool.tile([128, 256], F32, tag="pA")
                pB = pool.tile([128, 256], F32, tag="pB")
                rhs = rhs_buf[:, rhs_col:rhs_col + n]
                nc.tensor.matmul(out=pA[:, 0:n], lhsT=wrz, rhs=rhs,
                                 start=True, stop=True)
                nc.tensor.matmul(out=pB[:, 0:n], lhsT=wn, rhs=rhs,
                                 start=True, stop=True)
                srz = sp.tile([128, 256], F32, tag="srz")
                nc.scalar.activation(out=srz[:, 0:n], in_=pA[:, 0:n],
                                     func=AF.Sigmoid,
                                     bias=gv[:, vo:vo + 1], scale=1.0)
                t1 = sp.tile([128, 256], F32, tag="t1")
                nc.vector.scalar_tensor_tensor(
                    out=t1[64:128, 0:n], in0=pB[64:128, 0:n],
                    scalar=gv[64:128, vo + 1:vo + 2],
                    in1=srz[64:128, 0:n], op0=OP.add, op1=OP.mult)
                t2 = sp.tile([128, 256], F32, tag="t2")
                nc.vector.tensor_tensor(out=t2[64:128, 0:n],
                                        in0=t1[64:128, 0:n],
                                        in1=pB[0:64, 0:n], op=OP.add)
                nt = sp.tile([128, 256], F32, tag="nt")
                nc.scalar.activation(out=nt[64:128, 0:n], in_=t2[64:128, 0:n],
                                     func=AF.Tanh,
                                     bias=gv[64:128, vo + 2:vo + 3], scale=1.0)
                pD = pool.tile([64, 256], F32, tag="pD")
                h_prev = rhs_buf[64:128, rhs_col:rhs_col + n]
                et = sp2.tile([128, 256], F32, tag="et")
                if att_rhs is None:
                    # GRU: h' = n + z*(h - n)
                    nc.vector.tensor_tensor(out=pD[0:64, 0:n], in0=h_prev,
                                            in1=nt[64:128, 0:n],
                                            op=OP.subtract)
                    nc.vector.tensor_tensor(out=et[64:128, 0:n],
                                            in0=pD[0:64, 0:n],
                                            in1=srz[0:64, 0:n], op=OP.mult)
                    nc.vector.tensor_tensor(
                        out=out_buf[64:128, out_col:out_col + n],
                        in0=et[64:128, 0:n], in1=nt[64:128, 0:n], op=OP.add)
                else:
                    # AUGRU: h' = h + att*z*(n - h)
                    nc.vector.tensor_tensor(out=pD[0:64, 0:n],
                                            in0=nt[64:128, 0:n],
                                            in1=h_prev, op=OP.subtract)
                    pAtt = pool.tile([64, 256], F32, tag="pAtt")
                    nc.tensor.matmul(out=pAtt[:, 0:n], lhsT=ones1[:],
                                     rhs=att_rhs, start=True, stop=True)
                    zt = sp2.tile([128, 256], F32, tag="zt")
                    nc.vector.tensor_tensor(out=zt[0:64, 0:n],
                                            in0=pAtt[0:64, 0:n],
                                            in1=srz[0:64, 0:n], op=OP.mult)
                    nc.vector.tensor_tensor(out=et[64:128, 0:n],
                                            in0=pD[0:64, 0:n],
                                            in1=zt[0:64, 0:n], op=OP.mult)
                    nc.vector.tensor_tensor(
                        out=out_buf[64:128, out_col:out_col + n],
                        in0=et[64:128, 0:n],
                        in1=rhs_buf[64:128, rhs_col:rhs_col + n], op=OP.add)

            if KDEBUG:
                nc.sync.dma_start(out=dbg_x[:], in_=BUF[0:64, 0:1024])
                nc.sync.dma_start(out=dbg_q[:], in_=qT[64:128, :])
                nc.sync.dma_start(out=dbg_sp[:], in_=spT[0][:])

            # ---------------- GRU scan ----------------------------------
            with tc.tile_pool(name="sps", bufs=2, space="PSUM") as sps:
                for t in range(tmax):
                    scan_step(sps, t, BUF, int(xcols[t]),
                              wrzng[:, 0:128], wrzng[:, 128:256], 0,
                              BUF, int(hcols[t]))
            if KDEBUG:
                nc.sync.dma_start(out=dbg_h[:], in_=BUF[64:128, n0:n0 + 512])

            # ---------------- attention ---------------------------------
            with tc.tile_pool(name="apsB", bufs=1, space="PSUM") as apsB, \
                 tc.tile_pool(name="aps", bufs=2, space="PSUM") as aps:
                psB = [apsB.tile([128, T], F32, tag=f"psB{i}", name=f"psB{i}")
                       for i in range(2)]
                nc.vector.memset(psB[0][:], 0.0)
                nc.vector.memset(psB[1][:], 0.0)

                for ch in chunks:
                    wch = int(sum(int(nts[t]) for t in ch))
                    qk = sp.tile([128, 512], F32, tag="qk")
                    col = 0
                    for t in ch:
                        n = int(nts[t])
                        hc = int(hcols[t])
                        nc.vector.tensor_tensor(
                            out=qk[64:128, col:col + n],
                            in0=BUF[64:128, hc:hc + n],
                            in1=qT[64:128, 0:n], op=OP.mult)
                        col += n
                    pL1 = aps.tile([64, 512], F32, tag="pL1")
                    col = 0
                    for t in ch:
                        n = int(nts[t])
                        hc = int(hcols[t])
                        nc.tensor.matmul(out=pL1[:, col:col + n],
                                         lhsT=watt[64:128, 0:64],
                                         rhs=BUF[64:128, hc:hc + n],
                                         start=True, stop=False)
                        nc.tensor.matmul(out=pL1[:, col:col + n],
                                         lhsT=watt[64:128, 64:128],
                                         rhs=qk[64:128, col:col + n],
                                         start=False, stop=False)
                        nc.tensor.matmul(out=pL1[:, col:col + n],
                                         lhsT=watt[64:128, 128:192],
                                         rhs=qT[64:128, 0:n],
                                         start=False, stop=True)
                        col += n
                    h1 = sp.tile([64, 512], F32, tag="h1")
                    nc.scalar.activation(out=h1[:, 0:wch], in_=pL1[:, 0:wch],
                                         func=AF.Relu, bias=ab[:, 0:1],
                                         scale=1.0)
                    pL2 = aps.tile([16, 512], F32, tag="pL2")
                    nc.tensor.matmul(out=pL2[:, 0:wch], lhsT=w2a[:],
                                     rhs=h1[:, 0:wch], start=True, stop=True)
                    h2 = sp.tile([16, 512], F32, tag="h2")
                    nc.scalar.activation(out=h2[:, 0:wch], in_=pL2[:, 0:wch],
                                         func=AF.Relu, bias=ab[0:16, 1:2],
                                         scale=1.0)
                    col = 0
                    for t in ch:
                        n = int(nts[t])
                        for piece in range(2):
                            lo = piece * 128
                            if lo >= n:
                                break
                            pw = min(128, n - lo)
                            nc.tensor.matmul(
                                out=psB[piece][0:pw, t:t + 1],
                                lhsT=h2[:, col + lo:col + lo + pw],
                                rhs=w3a[:], start=True, stop=True)
                        col += n

                # masks from iota + lens, then softmax (batch-major)
                ar_i = idxp.tile([128, T], I32)
                nc.gpsimd.iota(out=ar_i[:], pattern=[[1, T]], base=0,
                               channel_multiplier=0)
                arB = sp.tile([128, T], F32, tag="arB")
                nc.vector.tensor_copy(out=arB[:], in_=ar_i[:])
                lensf = sp.tile([128, 2], F32, tag="lensf")
                nc.vector.tensor_copy(out=lensf[:], in_=idxT[:, ni - 2:ni])
                for i in range(2):
                    m1 = sp.tile([128, T], F32, tag="m1")
                    nc.vector.tensor_scalar(
                        out=m1[:], in0=arB[:], scalar1=lensf[:, i:i + 1],
                        scalar2=1.0, op0=OP.subtract, op1=OP.add)
                    msk = sp.tile([128, T], F32, tag="msk")
                    nc.vector.tensor_scalar(
                        out=msk[:], in0=m1[:], scalar1=0.0, scalar2=-1e9,
                        op0=OP.max, op1=OP.mult)
                    if KDEBUG:
                        nc.sync.dma_start(out=dbg_msk[:, i * T:(i + 1) * T],
                                          in_=msk[:])
                    sc_t = sp.tile([128, T], F32, tag="sct")
                    nc.vector.tensor_tensor(out=sc_t[:], in0=psB[i][:],
                                            in1=msk[:], op=OP.add)
                    mx = sp.tile([128, 1], F32, tag="mx")
                    nc.vector.tensor_reduce(out=mx[:], in_=sc_t[:],
                                            axis=AX.X, op=OP.max)
                    nmx = sp.tile([128, 1], F32, tag="nmx")
                    nc.vector.tensor_scalar_mul(nmx[:], mx[:], -1.0)
                    ex = sp.tile([128, 256], F32, tag="ex")
                    nc.vector.memset(ex[:], 0.0)
                    nc.scalar.activation(out=ex[:, 0:T], in_=sc_t[:],
                                         func=AF.Exp, bias=nmx[:], scale=1.0)
                    sm = sp.tile([128, 1], F32, tag="sm")
                    nc.vector.tensor_reduce(out=sm[:], in_=ex[:, 0:T],
                                            axis=AX.X, op=OP.add)
                    rs = sp.tile([128, 1], F32, tag="rs")
                    nc.vector.reciprocal(out=rs[:], in_=sm[:])
                    nc.vector.memset(attB[i][:], 0.0)
                    nc.vector.tensor_scalar(
                        out=attB[i][:, 0:T], in0=ex[:, 0:T], scalar1=rs[:],
                        scalar2=None, op0=OP.mult)
                if KDEBUG:
                    nc.sync.dma_start(out=dbg_att[:], in_=attB[0][:])

                # transpose attB -> attT (rows = t, cols = r)
                for th in range(2):
                    tw = 128 if th == 0 else T - 128
                    for rh in range(2):
                        pat = aps.tile([128, 128], F32, tag="pAT")
                        nc.tensor.transpose(
                            out=pat[0:tw, :],
                            in_=attB[rh][:, th * 128:th * 128 + tw],
                            identity=ident[:])
                        nc.vector.tensor_copy(
                            out=attT[th][0:tw, rh * 128:(rh + 1) * 128],
                            in_=pat[0:tw, :])

            # ---------------- AUGRU scan --------------------------------
            nc.vector.memset(RH2[:], 0.0)
            with tc.tile_pool(name="aups", bufs=2, space="PSUM") as aups, \
                 tc.tile_pool(name="strp", bufs=2) as strp:
                nstrip = (tmax + 7) // 8
                for s in range(nstrip):
                    t0 = s * 8
                    t1s = min(t0 + 8, tmax)
                    rows = t1s - t0
                    strip = strp.tile([1, 8 * 256], F32, tag="strip")
                    th = t0 // 128
                    r0 = t0 - th * 128
                    nc.sync.dma_start(
                        out=strip[0:1, 0:rows * 256].rearrange(
                            "o (t r) -> o t r", t=rows),
                        in_=attT[th][r0:r0 + rows, :])
                    for t in range(t0, t1s):
                        n = int(nts[t])
                        hc = int(hcols[t])
                        nc.gpsimd.tensor_copy(out=RH2[0:64, 0:n],
                                              in_=BUF[64:128, hc:hc + n])
                        arhs = strip[0:1, (t - t0) * 256:(t - t0) * 256 + n]
                        scan_step(aups, t, RH2, 0,
                                  wrzna[:, 0:128], wrzna[:, 128:256], 3,
                                  RH2, 0, att_rhs=arhs)

            # ---------------- DNN head ----------------------------------
            with tc.tile_pool(name="mps", bufs=2, space="PSUM") as mps:
                densTt = big.tile([128, BC], F32, tag="densTt")
                nc.vector.memset(densTt[:], 0.0)
                nc.sync.dma_start(out=densTt[0:DL, :], in_=densT[:])
                nc.vector.tensor_copy(out=densTt[64:128, :],
                                      in_=RH2[64:128, :])

                groups = [spT[0], spT[1], densTt]
                gwidth = [128, 128, 128]
                stats = sp.tile([128, 6], F32, tag="stats")
                nc.vector.memset(stats[:], 0.0)
                scratch = sp.tile([128, BC], F32, tag="scr")
                for gi_, (g, wd) in enumerate(zip(groups, gwidth)):
                    nc.vector.tensor_reduce(out=stats[0:wd, gi_:gi_ + 1],
                                            in_=g[0:wd, :], axis=AX.X,
                                            op=OP.add)
                    nc.vector.scalar_tensor_tensor(
                        out=scratch[0:wd, :], in0=g[0:wd, :], scalar=0.0,
                        in1=g[0:wd, :], op0=OP.add, op1=OP.mult,
                        accum_out=stats[0:wd, 3 + gi_:4 + gi_])

                cc_in2 = dramp.tile([128, 6], F32)
                cc_out2 = dramp.tile([128, 6], F32)
                nc.sync.dma_start(out=cc_in2[:], in_=stats[:])
                nc.gpsimd.collective_compute(
                    "AllReduce", OP.add,
                    replica_groups=[list(range(NCORES))],
                    ins=[cc_in2.opt()], outs=[cc_out2.opt()])
                gstats = sp.tile([128, 6], F32, tag="gstats")
                nc.sync.dma_start(out=gstats[:], in_=cc_out2[:])

                mu = sp.tile([128, 3], F32, tag="mu")
                nc.vector.tensor_scalar_mul(mu[:], gstats[:, 0:3], 1.0 / B)
                ex2 = sp.tile([128, 3], F32, tag="ex2")
                nc.vector.tensor_scalar_mul(ex2[:], gstats[:, 3:6], 1.0 / B)
                var = sp.tile([128, 3], F32, tag="var")
                nc.vector.tensor_tensor(out=var[:], in0=mu[:], in1=mu[:],
                                        op=OP.mult)
                nc.vector.tensor_tensor(out=var[:], in0=ex2[:], in1=var[:],
                                        op=OP.subtract)
                epst = sp.tile([128, 1], F32, tag="epst")
                nc.vector.memset(epst[:], 1e-5)
                sdv = sp.tile([128, 3], F32, tag="sdv")
                nc.scalar.activation(out=sdv[:], in_=var[:], func=AF.Sqrt,
                                     bias=epst[:], scale=1.0)
                rst = sp.tile([128, 3], F32, tag="rst")
                nc.vector.reciprocal(out=rst[:], in_=sdv[:])
                scl = sp.tile([128, 3], F32, tag="scl")
                nc.vector.tensor_tensor(out=scl[:], in0=bn_gt[:, 0:3],
                                        in1=rst[:], op=OP.mult)
                shf = sp.tile([128, 3], F32, tag="shf")
                nc.vector.tensor_tensor(out=shf[:], in0=mu[:], in1=scl[:],
                                        op=OP.mult)
                nc.vector.tensor_tensor(out=shf[:], in0=bn_gt[:, 3:6],
                                        in1=shf[:], op=OP.subtract)

                for gi_, (g, wd) in enumerate(zip(groups, gwidth)):
                    nc.vector.tensor_scalar(
                        out=g[0:wd, :], in0=g[0:wd, :],
                        scalar1=scl[0:wd, gi_:gi_ + 1],
                        scalar2=shf[0:wd, gi_:gi_ + 1],
                        op0=OP.mult, op1=OP.add)

                h1d = [sp.tile([128, BC], F32, tag=f"h1d{i}", name=f"h1d{i}")
                       for i in range(2)]
                for mh in range(2):
                    pm = mps.tile([128, BC], F32, tag="pm1")
                    for gi_, (g, wd) in enumerate(zip(groups, gwidth)):
                        nc.tensor.matmul(
                            out=pm[:],
                            lhsT=w1t[gi_][0:wd, mh * 128:(mh + 1) * 128],
                            rhs=g[0:wd, :], start=(gi_ == 0), stop=(gi_ == 2))
                    nc.scalar.activation(out=h1d[mh][:], in_=pm[:],
                                         func=AF.Relu,
                                         bias=dbt[:, mh:mh + 1], scale=1.0)
                pm2 = mps.tile([128, BC], F32, tag="pm2")
                for mh in range(2):
                    nc.tensor.matmul(out=pm2[:],
                                     lhsT=w2all[:, mh * 128:(mh + 1) * 128],
                                     rhs=h1d[mh][:], start=(mh == 0),
                                     stop=(mh == 1))
                h2d = sp.tile([128, BC], F32, tag="h2d")
                nc.scalar.activation(out=h2d[:], in_=pm2[:], func=AF.Relu,
                                     bias=dbt[:, 2:3], scale=1.0)
                pmo = mps.tile([1, BC], F32, tag="pmo")
                nc.tensor.matmul(out=pmo[:], lhsT=owt[:], rhs=h2d[:],
                                 start=True, stop=True)
                res = sp.tile([1, BC], F32, tag="res")
                nc.vector.tensor_scalar(
                    out=res[:], in0=pmo[:], scalar1=obt[0:1, 0:1],
                    scalar2=None, op0=OP.add)
                nc.sync.dma_start(out=out[:], in_=res[:])

    nc.compile()
    return nc


# --------------------------------------------------------------------------
def _pack_weights(gw):
    """Pack all network weights into 8 fp16 rows of WROW elems each."""

    def stack_rz(wih, whh):
        m = np.zeros((128, 128), np.float32)
        m[0:64, 0:64] = wih[64:128].T      # z, x-side
        m[64:128, 0:64] = whh[64:128].T    # z, h-side
        m[0:64, 64:128] = wih[0:64].T      # r, x-side
        m[64:128, 64:128] = whh[0:64].T    # r, h-side
        return m

    def block_n(wih, whh):
        m = np.zeros((128, 128), np.float32)
        m[0:64, 0:64] = wih[128:192].T     # i_n (-> M 0:64)
        m[64:128, 64:128] = whh[128:192].T  # h_n (-> M 64:128)
        return m

    def vecs(bih, bhh):
        brz = np.zeros(128, np.float32)
        brz[0:64] = bih[64:128] + bhh[64:128]   # z
        brz[64:128] = bih[0:64] + bhh[0:64]     # r
        bhhn = np.zeros(128, np.float32)
        bhhn[64:128] = bhh[128:192]
        bihn = np.zeros(128, np.float32)
        bihn[64:128] = bih[128:192]
        return brz, bhhn, bihn

    gvecs = np.zeros((128, 6), np.float32)
    gvecs[:, 0], gvecs[:, 1], gvecs[:, 2] = vecs(gw["gru_bih"], gw["gru_bhh"])
    gvecs[:, 3], gvecs[:, 4], gvecs[:, 5] = vecs(gw["augru_bih"],
                                                 gw["augru_bhh"])

    w1 = gw["att_w1"]
    w_att = np.zeros((128, 192), np.float32)
    w_att[64:128, 0:64] = w1[64:128] - w1[128:192]   # k-term
    w_att[64:128, 64:128] = w1[192:256]              # q*k-term
    w_att[64:128, 128:192] = w1[0:64] + w1[128:192]  # q-term
    attb = np.zeros((64, 2), np.float32)
    attb[:, 0] = gw["att_b1"]
    attb[0:16, 1] = gw["att_b2"]

    bn_g = np.zeros((128, 6), np.float32)
    for g in range(2):
        bn_g[:, g] = gw["bn_gamma"][g * 128:(g + 1) * 128]
        bn_g[:, 3 + g] = gw["bn_beta"][g * 128:(g + 1) * 128]
    bn_g[0:DL, 2] = gw["bn_gamma"][256:272]
    bn_g[0:DL, 5] = gw["bn_beta"][256:272]
    bn_g[64:128, 2] = gw["bn_gamma"][272:336]
    bn_g[64:128, 5] = gw["bn_beta"][272:336]
    dnn_w1p = np.zeros((384, 256), np.float32)
    dnn_w1p[0:256] = gw["dnn_w1"][0:256]
    dnn_w1p[256:272] = gw["dnn_w1"][256:272]
    dnn_w1p[320:384] = gw["dnn_w1"][272:336]
    dnn_b = np.zeros((128, 3), np.float32)
    dnn_b[:, 0] = gw["dnn_b1"][0:128]
    dnn_b[:, 1] = gw["dnn_b1"][128:256]
    dnn_b[:, 2] = gw["dnn_b2"]
    w2all = np.zeros((128, 256), np.float32)
    w2all[:, 0:128] = gw["dnn_w2"][0:128]
    w2all[:, 128:256] = gw["dnn_w2"][128:256]

    rows = np.zeros((8, WROW), np.float16)
    for i in range(3):
        rows[i, 0:32768] = dnn_w1p[128 * i:128 * (i + 1)].ravel()
    rows[3, 0:32768] = w2all.ravel()
    rows[4] = np.concatenate([stack_rz(gw["gru_wih"], gw["gru_whh"]),
                              block_n(gw["gru_wih"], gw["gru_whh"])],
                             axis=1).ravel()
    rows[5] = np.concatenate([stack_rz(gw["augru_wih"], gw["augru_whh"]),
                              block_n(gw["augru_wih"], gw["augru_whh"])],
                             axis=1).ravel()
    r6 = np.zeros(WROW, np.float32)
    r6[0:24576] = w_att.ravel()
    r6[O_GV:O_GV + 768] = gvecs.ravel()
    r6[O_W2A:O_W2A + 1024] = gw["att_w2"].ravel()
    r6[O_W3A:O_W3A + 16] = gw["att_w3"].ravel()
    r6[O_AB:O_AB + 128] = attb.ravel()
    r6[O_BNG:O_BNG + 768] = bn_g.ravel()
    r6[O_DBT:O_DBT + 384] = dnn_b.ravel()
    r6[O_OWT:O_OWT + 128] = gw["out_w"].ravel()
    r6[O_OBT] = float(np.asarray(gw["out_b"]).ravel()[0])
    rows[6] = r6.astype(np.float16)
    # row 7 stays all-zero: ZOFF pad gathers read from here
    return rows


def _host_prep(inputs, sch):
    lens = np.asarray(inputs["hist_valid_lens"]).astype(np.int64)
    order = sch["order"]
    tmax, nts, xcols = sch["tmax"], sch["nts"], sch["xcols"]
    nch, ni, xspan = sch["nch"], sch["ni"], sch["xspan"]

    emb16 = np.asarray(inputs["emb"], np.float32).astype(np.float16)
    hist_item = np.asarray(inputs["hist_item"]).astype(np.int64)
    tgt = np.asarray(inputs["target_item"]).astype(np.int64)
    spf = np.asarray(inputs["sparse_feature"]).astype(np.int64)
    dense = np.asarray(inputs["dense_feature"], np.float32)

    gw = {k: np.asarray(inputs[k], np.float32) for k in
          ("gru_wih", "gru_whh", "gru_bih", "gru_bhh",
           "augru_wih", "augru_whh", "augru_bih", "augru_bhh",
           "att_w1", "att_b1", "att_w2", "att_b2", "att_w3", "att_b3",
           "bn_gamma", "bn_beta", "dnn_w1", "dnn_b1", "dnn_w2", "dnn_b2",
           "out_w", "out_b")}
    wrows = _pack_weights(gw)

    def off(ids):
        # row index into the blob viewed as [(8*SHW)//E, E]
        return ((ids // VSH) * (SHW // E) + (ids % VSH)).astype(np.int32)

    # schedule column -> (t, r)
    dcol_t = np.zeros(xspan, np.int64)
    dcol_r = np.zeros(xspan, np.int64)
    for t in range(tmax):
        c0, n = int(xcols[t]), int(nts[t])
        dcol_t[c0:c0 + n] = t
        dcol_r[c0:c0 + n] = np.arange(n)

    cols = np.arange(xspan)
    chs = cols // 128
    ps = cols % 128

    in_maps = []
    for c in range(NCORES):
        rows = order[c::NCORES]
        shard = np.concatenate(
            [emb16[VSH * c:VSH * (c + 1)].ravel(), wrows[c]]).reshape(128,
                                                                      SH_F)
        idx = np.full((128, ni), ZOFF, np.int32)
        ids = hist_item[rows[dcol_r], dcol_t, :]          # [xspan, 2]
        offs = off(ids)
        idx[ps, 2 * chs] = offs[:, 0]
        idx[ps, 2 * chs + 1] = offs[:, 1]
        qoff = off(tgt[rows])                             # [256, 2]
        for half in range(2):
            idx[:, 2 * nch + 2 * half] = qoff[128 * half:128 * (half + 1), 0]
            idx[:, 2 * nch + 2 * half + 1] = qoff[128 * half:128 * (half + 1), 1]
        spoff = off(spf[rows])                            # [256, 8]
        for rh in range(2):
            for gf in range(2):
                for j in range(4):
                    idx[:, 2 * nch + 4 + (rh * 2 + gf) * 4 + j] = \
                        spoff[128 * rh:128 * (rh + 1), 4 * gf + j]
        idx[:, ni - 2] = lens[rows[0:128]]
        idx[:, ni - 1] = lens[rows[128:256]]

        densT = np.ascontiguousarray(dense[rows, :].T)
        in_maps.append(dict(shard=shard, idx=idx, densT=densT))
    return in_maps, order


_CACHE = {}

# --------------------------------------------------------------------------
# run_bass_kernel_spmd re-creates a fresh jax.jit wrapper on every call,
# which costs ~0.5s/call in re-trace + executable re-load even when every
# compile cache hits.  Memoize the compiled executable per Bass module so
# repeat calls only pay transfer + execution.  Semantics are identical to
# bass2jax.run_bass_via_pjrt (same _bass_exec_p custom call, same NEFF).
_EXEC_CACHE = {}


def _cached_run_bass_via_pjrt(nc, in_maps, n_cores):
    import jax
    import numpy as _np
    from jax.sharding import Mesh, PartitionSpec
    from jax.experimental.shard_map import shard_map
    from concourse import bass2jax, mybir as _mb
    from concourse.bass2jax import (_bass_exec_p, partition_id_tensor,
                                    install_neuronx_cc_hook)

    install_neuronx_cc_hook()
    if nc.dbg_addr is not None:
        if nc.dbg_callbacks:
            raise RuntimeError("dbg_callbacks unsupported here")
        in_maps = [{**m, nc.dbg_addr.name: _np.zeros((1, 2), _np.uint32)}
                   for m in in_maps]

    key = id(nc)
    if key not in _EXEC_CACHE:
        partition_name = (nc.partition_id_tensor.name
                          if nc.partition_id_tensor else None)
        in_names, out_names, out_avals = [], [], []
        for alloc in nc.m.functions[0].allocations:
            if not isinstance(alloc, _mb.MemoryLocationSet):
                continue
            name = alloc.memorylocations[0].name
            if alloc.kind == "ExternalInput":
                if name != partition_name:
                    in_names.append(name)
            elif alloc.kind == "ExternalOutput":
                out_names.append(name)
                out_avals.append(jax.core.ShapedArray(
                    tuple(alloc.tensor_shape), _mb.dt.np(alloc.dtype)))
        n_params = len(in_names)
        n_outs = len(out_avals)
        in_names_full = in_names + out_names
        if partition_name is not None:
            in_names_full.append(partition_name)
        donate = tuple(range(n_params, n_params + n_outs))

        def _body(*args):
            operands = list(args)
            if partition_name is not None:
                operands.append(partition_id_tensor())
            outs = _bass_exec_p.bind(
                *operands, out_avals=tuple(out_avals),
                in_names=tuple(in_names_full), out_names=tuple(out_names),
                lowering_input_output_aliases=(),
                sim_require_finite=True, sim_require_nnan=True, nc=nc)
            return tuple(outs)

        devices = jax.devices()[:n_cores]
        assert len(devices) == n_cores
        mesh = Mesh(_np.asarray(devices), ("core",))
        in_specs = (PartitionSpec("core"),) * (n_params + n_outs)
        out_specs = (PartitionSpec("core"),) * n_outs
        sharded = jax.jit(
            shard_map(_body, mesh=mesh, in_specs=in_specs,
                      out_specs=out_specs, check_rep=False),
            donate_argnums=donate, keep_unused=True)
        _EXEC_CACHE[key] = (sharded, in_names, out_names, out_avals, n_params)

    sharded, in_names, out_names, out_avals, n_params = _EXEC_CACHE[key]
    per_core = [[_np.asarray(m[nm]) for nm in in_names] for m in in_maps]
    concat_in = [_np.concatenate([per_core[c][i] for c in range(n_cores)],
                                 axis=0) for i in range(n_params)]
    concat_zeros = [_np.zeros((n_cores * a.shape[0], *a.shape[1:]), a.dtype)
                    for a in out_avals]
    out_arrs = sharded(*concat_in, *concat_zeros)
    return [
        {name: _np.asarray(out_arrs[i]).reshape(n_cores,
                                                *out_avals[i].shape)[c]
         for i, name in enumerate(out_names)}
        for c in range(n_cores)
    ]


def _install_pjrt_cache():
    from concourse import bass2jax
    if getattr(bass2jax.run_bass_via_pjrt, "_dien_cached", False):
        return
    _cached_run_bass_via_pjrt._dien_cached = True
    bass2jax.run_bass_via_pjrt = _cached_run_bass_via_pjrt


def kernel(**inputs):
    _install_pjrt_cache()
    lens = np.asarray(inputs["hist_valid_lens"]).astype(np.int64)
    key = hashlib.sha1(lens.tobytes()).hexdigest()
    sch = _make_schedule(lens)
    if key not in _CACHE:
        _CACHE[key] = _build(sch)
    nc = _CACHE[key]
    in_maps, order = _host_prep(inputs, sch)
    import os, time
    trace = bool(os.environ.get("KTRACE"))
    t0 = time.perf_counter()
    res = run_bass_kernel_spmd(nc, in_maps, core_ids=list(range(NCORES)),
                               trace=trace)
    kernel.last_spmd_s = time.perf_counter() - t0
    if trace and res.exec_time_ns is not None:
        print(f"HW exec time: {res.exec_time_ns} ns")
    kernel.last_res = res
    kernel.last_sch = sch
    kernel.last_maps = in_maps
    out = np.zeros((B, 1), np.float32)
    for c in range(NCORES):
        rows = order[c::NCORES]
        out[rows, 0] = res.results[c]["out"][0]
    return out


# revision 24
# speedup vs baseline: 10.0842x; 1.1596x over previous
"""DIEN forward on 8 Trainium2 NeuronCores (Bass/Tile).

Data-parallel with ragged packing; device-side embedding gather.

The axon RPC wall time is dominated by input upload (~55 MB/s tunnel), so
the kernel minimizes bytes shipped per call:
 - The 100k x 32 embedding table is converted to fp16 on the host, row-
   sharded 1/8th per core (0.8 MB each), AllGathered on-device into DRAM,
   and all embedding lookups (history, target, sparse) run as on-device
   indirect-DMA gathers + PE transposes into the feature-major layout.
 - All network weights are packed fp16 into the same sharded blob
   (64 KB/core) and sliced out of the AllGathered copy.
 - Attention masks are built on-device from iota + per-row lengths.
 - Host ships only: blob shard (fp16), gather offsets (int32), dense
   features (f32) -- ~1.1 MB/core vs ~7.6 MB/core for pre-gathered f32.

Compute layout (unchanged from the packed-ragged design):
 - Host sorts batch rows by descending hist_valid_lens, deals them to the 8
   cores round-robin over the sorted order, and builds a shared per-timestep
   active-column schedule N_t (max over cores, padded to x4).  All per-step
   shapes are compile-time constants.
 - Feature-major layout: interests/h at SBUF partitions 64:128 of one packed
   buffer BUF, gathered hist embeddings x^T at partitions 0:64, with x(t)
   stored at the columns of h(t-1) so each GRU step runs K=128-stacked
   matmuls over [x_t ; h_{t-1}] with a single rhs AP.
 - Attention runs over packed ragged columns; L3 scatters scores straight
   into a batch-major [r, t] PSUM via per-region column matmuls; softmax is
   batch-major; att is transposed back (PE) and streamed to the AUGRU as
   [1, N] rows broadcast via K=1 ones-matmuls.
 - BatchNorm (training stats over the full 2048 batch) via a tiny AllReduce.

kernel(**inputs) takes FULL unsharded inputs, returns [B, 1] float32.
"""

import hashlib
import numpy as np

import concourse.bass as bass
import concourse.bacc as bacc
import concourse.tile as tile
from concourse import mybir
from concourse.bass_utils import run_bass_kernel_spmd
from concourse.masks import make_identity

F32 = mybir.dt.float32
F16 = mybir.dt.float16
F8E4 = mybir.dt.float8e4
F8E3 = mybir.dt.float8e3
I32 = mybir.dt.int32
AF = mybir.ActivationFunctionType
OP = mybir.AluOpType
AX = mybir.AxisListType

B, T, E, NF, SL, DL, VOCAB = 2048, 200, 32, 2, 8, 16, 100000
D = E * NF          # 64
NCORES = 8
BC = B // NCORES    # 256

# sharded blobs, per core: emb rows (fp16 or scaled fp8) + fp16 weight row
VSH = VOCAB // NCORES            # 12500 emb rows per shard
EMB_ELEMS = VSH * E              # 400000
EPF = EMB_ELEMS // 128           # 3125 (eshard shipped as [128, EPF])
WROW = 32768                     # weight elems per shard row
ZOFF = 0                         # pad gathers read emb row 0 (value unused)
GRP = 16                         # gather chunks (128 cols) per staging tile

# embedding-table wire format: "f16", "f8e4" (e4m3) or "f8e3" (e3m4),
# scaled by ESCALE on the host and divided back out on-device.
import os as _os
EMB_WIRE = _os.environ.get("EMB_WIRE", "f8e3")
ESCALE = 1.0 if EMB_WIRE == "f16" else 256.0

# weight-area element offsets (row 6 smalls)
O_GV = 24576
O_W2A = O_GV + 768
O_W3A = O_W2A + 1024
O_AB = O_W3A + 16
O_BNG = O_AB + 128
O_DBT = O_BNG + 768
O_OWT = O_DBT + 384
O_OBT = O_OWT + 128


# --------------------------------------------------------------------------
def _make_schedule(lens):
    order = np.argsort(-lens, kind="stable")
    core_lens = lens[order].reshape(-1, NCORES)       # [BC, 8]
    tmax = int(lens.max())
    nts = []
    for t in range(tmax):
        n = int((core_lens > t).sum(axis=0).max())
        n = min(BC, ((n + 3) // 4) * 4)
        nts.append(max(n, 4))
    nts = np.array(nts, np.int32)
    offs = np.zeros(tmax + 1, np.int64)
    offs[1:] = np.cumsum(nts)
    m_total = int(offs[tmax])
    n0 = int(nts[0])
    xcols = np.zeros(tmax, np.int64)
    xcols[1:] = n0 + offs[:tmax - 1]                  # x(t) at h(t-1) cols
    hcols = n0 + offs[:tmax]
    xspan = int(n0 + offs[tmax - 1]) if tmax > 1 else int(nts[0])
    xspan = max(xspan, int(xcols[tmax - 1] + nts[tmax - 1]))
    nch = (xspan + 127) // 128                        # 128-col gather chunks
    buf_cols = max(n0 + m_total, nch * 128 + 128)
    ni = 2 * nch + 22                                 # idx tensor width
    return dict(order=order, tmax=tmax, nts=nts, offs=offs, m_total=m_total,
                n0=n0, xcols=xcols, hcols=hcols, xspan=xspan,
                nch=nch, ni=ni, buf_cols=buf_cols)


def _att_chunks(sch):
    chunks, cur, w = [], [], 0
    for t in range(sch["tmax"]):
        n = int(sch["nts"][t])
        if w + n > 512 and cur:
            chunks.append(cur)
            cur, w = [], 0
        cur.append(t)
        w += n
    if cur:
        chunks.append(cur)
    return chunks


# --------------------------------------------------------------------------
def _build(sch):
    nc = bacc.Bacc("TRN2", target_bir_lowering=False, debug=False,
                   num_devices=NCORES)
    tmax, nts = sch["tmax"], sch["nts"]
    hcols, xcols = sch["hcols"], sch["xcols"]
    n0, buf_cols = sch["n0"], sch["buf_cols"]
    nch, ni = sch["nch"], sch["ni"]
    chunks = _att_chunks(sch)

    EMB_DT = {"f16": F16, "f8e4": F8E4, "f8e3": F8E3}[EMB_WIRE]
    eshard = nc.dram_tensor("eshard", [128, EPF], EMB_DT, kind="ExternalInput")
    wshard = nc.dram_tensor("wshard", [128, WROW // 128], F16,
                            kind="ExternalInput")
    idx = nc.dram_tensor("idx", [128, ni], I32, kind="ExternalInput")
    densT = nc.dram_tensor("densT", [DL, BC], F32, kind="ExternalInput")
    out = nc.dram_tensor("out", [1, BC], F32, kind="ExternalOutput")
    import os
    KDEBUG = bool(os.environ.get("KDEBUG"))
    if KDEBUG:
        dbg_gath = nc.dram_tensor("dbg_gath", [8, 512], F32,
                                  kind="ExternalOutput")
        dbg_x = nc.dram_tensor("dbg_x", [64, 1024], F32, kind="ExternalOutput")
        dbg_q = nc.dram_tensor("dbg_q", [64, BC], F32, kind="ExternalOutput")
        dbg_sp = nc.dram_tensor("dbg_sp", [128, BC], F32,
                                kind="ExternalOutput")
        dbg_msk = nc.dram_tensor("dbg_msk", [128, 2 * T], F32,
                                 kind="ExternalOutput")
        dbg_h = nc.dram_tensor("dbg_h", [64, 512], F32, kind="ExternalOutput")
        dbg_wt = nc.dram_tensor("dbg_wt", [128, 256], F32,
                                kind="ExternalOutput")
        dbg_att = nc.dram_tensor("dbg_att", [128, 256], F32,
                                 kind="ExternalOutput")

    with tile.TileContext(nc) as tc:
        with tc.tile_pool(name="big", bufs=1) as big, \
             tc.tile_pool(name="w", bufs=1) as w, \
             tc.tile_pool(name="stage", bufs=2) as stage, \
             tc.tile_pool(name="gp", bufs=2) as gpool, \
             tc.tile_pool(name="idx", bufs=1) as idxp, \
             tc.tile_pool(name="s", bufs=2) as sp, \
             tc.tile_pool(name="s2", bufs=2) as sp2, \
             tc.tile_pool(name="dram", bufs=1, space="DRAM") as dramp:

            BUF = big.tile([128, buf_cols], F32)
            RH2 = big.tile([128, BC], F32)
            qT = big.tile([128, BC], F32)
            spT = [big.tile([128, BC], F32, tag=f"spT{i}", name=f"spT{i}")
                   for i in range(2)]
            attB = [big.tile([128, 256], F32, tag=f"attB{i}", name=f"attB{i}")
                    for i in range(2)]
            attT = [big.tile([128, 256], F32, tag=f"attT{i}", name=f"attT{i}")
                    for i in range(2)]

            ident = w.tile([128, 128], F32)
            make_identity(nc, ident[:])
            ident16 = w.tile([128, 128], F16)
            make_identity(nc, ident16[:])
            ones1 = w.tile([1, 64], F32)
            nc.vector.memset(ones1[:], 1.0)

            # ---------------- shard upload + AllGather -------------------
            stg = w.tile([128, EPF], EMB_DT)
            nc.sync.dma_start(out=stg[:], in_=eshard[:])
            cc_in = dramp.tile([128, EPF], EMB_DT)
            nc.sync.dma_start(out=cc_in[:], in_=stg[:])
            gath = dramp.tile([NCORES, EMB_ELEMS], EMB_DT)
            nc.gpsimd.collective_compute(
                "AllGather", OP.bypass,
                replica_groups=[list(range(NCORES))],
                ins=[cc_in[:]], outs=[gath[:]])
            stgw = w.tile([128, WROW // 128], F16)
            nc.sync.dma_start(out=stgw[:], in_=wshard[:])
            cc_inw = dramp.tile([128, WROW // 128], F16)
            nc.sync.dma_start(out=cc_inw[:], in_=stgw[:])
            gathw = dramp.tile([NCORES, WROW], F16)
            nc.gpsimd.collective_compute(
                "AllGather", OP.bypass,
                replica_groups=[list(range(NCORES))],
                ins=[cc_inw[:]], outs=[gathw[:]])

            idxT = idxp.tile([128, ni], I32)
            nc.sync.dma_start(out=idxT[:], in_=idx[:])

            # ---------------- weight slices from blob --------------------
            def wload(shape, row, off, tag):
                p, f = shape
                st = stage.tile(shape, F16, tag=f"st_{tag}", name=f"st_{tag}")
                nc.sync.dma_start(
                    out=st[:],
                    in_=gathw[row:row + 1, off:off + p * f].rearrange(
                        "a (p f) -> (a p) f", p=p))
                ft = w.tile(shape, F32, tag=f"w_{tag}", name=f"w_{tag}")
                nc.vector.tensor_copy(out=ft[:], in_=st[:])
                return ft

            w1t = [wload([128, 256], i, 0, f"w1t{i}") for i in range(3)]
            w2all = wload([128, 256], 3, 0, "w2all")
            wrzng = wload([128, 256], 4, 0, "wrzng")
            wrzna = wload([128, 256], 5, 0, "wrzna")
            watt = wload([128, 192], 6, 0, "watt")
            gv = wload([128, 6], 6, O_GV, "gv")
            w2a = wload([64, 16], 6, O_W2A, "w2a")
            w3a = wload([16, 1], 6, O_W3A, "w3a")
            ab = wload([64, 2], 6, O_AB, "ab")
            bn_gt = wload([128, 6], 6, O_BNG, "bng")
            dbt = wload([128, 3], 6, O_DBT, "dbt")
            owt = wload([128, 1], 6, O_OWT, "owt")
            obt = wload([1, 1], 6, O_OBT, "obt")

            nc.vector.memset(BUF[64:128, 0:n0], 0.0)

            if KDEBUG:
                dg16 = w.tile([8, 512], EMB_DT, tag="dg16")
                for r in range(8):
                    nc.sync.dma_start(out=dg16[r:r + 1, :],
                                      in_=gath[r:r + 1, 0:512])
                dg32 = w.tile([8, 512], F32, tag="dg32")
                nc.vector.tensor_copy(out=dg32[:], in_=dg16[:])
                nc.sync.dma_start(out=dbg_gath[:], in_=dg32[:])
                nc.sync.dma_start(out=dbg_wt[:], in_=wrzng[:])

            # ---------------- device-side embedding gathers --------------
            # HW SWDGE only handles the canonical indirect shape: offset AP
            # [128, 1], dest [128, rowlen] (one gathered row per partition
            # per instruction).  Offsets are emb row indices (gath viewed
            # as [VOCAB, E]).  fp8-wire tiles are converted+rescaled to
            # fp16 right after the gather so the transpose path stays fp16.
            in_rows = gath[:, :].rearrange("r (q e) -> (r q) e", e=E)
            fp8 = EMB_WIRE != "f16"

            def gather1(dst, col):
                nc.gpsimd.indirect_dma_start(
                    out=dst, out_offset=None, in_=in_rows,
                    in_offset=bass.IndirectOffsetOnAxis(
                        ap=idxT[:, col:col + 1], axis=0))

            def to16(gt8, width, tag):
                if not fp8:
                    return gt8
                gt = gpool.tile([128, width], F16, tag=tag)
                nc.vector.tensor_scalar_mul(gt[:], gt8[:, 0:width],
                                            1.0 / ESCALE)
                return gt

            with tc.tile_pool(name="tps", bufs=3, space="PSUM") as tps:
                ngrp = (nch + GRP - 1) // GRP
                for g in range(ngrp):
                    c0 = g * GRP
                    gw = min(GRP, nch - c0)
                    gt8 = gpool.tile([128, GRP * 64], EMB_DT, tag="gt8")
                    for c in range(gw):
                        for k in range(2):
                            gather1(
                                gt8[:, c * 64 + k * 32:c * 64 + k * 32 + 32],
                                2 * (c0 + c) + k)
                    gt = to16(gt8, gw * 64, "gt")
                    for c in range(gw):
                        j = c0 + c
                        pt = tps.tile([64, 128], F16, tag="pt")
                        nc.tensor.transpose(out=pt[:],
                                            in_=gt[:, c * 64:c * 64 + 64],
                                            identity=ident16[:])
                        nc.vector.tensor_copy(
                            out=BUF[0:64, j * 128:j * 128 + 128], in_=pt[:])

                # target-item embeddings -> qT[64:128, :]
                gq8 = gpool.tile([128, 128], EMB_DT, tag="gq8")
                for c in range(2):
                    for k in range(2):
                        gather1(gq8[:, c * 64 + k * 32:c * 64 + k * 32 + 32],
                                2 * nch + 2 * c + k)
                gq = to16(gq8, 128, "gq")
                for c in range(2):
                    pt = tps.tile([64, 128], F16, tag="pt")
                    nc.tensor.transpose(out=pt[:], in_=gq[:, c * 64:c * 64 + 64],
                                        identity=ident16[:])
                    nc.vector.tensor_copy(
                        out=qT[64:128, c * 128:c * 128 + 128], in_=pt[:])

                # sparse-feature embeddings -> spT[0], spT[1]
                gsp8 = gpool.tile([128, 512], EMB_DT, tag="gsp8")
                for j in range(16):
                    gather1(gsp8[:, j * 32:(j + 1) * 32], 2 * nch + 4 + j)
                gsp = to16(gsp8, 512, "gsp")
                for rh in range(2):
                    for gf in range(2):
                        pt2 = tps.tile([128, 128], F16, tag="pt2")
                        base = (rh * 2 + gf) * 128
                        nc.tensor.transpose(out=pt2[:],
                                            in_=gsp[:, base:base + 128],
                                            identity=ident16[:])
                        nc.vector.tensor_copy(
                            out=spT[gf][:, rh * 128:rh * 128 + 128],
                            in_=pt2[:])

            # ---------------- scan step ---------------------------------
            def scan_step(pool, t, rhs_buf, rhs_col, wrz, wn, vo, out_buf,
                          out_col, att_rhs=None):
                n = int(nts[t])
                pA = pool.tile([128, 256], F32, tag="pA")
                pB = pool.tile([128, 256], F32, tag="pB")
                rhs = rhs_buf[:, rhs_col:rhs_col + n]
                nc.tensor.matmul(out=pA[:, 0:n], lhsT=wrz, rhs=rhs,
                                 start=True, stop=True)
                nc.tensor.matmul(out=pB[:, 0:n], lhsT=wn, rhs=rhs,
                                 start=True, stop=True)
                srz = sp.tile([128, 256], F32, tag="srz")
                nc.scalar.activation(out=srz[:, 0:n], in_=pA[:, 0:n],
                                     func=AF.Sigmoid,
                                     bias=gv[:, vo:vo + 1], scale=1.0)
                t1 = sp.tile([128, 256], F32, tag="t1")
                nc.vector.scalar_tensor_tensor(
                    out=t1[64:128, 0:n], in0=pB[64:128, 0:n],
                    scalar=gv[64:128, vo + 1:vo + 2],
                    in1=srz[64:128, 0:n], op0=OP.add, op1=OP.mult)
                t2 = sp.tile([128, 256], F32, tag="t2")
                nc.vector.tensor_tensor(out=t2[64:128, 0:n],
                                        in0=t1[64:128, 0:n],
                                        in1=pB[0:64, 0:n], op=OP.add)
                nt = sp.tile([128, 256], F32, tag="nt")
                nc.scalar.activation(out=nt[64:128, 0:n], in_=t2[64:128, 0:n],
                                     func=AF.Tanh,
                                     bias=gv[64:128, vo + 2:vo + 3], scale=1.0)
                pD = pool.tile([64, 256], F32, tag="pD")
                h_prev = rhs_buf[64:128, rhs_col:rhs_col + n]
                et = sp2.tile([128, 256], F32, tag="et")
                if att_rhs is None:
                    # GRU: h' = n + z*(h - n)
                    nc.vector.tensor_tensor(out=pD[0:64, 0:n], in0=h_prev,
                                            in1=nt[64:128, 0:n],
                                            op=OP.subtract)
                    nc.vector.tensor_tensor(out=et[64:128, 0:n],
                                            in0=pD[0:64, 0:n],
                                            in1=srz[0:64, 0:n], op=OP.mult)
                    nc.vector.tensor_tensor(
                        out=out_buf[64:128, out_col:out_col + n],
                        in0=et[64:128, 0:n], in1=nt[64:128, 0:n], op=OP.add)
                else:
                    # AUGRU: h' = h + att*z*(n - h)
                    nc.vector.tensor_tensor(out=pD[0:64, 0:n],
                                            in0=nt[64:128, 0:n],
                                            in1=h_prev, op=OP.subtract)
                    pAtt = pool.tile([64, 256], F32, tag="pAtt")
                    nc.tensor.matmul(out=pAtt[:, 0:n], lhsT=ones1[:],
                                     rhs=att_rhs, start=True, stop=True)
                    zt = sp2.tile([128, 256], F32, tag="zt")
                    nc.vector.tensor_tensor(out=zt[0:64, 0:n],
                                            in0=pAtt[0:64, 0:n],
                                            in1=srz[0:64, 0:n], op=OP.mult)
                    nc.vector.tensor_tensor(out=et[64:128, 0:n],
                                            in0=pD[0:64, 0:n],
                                            in1=zt[0:64, 0:n], op=OP.mult)
                    nc.vector.tensor_tensor(
                        out=out_buf[64:128, out_col:out_col + n],
                        in0=et[64:128, 0:n],
                        in1=rhs_buf[64:128, rhs_col:rhs_col + n], op=OP.add)

            if KDEBUG:
                nc.sync.dma_start(out=dbg_x[:], in_=BUF[0:64, 0:1024])
                nc.sync.dma_start(out=dbg_q[:], in_=qT[64:128, :])
                nc.sync.dma_start(out=dbg_sp[:], in_=spT[0][:])

            # ---------------- GRU scan ----------------------------------
            with tc.tile_pool(name="sps", bufs=2, space="PSUM") as sps:
                for t in range(tmax):
                    scan_step(sps, t, BUF, int(xcols[t]),
                              wrzng[:, 0:128], wrzng[:, 128:256], 0,
                              BUF, int(hcols[t]))
            if KDEBUG:
                nc.sync.dma_start(out=dbg_h[:], in_=BUF[64:128, n0:n0 + 512])

            # ---------------- attention ---------------------------------
            with tc.tile_pool(name="apsB", bufs=1, space="PSUM") as apsB, \
                 tc.tile_pool(name="aps", bufs=2, space="PSUM") as aps:
                psB = [apsB.tile([128, T], F32, tag=f"psB{i}", name=f"psB{i}")
                       for i in range(2)]
                nc.vector.memset(psB[0][:], 0.0)
                nc.vector.memset(psB[1][:], 0.0)

                for ch in chunks:
                    wch = int(sum(int(nts[t]) for t in ch))
                    qk = sp.tile([128, 512], F32, tag="qk")
                    col = 0
                    for t in ch:
                        n = int(nts[t])
                        hc = int(hcols[t])
                        nc.vector.tensor_tensor(
                            out=qk[64:128, col:col + n],
                            in0=BUF[64:128, hc:hc + n],
                            in1=qT[64:128, 0:n], op=OP.mult)
                        col += n
                    pL1 = aps.tile([64, 512], F32, tag="pL1")
                    col = 0
                    for t in ch:
                        n = int(nts[t])
                        hc = int(hcols[t])
                        nc.tensor.matmul(out=pL1[:, col:col + n],
                                         lhsT=watt[64:128, 0:64],
                                         rhs=BUF[64:128, hc:hc + n],
                                         start=True, stop=False)
                        nc.tensor.matmul(out=pL1[:, col:col + n],
                                         lhsT=watt[64:128, 64:128],
                                         rhs=qk[64:128, col:col + n],
                                         start=False, stop=False)
                        nc.tensor.matmul(out=pL1[:, col:col + n],
                                         lhsT=watt[64:128, 128:192],
                                         rhs=qT[64:128, 0:n],
                                         start=False, stop=True)
                        col += n
                    h1 = sp.tile([64, 512], F32, tag="h1")
                    nc.scalar.activation(out=h1[:, 0:wch], in_=pL1[:, 0:wch],
                                         func=AF.Relu, bias=ab[:, 0:1],
                                         scale=1.0)
                    pL2 = aps.tile([16, 512], F32, tag="pL2")
                    nc.tensor.matmul(out=pL2[:, 0:wch], lhsT=w2a[:],
                                     rhs=h1[:, 0:wch], start=True, stop=True)
                    h2 = sp.tile([16, 512], F32, tag="h2")
                    nc.scalar.activation(out=h2[:, 0:wch], in_=pL2[:, 0:wch],
                                         func=AF.Relu, bias=ab[0:16, 1:2],
                                         scale=1.0)
                    col = 0
                    for t in ch:
                        n = int(nts[t])
                        for piece in range(2):
                            lo = piece * 128
                            if lo >= n:
                                break
                            pw = min(128, n - lo)
                            nc.tensor.matmul(
                                out=psB[piece][0:pw, t:t + 1],
                                lhsT=h2[:, col + lo:col + lo + pw],
                                rhs=w3a[:], start=True, stop=True)
                        col += n

                # masks from iota + lens, then softmax (batch-major)
                ar_i = idxp.tile([128, T], I32)
                nc.gpsimd.iota(out=ar_i[:], pattern=[[1, T]], base=0,
                               channel_multiplier=0)
                arB = sp.tile([128, T], F32, tag="arB")
                nc.vector.tensor_copy(out=arB[:], in_=ar_i[:])
                lensf = sp.tile([128, 2], F32, tag="lensf")
                nc.vector.tensor_copy(out=lensf[:], in_=idxT[:, ni - 2:ni])
                for i in range(2):
                    m1 = sp.tile([128, T], F32, tag="m1")
                    nc.vector.tensor_scalar(
                        out=m1[:], in0=arB[:], scalar1=lensf[:, i:i + 1],
                        scalar2=1.0, op0=OP.subtract, op1=OP.add)
                    msk = sp.tile([128, T], F32, tag="msk")
                    nc.vector.tensor_scalar(
                        out=msk[:], in0=m1[:], scalar1=0.0, scalar2=-1e9,
                        op0=OP.max, op1=OP.mult)
                    if KDEBUG:
                        nc.sync.dma_start(out=dbg_msk[:, i * T:(i + 1) * T],
                                          in_=msk[:])
                    sc_t = sp.tile([128, T], F32, tag="sct")
                    nc.vector.tensor_tensor(out=sc_t[:], in0=psB[i][:],
                                            in1=msk[:], op=OP.add)
                    mx = sp.tile([128, 1], F32, tag="mx")
                    nc.vector.tensor_reduce(out=mx[:], in_=sc_t[:],
                                            axis=AX.X, op=OP.max)
                    nmx = sp.tile([128, 1], F32, tag="nmx")
                    nc.vector.tensor_scalar_mul(nmx[:], mx[:], -1.0)
                    ex = sp.tile([128, 256], F32, tag="ex")
                    nc.vector.memset(ex[:], 0.0)
                    nc.scalar.activation(out=ex[:, 0:T], in_=sc_t[:],
                                         func=AF.Exp, bias=nmx[:], scale=1.0)
                    sm = sp.tile([128, 1], F32, tag="sm")
                    nc.vector.tensor_reduce(out=sm[:], in_=ex[:, 0:T],
                                            axis=AX.X, op=OP.add)
                    rs = sp.tile([128, 1], F32, tag="rs")
                    nc.vector.reciprocal(out=rs[:], in_=sm[:])
                    nc.vector.memset(attB[i][:], 0.0)
                    nc.vector.tensor_scalar(
                        out=attB[i][:, 0:T], in0=ex[:, 0:T], scalar1=rs[:],
                        scalar2=None, op0=OP.mult)
                if KDEBUG:
                    nc.sync.dma_start(out=dbg_att[:], in_=attB[0][:])

                # transpose attB -> attT (rows = t, cols = r)
                for th in range(2):
                    tw = 128 if th == 0 else T - 128
                    for rh in range(2):
                        pat = aps.tile([128, 128], F32, tag="pAT")
                        nc.tensor.transpose(
                            out=pat[0:tw, :],
                            in_=attB[rh][:, th * 128:th * 128 + tw],
                            identity=ident[:])
                        nc.vector.tensor_copy(
                            out=attT[th][0:tw, rh * 128:(rh + 1) * 128],
                            in_=pat[0:tw, :])

            # ---------------- AUGRU scan --------------------------------
            nc.vector.memset(RH2[:], 0.0)
            with tc.tile_pool(name="aups", bufs=2, space="PSUM") as aups, \
                 tc.tile_pool(name="strp", bufs=2) as strp:
                nstrip = (tmax + 7) // 8
                for s in range(nstrip):
                    t0 = s * 8
                    t1s = min(t0 + 8, tmax)
                    rows = t1s - t0
                    strip = strp.tile([1, 8 * 256], F32, tag="strip")
                    th = t0 // 128
                    r0 = t0 - th * 128
                    nc.sync.dma_start(
                        out=strip[0:1, 0:rows * 256].rearrange(
                            "o (t r) -> o t r", t=rows),
                        in_=attT[th][r0:r0 + rows, :])
                    for t in range(t0, t1s):
                        n = int(nts[t])
                        hc = int(hcols[t])
                        nc.gpsimd.tensor_copy(out=RH2[0:64, 0:n],
                                              in_=BUF[64:128, hc:hc + n])
                        arhs = strip[0:1, (t - t0) * 256:(t - t0) * 256 + n]
                        scan_step(aups, t, RH2, 0,
                                  wrzna[:, 0:128], wrzna[:, 128:256], 3,
                                  RH2, 0, att_rhs=arhs)

            # ---------------- DNN head ----------------------------------
            with tc.tile_pool(name="mps", bufs=2, space="PSUM") as mps:
                densTt = big.tile([128, BC], F32, tag="densTt")
                nc.vector.memset(densTt[:], 0.0)
                nc.sync.dma_start(out=densTt[0:DL, :], in_=densT[:])
                nc.vector.tensor_copy(out=densTt[64:128, :],
                                      in_=RH2[64:128, :])

                groups = [spT[0], spT[1], densTt]
                gwidth = [128, 128, 128]
                stats = sp.tile([128, 6], F32, tag="stats")
                nc.vector.memset(stats[:], 0.0)
                scratch = sp.tile([128, BC], F32, tag="scr")
                for gi_, (g, wd) in enumerate(zip(groups, gwidth)):
                    nc.vector.tensor_reduce(out=stats[0:wd, gi_:gi_ + 1],
                                            in_=g[0:wd, :], axis=AX.X,
                                            op=OP.add)
                    nc.vector.scalar_tensor_tensor(
                        out=scratch[0:wd, :], in0=g[0:wd, :], scalar=0.0,
                        in1=g[0:wd, :], op0=OP.add, op1=OP.mult,
                        accum_out=stats[0:wd, 3 + gi_:4 + gi_])

                cc_in2 = dramp.tile([128, 6], F32)
                cc_out2 = dramp.tile([128, 6], F32)
                nc.sync.dma_start(out=cc_in2[:], in_=stats[:])
                nc.gpsimd.collective_compute(
                    "AllReduce", OP.add,
                    replica_groups=[list(range(NCORES))],
                    ins=[cc_in2.opt()], outs=[cc_out2.opt()])
                gstats = sp.tile([128, 6], F32, tag="gstats")
                nc.sync.dma_start(out=gstats[:], in_=cc_out2[:])

                mu = sp.tile([128, 3], F32, tag="mu")
                nc.vector.tensor_scalar_mul(mu[:], gstats[:, 0:3], 1.0 / B)
                ex2 = sp.tile([128, 3], F32, tag="ex2")
                nc.vector.tensor_scalar_mul(ex2[:], gstats[:, 3:6], 1.0 / B)
                var = sp.tile([128, 3], F32, tag="var")
                nc.vector.tensor_tensor(out=var[:], in0=mu[:], in1=mu[:],
                                        op=OP.mult)
                nc.vector.tensor_tensor(out=var[:], in0=ex2[:], in1=var[:],
                                        op=OP.subtract)
                epst = sp.tile([128, 1], F32, tag="epst")
                nc.vector.memset(epst[:], 1e-5)
                sdv = sp.tile([128, 3], F32, tag="sdv")
                nc.scalar.activation(out=sdv[:], in_=var[:], func=AF.Sqrt,
                                     bias=epst[:], scale=1.0)
                rst = sp.tile([128, 3], F32, tag="rst")
                nc.vector.reciprocal(out=rst[:], in_=sdv[:])
                scl = sp.tile([128, 3], F32, tag="scl")
                nc.vector.tensor_tensor(out=scl[:], in0=bn_gt[:, 0:3],
                                        in1=rst[:], op=OP.mult)
                shf = sp.tile([128, 3], F32, tag="shf")
                nc.vector.tensor_tensor(out=shf[:], in0=mu[:], in1=scl[:],
                                        op=OP.mult)
                nc.vector.tensor_tensor(out=shf[:], in0=bn_gt[:, 3:6],
                                        in1=shf[:], op=OP.subtract)

                for gi_, (g, wd) in enumerate(zip(groups, gwidth)):
                    nc.vector.tensor_scalar(
                        out=g[0:wd, :], in0=g[0:wd, :],
                        scalar1=scl[0:wd, gi_:gi_ + 1],
                        scalar2=shf[0:wd, gi_:gi_ + 1],
                        op0=OP.mult, op1=OP.add)

                h1d = [sp.tile([128, BC], F32, tag=f"h1d{i}", name=f"h1d{i}")
                       for i in range(2)]
                for mh in range(2):
                    pm = mps.tile([128, BC], F32, tag="pm1")
                    for gi_, (g, wd) in enumerate(zip(groups, gwidth)):
                        nc.tensor.matmul(
                            out=pm[:],
                            lhsT=w1t[gi_][0:wd, mh * 128:(mh + 1) * 128],
                            rhs=g[0:wd, :], start=(gi_ == 0), stop=(gi_ == 2))
                    nc.scalar.activation(out=h1d[mh][:], in_=pm[:],
                                         func=AF.Relu,
                                         bias=dbt[:, mh:mh + 1], scale=1.0)
                pm2 = mps.tile([128, BC], F32, tag="pm2")
                for mh in range(2):
                    nc.tensor.matmul(out=pm2[:],
                                     lhsT=w2all[:, mh * 128:(mh + 1) * 128],
                                     rhs=h1d[mh][:], start=(mh == 0),
                                     stop=(mh == 1))
                h2d = sp.tile([128, BC], F32, tag="h2d")
                nc.scalar.activation(out=h2d[:], in_=pm2[:], func=AF.Relu,
                                     bias=dbt[:, 2:3], scale=1.0)
                pmo = mps.tile([1, BC], F32, tag="pmo")
                nc.tensor.matmul(out=pmo[:], lhsT=owt[:], rhs=h2d[:],
                                 start=True, stop=True)
                res = sp.tile([1, BC], F32, tag="res")
                nc.vector.tensor_scalar(
                    out=res[:], in0=pmo[:], scalar1=obt[0:1, 0:1],
                    scalar2=None, op0=OP.add)
                nc.sync.dma_start(out=out[:], in_=res[:])

    nc.compile()
    return nc


# --------------------------------------------------------------------------
def _pack_weights(gw):
    """Pack all network weights into 8 fp16 rows of WROW elems each."""

    def stack_rz(wih, whh):
        m = np.zeros((128, 128), np.float32)
        m[0:64, 0:64] = wih[64:128].T      # z, x-side
        m[64:128, 0:64] = whh[64:128].T    # z, h-side
        m[0:64, 64:128] = wih[0:64].T      # r, x-side
        m[64:128, 64:128] = whh[0:64].T    # r, h-side
        return m

    def block_n(wih, whh):
        m = np.zeros((128, 128), np.float32)
        m[0:64, 0:64] = wih[128:192].T     # i_n (-> M 0:64)
        m[64:128, 64:128] = whh[128:192].T  # h_n (-> M 64:128)
        return m

    def vecs(bih, bhh):
        brz = np.zeros(128, np.float32)
        brz[0:64] = bih[64:128] + bhh[64:128]   # z
        brz[64:128] = bih[0:64] + bhh[0:64]     # r
        bhhn = np.zeros(128, np.float32)
        bhhn[64:128] = bhh[128:192]
        bihn = np.zeros(128, np.float32)
        bihn[64:128] = bih[128:192]
        return brz, bhhn, bihn

    gvecs = np.zeros((128, 6), np.float32)
    gvecs[:, 0], gvecs[:, 1], gvecs[:, 2] = vecs(gw["gru_bih"], gw["gru_bhh"])
    gvecs[:, 3], gvecs[:, 4], gvecs[:, 5] = vecs(gw["augru_bih"],
                                                 gw["augru_bhh"])

    w1 = gw["att_w1"]
    w_att = np.zeros((128, 192), np.float32)
    w_att[64:128, 0:64] = w1[64:128] - w1[128:192]   # k-term
    w_att[64:128, 64:128] = w1[192:256]              # q*k-term
    w_att[64:128, 128:192] = w1[0:64] + w1[128:192]  # q-term
    attb = np.zeros((64, 2), np.float32)
    attb[:, 0] = gw["att_b1"]
    attb[0:16, 1] = gw["att_b2"]

    bn_g = np.zeros((128, 6), np.float32)
    for g in range(2):
        bn_g[:, g] = gw["bn_gamma"][g * 128:(g + 1) * 128]
        bn_g[:, 3 + g] = gw["bn_beta"][g * 128:(g + 1) * 128]
    bn_g[0:DL, 2] = gw["bn_gamma"][256:272]
    bn_g[0:DL, 5] = gw["bn_beta"][256:272]
    bn_g[64:128, 2] = gw["bn_gamma"][272:336]
    bn_g[64:128, 5] = gw["bn_beta"][272:336]
    dnn_w1p = np.zeros((384, 256), np.float32)
    dnn_w1p[0:256] = gw["dnn_w1"][0:256]
    dnn_w1p[256:272] = gw["dnn_w1"][256:272]
    dnn_w1p[320:384] = gw["dnn_w1"][272:336]
    dnn_b = np.zeros((128, 3), np.float32)
    dnn_b[:, 0] = gw["dnn_b1"][0:128]
    dnn_b[:, 1] = gw["dnn_b1"][128:256]
    dnn_b[:, 2] = gw["dnn_b2"]
    w2all = np.zeros((128, 256), np.float32)
    w2all[:, 0:128] = gw["dnn_w2"][0:128]
    w2all[:, 128:256] = gw["dnn_w2"][128:256]

    rows = np.zeros((8, WROW), np.float16)
    for i in range(3):
        rows[i, 0:32768] = dnn_w1p[128 * i:128 * (i + 1)].ravel()
    rows[3, 0:32768] = w2all.ravel()
    rows[4] = np.concatenate([stack_rz(gw["gru_wih"], gw["gru_whh"]),
                              block_n(gw["gru_wih"], gw["gru_whh"])],
                             axis=1).ravel()
    rows[5] = np.concatenate([stack_rz(gw["augru_wih"], gw["augru_whh"]),
                              block_n(gw["augru_wih"], gw["augru_whh"])],
                             axis=1).ravel()
    r6 = np.zeros(WROW, np.float32)
    r6[0:24576] = w_att.ravel()
    r6[O_GV:O_GV + 768] = gvecs.ravel()
    r6[O_W2A:O_W2A + 1024] = gw["att_w2"].ravel()
    r6[O_W3A:O_W3A + 16] = gw["att_w3"].ravel()
    r6[O_AB:O_AB + 128] = attb.ravel()
    r6[O_BNG:O_BNG + 768] = bn_g.ravel()
    r6[O_DBT:O_DBT + 384] = dnn_b.ravel()
    r6[O_OWT:O_OWT + 128] = gw["out_w"].ravel()
    r6[O_OBT] = float(np.asarray(gw["out_b"]).ravel()[0])
    rows[6] = r6.astype(np.float16)
    # row 7 stays all-zero: ZOFF pad gathers read from here
    return rows


def _host_prep(inputs, sch):
    lens = np.asarray(inputs["hist_valid_lens"]).astype(np.int64)
    order = sch["order"]
    tmax, nts, xcols = sch["tmax"], sch["nts"], sch["xcols"]
    nch, ni, xspan = sch["nch"], sch["ni"], sch["xspan"]

    from concourse import mybir as _mb
    emb_f = np.asarray(inputs["emb"], np.float32)
    if EMB_WIRE == "f16":
        embw = emb_f.astype(np.float16)
    else:
        wdt = _mb.dt.np({"f8e4": F8E4, "f8e3": F8E3}[EMB_WIRE])
        embw = (emb_f * ESCALE).astype(wdt)
    hist_item = np.asarray(inputs["hist_item"]).astype(np.int64)
    tgt = np.asarray(inputs["target_item"]).astype(np.int64)
    spf = np.asarray(inputs["sparse_feature"]).astype(np.int64)
    dense = np.asarray(inputs["dense_feature"], np.float32)

    gw = {k: np.asarray(inputs[k], np.float32) for k in
          ("gru_wih", "gru_whh", "gru_bih", "gru_bhh",
           "augru_wih", "augru_whh", "augru_bih", "augru_bhh",
           "att_w1", "att_b1", "att_w2", "att_b2", "att_w3", "att_b3",
           "bn_gamma", "bn_beta", "dnn_w1", "dnn_b1", "dnn_w2", "dnn_b2",
           "out_w", "out_b")}
    wrows = _pack_weights(gw)

    def off(ids):
        # emb row index (the AllGathered blob is exactly [VOCAB, E])
        return ids.astype(np.int32)

    # schedule column -> (t, r)
    dcol_t = np.zeros(xspan, np.int64)
    dcol_r = np.zeros(xspan, np.int64)
    for t in range(tmax):
        c0, n = int(xcols[t]), int(nts[t])
        dcol_t[c0:c0 + n] = t
        dcol_r[c0:c0 + n] = np.arange(n)

    cols = np.arange(xspan)
    chs = cols // 128
    ps = cols % 128

    in_maps = []
    for c in range(NCORES):
        rows = order[c::NCORES]
        eshard = embw[VSH * c:VSH * (c + 1)].reshape(128, EPF)
        wshard = wrows[c].reshape(128, WROW // 128)
        idx = np.full((128, ni), ZOFF, np.int32)
        ids = hist_item[rows[dcol_r], dcol_t, :]          # [xspan, 2]
        offs = off(ids)
        idx[ps, 2 * chs] = offs[:, 0]
        idx[ps, 2 * chs + 1] = offs[:, 1]
        qoff = off(tgt[rows])                             # [256, 2]
        for half in range(2):
            idx[:, 2 * nch + 2 * half] = qoff[128 * half:128 * (half + 1), 0]
            idx[:, 2 * nch + 2 * half + 1] = qoff[128 * half:128 * (half + 1), 1]
        spoff = off(spf[rows])                            # [256, 8]
        for rh in range(2):
            for gf in range(2):
                for j in range(4):
                    idx[:, 2 * nch + 4 + (rh * 2 + gf) * 4 + j] = \
                        spoff[128 * rh:128 * (rh + 1), 4 * gf + j]
        idx[:, ni - 2] = lens[rows[0:128]]
        idx[:, ni - 1] = lens[rows[128:256]]

        densT = np.ascontiguousarray(dense[rows, :].T)
        in_maps.append(dict(eshard=eshard, wshard=wshard, idx=idx,
                            densT=densT))
    return in_maps, order


_CACHE = {}

# --------------------------------------------------------------------------
# run_bass_kernel_spmd re-creates a fresh jax.jit wrapper on every call,
# which costs ~0.5s/call in re-trace + executable re-load even when every
# compile cache hits.  Memoize the compiled executable per Bass module so
# repeat calls only pay transfer + execution.  Semantics are identical to
# bass2jax.run_bass_via_pjrt (same _bass_exec_p custom call, same NEFF).
_EXEC_CACHE = {}


def _cached_run_bass_via_pjrt(nc, in_maps, n_cores):
    import jax
    import numpy as _np
    from jax.sharding import Mesh, PartitionSpec
    from jax.experimental.shard_map import shard_map
    from concourse import bass2jax, mybir as _mb
    from concourse.bass2jax import (_bass_exec_p, partition_id_tensor,
                                    install_neuronx_cc_hook)

    install_neuronx_cc_hook()
    if nc.dbg_addr is not None:
        if nc.dbg_callbacks:
            raise RuntimeError("dbg_callbacks unsupported here")
        in_maps = [{**m, nc.dbg_addr.name: _np.zeros((1, 2), _np.uint32)}
                   for m in in_maps]

    key = id(nc)
    if key not in _EXEC_CACHE:
        partition_name = (nc.partition_id_tensor.name
                          if nc.partition_id_tensor else None)
        in_names, out_names, out_avals = [], [], []
        for alloc in nc.m.functions[0].allocations:
            if not isinstance(alloc, _mb.MemoryLocationSet):
                continue
            name = alloc.memorylocations[0].name
            if alloc.kind == "ExternalInput":
                if name != partition_name:
                    in_names.append(name)
            elif alloc.kind == "ExternalOutput":
                out_names.append(name)
                out_avals.append(jax.core.ShapedArray(
                    tuple(alloc.tensor_shape), _mb.dt.np(alloc.dtype)))
        n_params = len(in_names)
        n_outs = len(out_avals)
        in_names_full = in_names + out_names
        if partition_name is not None:
            in_names_full.append(partition_name)
        donate = tuple(range(n_params, n_params + n_outs))

        def _body(*args):
            operands = list(args)
            if partition_name is not None:
                operands.append(partition_id_tensor())
            outs = _bass_exec_p.bind(
                *operands, out_avals=tuple(out_avals),
                in_names=tuple(in_names_full), out_names=tuple(out_names),
                lowering_input_output_aliases=(),
                sim_require_finite=True, sim_require_nnan=True, nc=nc)
            return tuple(outs)

        devices = jax.devices()[:n_cores]
        assert len(devices) == n_cores
        mesh = Mesh(_np.asarray(devices), ("core",))
        in_specs = (PartitionSpec("core"),) * (n_params + n_outs)
        out_specs = (PartitionSpec("core"),) * n_outs
        sharded = jax.jit(
            shard_map(_body, mesh=mesh, in_specs=in_specs,
                      out_specs=out_specs, check_rep=False),
            donate_argnums=donate, keep_unused=True)
        _EXEC_CACHE[key] = (sharded, in_names, out_names, out_avals, n_params)

    sharded, in_names, out_names, out_avals, n_params = _EXEC_CACHE[key]
    per_core = [[_np.asarray(m[nm]) for nm in in_names] for m in in_maps]
    concat_in = [_np.concatenate([per_core[c][i] for c in range(n_cores)],
                                 axis=0) for i in range(n_params)]
    concat_zeros = [_np.zeros((n_cores * a.shape[0], *a.shape[1:]), a.dtype)
                    for a in out_avals]
    out_arrs = sharded(*concat_in, *concat_zeros)
    return [
        {name: _np.asarray(out_arrs[i]).reshape(n_cores,
                                                *out_avals[i].shape)[c]
         for i, name in enumerate(out_names)}
        for c in range(n_cores)
    ]


def _install_pjrt_cache():
    from concourse import bass2jax
    if getattr(bass2jax.run_bass_via_pjrt, "_dien_cached", False):
        return
    _cached_run_bass_via_pjrt._dien_cached = True
    bass2jax.run_bass_via_pjrt = _cached_run_bass_via_pjrt


def kernel(**inputs):
    _install_pjrt_cache()
    lens = np.asarray(inputs["hist_valid_lens"]).astype(np.int64)
    key = hashlib.sha1(lens.tobytes()).hexdigest()
    sch = _make_schedule(lens)
    if key not in _CACHE:
        _CACHE[key] = _build(sch)
    nc = _CACHE[key]
    in_maps, order = _host_prep(inputs, sch)
    import os, time
    trace = bool(os.environ.get("KTRACE"))
    t0 = time.perf_counter()
    res = run_bass_kernel_spmd(nc, in_maps, core_ids=list(range(NCORES)),
                               trace=trace)
    kernel.last_spmd_s = time.perf_counter() - t0
    if trace and res.exec_time_ns is not None:
        print(f"HW exec time: {res.exec_time_ns} ns")
    kernel.last_res = res
    kernel.last_sch = sch
    kernel.last_maps = in_maps
    out = np.zeros((B, 1), np.float32)
    for c in range(NCORES):
        rows = order[c::NCORES]
        out[rows, 0] = res.results[c]["out"][0]
    return out


# revision 25
# speedup vs baseline: 10.2731x; 1.0187x over previous
"""DIEN forward on 8 Trainium2 NeuronCores (Bass/Tile).

Data-parallel with ragged packing; device-side embedding gather.

The axon RPC wall time is dominated by input upload (~55 MB/s tunnel) and
per-call jit re-compilation, so the kernel minimizes both:
 - The 100k x 32 embedding table is scaled x256 and quantized to fp8
   (e3m4) on the host, row-sharded 1/8th per core (0.4 MB each),
   AllGathered on-device into DRAM, and all embedding lookups (history,
   target, sparse) run as on-device indirect-DMA gathers (canonical SWDGE
   shape: offset AP [128,1], dest [128,32] -- wider offset APs are broken
   on HW) followed by fp8->fp16 rescale and PE transposes into the
   feature-major layout.  End-to-end rel err ~8e-3 (vs 2.5e-4 for an fp16
   wire; tolerance is 2e-2).
 - All network weights are packed fp16 into a second sharded blob
   (64 KB/core), AllGathered, and sliced out on-device.
 - Attention masks are built on-device from iota + per-row lengths.
 - Host ships only: emb shard (fp8), weight shard (fp16), gather offsets
   (int32), dense features (f32) -- ~0.75 MB/core vs ~7.6 MB/core for
   pre-gathered f32 activations.
 - run_bass_kernel_spmd rebuilds its jax.jit wrapper every call (~0.5 s of
   re-trace + executable re-load even on full cache hits); kernel.py
   installs a memoizing replacement for bass2jax.run_bass_via_pjrt so
   repeat calls only pay transfer + execution (~0.2 s total).

Compute layout (unchanged from the packed-ragged design):
 - Host sorts batch rows by descending hist_valid_lens, deals them to the 8
   cores round-robin over the sorted order, and builds a shared per-timestep
   active-column schedule N_t (max over cores, padded to x4).  All per-step
   shapes are compile-time constants.
 - Feature-major layout: interests/h at SBUF partitions 64:128 of one packed
   buffer BUF, gathered hist embeddings x^T at partitions 0:64, with x(t)
   stored at the columns of h(t-1) so each GRU step runs K=128-stacked
   matmuls over [x_t ; h_{t-1}] with a single rhs AP.
 - Attention runs over packed ragged columns; L3 scatters scores straight
   into a batch-major [r, t] PSUM via per-region column matmuls; softmax is
   batch-major; att is transposed back (PE) and streamed to the AUGRU as
   [1, N] rows broadcast via K=1 ones-matmuls.
 - BatchNorm (training stats over the full 2048 batch) via a tiny AllReduce.

kernel(**inputs) takes FULL unsharded inputs, returns [B, 1] float32.
"""

import hashlib
import numpy as np

import concourse.bass as bass
import concourse.bacc as bacc
import concourse.tile as tile
from concourse import mybir
from concourse.bass_utils import run_bass_kernel_spmd
from concourse.masks import make_identity

F32 = mybir.dt.float32
F16 = mybir.dt.float16
F8E4 = mybir.dt.float8e4
F8E3 = mybir.dt.float8e3
I32 = mybir.dt.int32
AF = mybir.ActivationFunctionType
OP = mybir.AluOpType
AX = mybir.AxisListType

B, T, E, NF, SL, DL, VOCAB = 2048, 200, 32, 2, 8, 16, 100000
D = E * NF          # 64
NCORES = 8
BC = B // NCORES    # 256

# sharded blobs, per core: emb rows (fp16 or scaled fp8) + fp16 weight row
VSH = VOCAB // NCORES            # 12500 emb rows per shard
EMB_ELEMS = VSH * E              # 400000
EPF = EMB_ELEMS // 128           # 3125 (eshard shipped as [128, EPF])
WROW = 32768                     # weight elems per shard row
ZOFF = 0                         # pad gathers read emb row 0 (value unused)
GRP = 16                         # gather chunks (128 cols) per staging tile

# embedding-table wire format: "f16", "f8e4" (e4m3) or "f8e3" (e3m4),
# scaled by ESCALE on the host and divided back out on-device.
import os as _os
EMB_WIRE = _os.environ.get("EMB_WIRE", "f8e3")
ESCALE = 1.0 if EMB_WIRE == "f16" else 256.0

# weight-area element offsets (row 6 smalls)
O_GV = 24576
O_W2A = O_GV + 768
O_W3A = O_W2A + 1024
O_AB = O_W3A + 16
O_BNG = O_AB + 128
O_DBT = O_BNG + 768
O_OWT = O_DBT + 384
O_OBT = O_OWT + 128


# --------------------------------------------------------------------------
def _make_schedule(lens):
    order = np.argsort(-lens, kind="stable")
    core_lens = lens[order].reshape(-1, NCORES)       # [BC, 8]
    tmax = int(lens.max())
    nts = []
    for t in range(tmax):
        n = int((core_lens > t).sum(axis=0).max())
        n = min(BC, ((n + 3) // 4) * 4)
        nts.append(max(n, 4))
    nts = np.array(nts, np.int32)
    offs = np.zeros(tmax + 1, np.int64)
    offs[1:] = np.cumsum(nts)
    m_total = int(offs[tmax])
    n0 = int(nts[0])
    xcols = np.zeros(tmax, np.int64)
    xcols[1:] = n0 + offs[:tmax - 1]                  # x(t) at h(t-1) cols
    hcols = n0 + offs[:tmax]
    xspan = int(n0 + offs[tmax - 1]) if tmax > 1 else int(nts[0])
    xspan = max(xspan, int(xcols[tmax - 1] + nts[tmax - 1]))
    nch = (xspan + 127) // 128                        # 128-col gather chunks
    buf_cols = max(n0 + m_total, nch * 128 + 128)
    ni = 2 * nch + 22                                 # idx tensor width
    return dict(order=order, tmax=tmax, nts=nts, offs=offs, m_total=m_total,
                n0=n0, xcols=xcols, hcols=hcols, xspan=xspan,
                nch=nch, ni=ni, buf_cols=buf_cols)


def _att_chunks(sch):
    chunks, cur, w = [], [], 0
    for t in range(sch["tmax"]):
        n = int(sch["nts"][t])
        if w + n > 512 and cur:
            chunks.append(cur)
            cur, w = [], 0
        cur.append(t)
        w += n
    if cur:
        chunks.append(cur)
    return chunks


# --------------------------------------------------------------------------
def _build(sch):
    nc = bacc.Bacc("TRN2", target_bir_lowering=False, debug=False,
                   num_devices=NCORES)
    tmax, nts = sch["tmax"], sch["nts"]
    hcols, xcols = sch["hcols"], sch["xcols"]
    n0, buf_cols = sch["n0"], sch["buf_cols"]
    nch, ni = sch["nch"], sch["ni"]
    chunks = _att_chunks(sch)

    EMB_DT = {"f16": F16, "f8e4": F8E4, "f8e3": F8E3}[EMB_WIRE]
    eshard = nc.dram_tensor("eshard", [128, EPF], EMB_DT, kind="ExternalInput")
    wshard = nc.dram_tensor("wshard", [128, WROW // 128], F16,
                            kind="ExternalInput")
    idx = nc.dram_tensor("idx", [128, ni], I32, kind="ExternalInput")
    densT = nc.dram_tensor("densT", [DL, BC], F32, kind="ExternalInput")
    out = nc.dram_tensor("out", [1, BC], F32, kind="ExternalOutput")
    import os
    KDEBUG = bool(os.environ.get("KDEBUG"))
    if KDEBUG:
        dbg_gath = nc.dram_tensor("dbg_gath", [8, 512], F32,
                                  kind="ExternalOutput")
        dbg_x = nc.dram_tensor("dbg_x", [64, 1024], F32, kind="ExternalOutput")
        dbg_q = nc.dram_tensor("dbg_q", [64, BC], F32, kind="ExternalOutput")
        dbg_sp = nc.dram_tensor("dbg_sp", [128, BC], F32,
                                kind="ExternalOutput")
        dbg_msk = nc.dram_tensor("dbg_msk", [128, 2 * T], F32,
                                 kind="ExternalOutput")
        dbg_h = nc.dram_tensor("dbg_h", [64, 512], F32, kind="ExternalOutput")
        dbg_wt = nc.dram_tensor("dbg_wt", [128, 256], F32,
                                kind="ExternalOutput")
        dbg_att = nc.dram_tensor("dbg_att", [128, 256], F32,
                                 kind="ExternalOutput")

    with tile.TileContext(nc) as tc:
        with tc.tile_pool(name="big", bufs=1) as big, \
             tc.tile_pool(name="w", bufs=1) as w, \
             tc.tile_pool(name="stage", bufs=2) as stage, \
             tc.tile_pool(name="gp", bufs=2) as gpool, \
             tc.tile_pool(name="idx", bufs=1) as idxp, \
             tc.tile_pool(name="s", bufs=2) as sp, \
             tc.tile_pool(name="s2", bufs=2) as sp2, \
             tc.tile_pool(name="dram", bufs=1, space="DRAM") as dramp:

            BUF = big.tile([128, buf_cols], F32)
            RH2 = big.tile([128, BC], F32)
            qT = big.tile([128, BC], F32)
            spT = [big.tile([128, BC], F32, tag=f"spT{i}", name=f"spT{i}")
                   for i in range(2)]
            attB = [big.tile([128, 256], F32, tag=f"attB{i}", name=f"attB{i}")
                    for i in range(2)]
            attT = [big.tile([128, 256], F32, tag=f"attT{i}", name=f"attT{i}")
                    for i in range(2)]

            ident = w.tile([128, 128], F32)
            make_identity(nc, ident[:])
            ident16 = w.tile([128, 128], F16)
            make_identity(nc, ident16[:])
            ones1 = w.tile([1, 64], F32)
            nc.vector.memset(ones1[:], 1.0)

            # ---------------- shard upload + AllGather -------------------
            stg = w.tile([128, EPF], EMB_DT)
            nc.sync.dma_start(out=stg[:], in_=eshard[:])
            cc_in = dramp.tile([128, EPF], EMB_DT)
            nc.sync.dma_start(out=cc_in[:], in_=stg[:])
            gath = dramp.tile([NCORES, EMB_ELEMS], EMB_DT)
            nc.gpsimd.collective_compute(
                "AllGather", OP.bypass,
                replica_groups=[list(range(NCORES))],
                ins=[cc_in[:]], outs=[gath[:]])
            stgw = w.tile([128, WROW // 128], F16)
            nc.sync.dma_start(out=stgw[:], in_=wshard[:])
            cc_inw = dramp.tile([128, WROW // 128], F16)
            nc.sync.dma_start(out=cc_inw[:], in_=stgw[:])
            gathw = dramp.tile([NCORES, WROW], F16)
            nc.gpsimd.collective_compute(
                "AllGather", OP.bypass,
                replica_groups=[list(range(NCORES))],
                ins=[cc_inw[:]], outs=[gathw[:]])

            idxT = idxp.tile([128, ni], I32)
            nc.sync.dma_start(out=idxT[:], in_=idx[:])

            # ---------------- weight slices from blob --------------------
            def wload(shape, row, off, tag):
                p, f = shape
                st = stage.tile(shape, F16, tag=f"st_{tag}", name=f"st_{tag}")
                nc.sync.dma_start(
                    out=st[:],
                    in_=gathw[row:row + 1, off:off + p * f].rearrange(
                        "a (p f) -> (a p) f", p=p))
                ft = w.tile(shape, F32, tag=f"w_{tag}", name=f"w_{tag}")
                nc.vector.tensor_copy(out=ft[:], in_=st[:])
                return ft

            w1t = [wload([128, 256], i, 0, f"w1t{i}") for i in range(3)]
            w2all = wload([128, 256], 3, 0, "w2all")
            wrzng = wload([128, 256], 4, 0, "wrzng")
            wrzna = wload([128, 256], 5, 0, "wrzna")
            watt = wload([128, 192], 6, 0, "watt")
            gv = wload([128, 6], 6, O_GV, "gv")
            w2a = wload([64, 16], 6, O_W2A, "w2a")
            w3a = wload([16, 1], 6, O_W3A, "w3a")
            ab = wload([64, 2], 6, O_AB, "ab")
            bn_gt = wload([128, 6], 6, O_BNG, "bng")
            dbt = wload([128, 3], 6, O_DBT, "dbt")
            owt = wload([128, 1], 6, O_OWT, "owt")
            obt = wload([1, 1], 6, O_OBT, "obt")

            nc.vector.memset(BUF[64:128, 0:n0], 0.0)

            if KDEBUG:
                dg16 = w.tile([8, 512], EMB_DT, tag="dg16")
                for r in range(8):
                    nc.sync.dma_start(out=dg16[r:r + 1, :],
                                      in_=gath[r:r + 1, 0:512])
                dg32 = w.tile([8, 512], F32, tag="dg32")
                nc.vector.tensor_copy(out=dg32[:], in_=dg16[:])
                nc.sync.dma_start(out=dbg_gath[:], in_=dg32[:])
                nc.sync.dma_start(out=dbg_wt[:], in_=wrzng[:])

            # ---------------- device-side embedding gathers --------------
            # HW SWDGE only handles the canonical indirect shape: offset AP
            # [128, 1], dest [128, rowlen] (one gathered row per partition
            # per instruction).  Offsets are emb row indices (gath viewed
            # as [VOCAB, E]).  fp8-wire tiles are converted+rescaled to
            # fp16 right after the gather so the transpose path stays fp16.
            in_rows = gath[:, :].rearrange("r (q e) -> (r q) e", e=E)
            fp8 = EMB_WIRE != "f16"

            def gather1(dst, col):
                nc.gpsimd.indirect_dma_start(
                    out=dst, out_offset=None, in_=in_rows,
                    in_offset=bass.IndirectOffsetOnAxis(
                        ap=idxT[:, col:col + 1], axis=0))

            def to16(gt8, width, tag):
                if not fp8:
                    return gt8
                gt = gpool.tile([128, width], F16, tag=tag)
                nc.vector.tensor_scalar_mul(gt[:], gt8[:, 0:width],
                                            1.0 / ESCALE)
                return gt

            with tc.tile_pool(name="tps", bufs=3, space="PSUM") as tps:
                ngrp = (nch + GRP - 1) // GRP
                for g in range(ngrp):
                    c0 = g * GRP
                    gw = min(GRP, nch - c0)
                    gt8 = gpool.tile([128, GRP * 64], EMB_DT, tag="gt8")
                    for c in range(gw):
                        for k in range(2):
                            gather1(
                                gt8[:, c * 64 + k * 32:c * 64 + k * 32 + 32],
                                2 * (c0 + c) + k)
                    gt = to16(gt8, gw * 64, "gt")
                    for c in range(gw):
                        j = c0 + c
                        pt = tps.tile([64, 128], F16, tag="pt")
                        nc.tensor.transpose(out=pt[:],
                                            in_=gt[:, c * 64:c * 64 + 64],
                                            identity=ident16[:])
                        nc.vector.tensor_copy(
                            out=BUF[0:64, j * 128:j * 128 + 128], in_=pt[:])

                # target-item embeddings -> qT[64:128, :]
                gq8 = gpool.tile([128, 128], EMB_DT, tag="gq8")
                for c in range(2):
                    for k in range(2):
                        gather1(gq8[:, c * 64 + k * 32:c * 64 + k * 32 + 32],
                                2 * nch + 2 * c + k)
                gq = to16(gq8, 128, "gq")
                for c in range(2):
                    pt = tps.tile([64, 128], F16, tag="pt")
                    nc.tensor.transpose(out=pt[:], in_=gq[:, c * 64:c * 64 + 64],
                                        identity=ident16[:])
                    nc.vector.tensor_copy(
                        out=qT[64:128, c * 128:c * 128 + 128], in_=pt[:])

                # sparse-feature embeddings -> spT[0], spT[1]
                gsp8 = gpool.tile([128, 512], EMB_DT, tag="gsp8")
                for j in range(16):
                    gather1(gsp8[:, j * 32:(j + 1) * 32], 2 * nch + 4 + j)
                gsp = to16(gsp8, 512, "gsp")
                for rh in range(2):
                    for gf in range(2):
                        pt2 = tps.tile([128, 128], F16, tag="pt2")
                        base = (rh * 2 + gf) * 128
                        nc.tensor.transpose(out=pt2[:],
                                            in_=gsp[:, base:base + 128],
                                            identity=ident16[:])
                        nc.vector.tensor_copy(
                            out=spT[gf][:, rh * 128:rh * 128 + 128],
                            in_=pt2[:])

            # ---------------- scan step ---------------------------------
            def scan_step(pool, t, rhs_buf, rhs_col, wrz, wn, vo, out_buf,
                          out_col, att_rhs=None):
                n = int(nts[t])
                pA = pool.tile([128, 256], F32, tag="pA")
                pB = pool.tile([128, 256], F32, tag="pB")
                rhs = rhs_buf[:, rhs_col:rhs_col + n]
                nc.tensor.matmul(out=pA[:, 0:n], lhsT=wrz, rhs=rhs,
                                 start=True, stop=True)
                nc.tensor.matmul(out=pB[:, 0:n], lhsT=wn, rhs=rhs,
                                 start=True, stop=True)
                srz = sp.tile([128, 256], F32, tag="srz")
                nc.scalar.activation(out=srz[:, 0:n], in_=pA[:, 0:n],
                                     func=AF.Sigmoid,
                                     bias=gv[:, vo:vo + 1], scale=1.0)
                t1 = sp.tile([128, 256], F32, tag="t1")
                nc.vector.scalar_tensor_tensor(
                    out=t1[64:128, 0:n], in0=pB[64:128, 0:n],
                    scalar=gv[64:128, vo + 1:vo + 2],
                    in1=srz[64:128, 0:n], op0=OP.add, op1=OP.mult)
                t2 = sp.tile([128, 256], F32, tag="t2")
                nc.vector.tensor_tensor(out=t2[64:128, 0:n],
                                        in0=t1[64:128, 0:n],
                                        in1=pB[0:64, 0:n], op=OP.add)
                nt = sp.tile([128, 256], F32, tag="nt")
                nc.scalar.activation(out=nt[64:128, 0:n], in_=t2[64:128, 0:n],
                                     func=AF.Tanh,
                                     bias=gv[64:128, vo + 2:vo + 3], scale=1.0)
                pD = pool.tile([64, 256], F32, tag="pD")
                h_prev = rhs_buf[64:128, rhs_col:rhs_col + n]
                et = sp2.tile([128, 256], F32, tag="et")
                if att_rhs is None:
                    # GRU: h' = n + z*(h - n)
                    nc.vector.tensor_tensor(out=pD[0:64, 0:n], in0=h_prev,
                                            in1=nt[64:128, 0:n],
                                            op=OP.subtract)
                    nc.vector.tensor_tensor(out=et[64:128, 0:n],
                                            in0=pD[0:64, 0:n],
                                            in1=srz[0:64, 0:n], op=OP.mult)
                    nc.vector.tensor_tensor(
                        out=out_buf[64:128, out_col:out_col + n],
                        in0=et[64:128, 0:n], in1=nt[64:128, 0:n], op=OP.add)
                else:
                    # AUGRU: h' = h + att*z*(n - h)
                    nc.vector.tensor_tensor(out=pD[0:64, 0:n],
                                            in0=nt[64:128, 0:n],
                                            in1=h_prev, op=OP.subtract)
                    pAtt = pool.tile([64, 256], F32, tag="pAtt")
                    nc.tensor.matmul(out=pAtt[:, 0:n], lhsT=ones1[:],
                                     rhs=att_rhs, start=True, stop=True)
                    zt = sp2.tile([128, 256], F32, tag="zt")
                    nc.vector.tensor_tensor(out=zt[0:64, 0:n],
                                            in0=pAtt[0:64, 0:n],
                                            in1=srz[0:64, 0:n], op=OP.mult)
                    nc.vector.tensor_tensor(out=et[64:128, 0:n],
                                            in0=pD[0:64, 0:n],
                                            in1=zt[0:64, 0:n], op=OP.mult)
                    nc.vector.tensor_tensor(
                        out=out_buf[64:128, out_col:out_col + n],
                        in0=et[64:128, 0:n],
                        in1=rhs_buf[64:128, rhs_col:rhs_col + n], op=OP.add)

            if KDEBUG:
                nc.sync.dma_start(out=dbg_x[:], in_=BUF[0:64, 0:1024])
                nc.sync.dma_start(out=dbg_q[:], in_=qT[64:128, :])
                nc.sync.dma_start(out=dbg_sp[:], in_=spT[0][:])

            # ---------------- GRU scan ----------------------------------
            with tc.tile_pool(name="sps", bufs=2, space="PSUM") as sps:
                for t in range(tmax):
                    scan_step(sps, t, BUF, int(xcols[t]),
                              wrzng[:, 0:128], wrzng[:, 128:256], 0,
                              BUF, int(hcols[t]))
            if KDEBUG:
                nc.sync.dma_start(out=dbg_h[:], in_=BUF[64:128, n0:n0 + 512])

            # ---------------- attention ---------------------------------
            with tc.tile_pool(name="apsB", bufs=1, space="PSUM") as apsB, \
                 tc.tile_pool(name="aps", bufs=2, space="PSUM") as aps:
                psB = [apsB.tile([128, T], F32, tag=f"psB{i}", name=f"psB{i}")
                       for i in range(2)]
                nc.vector.memset(psB[0][:], 0.0)
                nc.vector.memset(psB[1][:], 0.0)

                for ch in chunks:
                    wch = int(sum(int(nts[t]) for t in ch))
                    qk = sp.tile([128, 512], F32, tag="qk")
                    col = 0
                    for t in ch:
                        n = int(nts[t])
                        hc = int(hcols[t])
                        nc.vector.tensor_tensor(
                            out=qk[64:128, col:col + n],
                            in0=BUF[64:128, hc:hc + n],
                            in1=qT[64:128, 0:n], op=OP.mult)
                        col += n
                    pL1 = aps.tile([64, 512], F32, tag="pL1")
                    col = 0
                    for t in ch:
                        n = int(nts[t])
                        hc = int(hcols[t])
                        nc.tensor.matmul(out=pL1[:, col:col + n],
                                         lhsT=watt[64:128, 0:64],
                                         rhs=BUF[64:128, hc:hc + n],
                                         start=True, stop=False)
                        nc.tensor.matmul(out=pL1[:, col:col + n],
                                         lhsT=watt[64:128, 64:128],
                                         rhs=qk[64:128, col:col + n],
                                         start=False, stop=False)
                        nc.tensor.matmul(out=pL1[:, col:col + n],
                                         lhsT=watt[64:128, 128:192],
                                         rhs=qT[64:128, 0:n],
                                         start=False, stop=True)
                        col += n
                    h1 = sp.tile([64, 512], F32, tag="h1")
                    nc.scalar.activation(out=h1[:, 0:wch], in_=pL1[:, 0:wch],
                                         func=AF.Relu, bias=ab[:, 0:1],
                                         scale=1.0)
                    pL2 = aps.tile([16, 512], F32, tag="pL2")
                    nc.tensor.matmul(out=pL2[:, 0:wch], lhsT=w2a[:],
                                     rhs=h1[:, 0:wch], start=True, stop=True)
                    h2 = sp.tile([16, 512], F32, tag="h2")
                    nc.scalar.activation(out=h2[:, 0:wch], in_=pL2[:, 0:wch],
                                         func=AF.Relu, bias=ab[0:16, 1:2],
                                         scale=1.0)
                    col = 0
                    for t in ch:
                        n = int(nts[t])
                        for piece in range(2):
                            lo = piece * 128
                            if lo >= n:
                                break
                            pw = min(128, n - lo)
                            nc.tensor.matmul(
                                out=psB[piece][0:pw, t:t + 1],
                                lhsT=h2[:, col + lo:col + lo + pw],
                                rhs=w3a[:], start=True, stop=True)
                        col += n

                # masks from iota + lens, then softmax (batch-major)
                ar_i = idxp.tile([128, T], I32)
                nc.gpsimd.iota(out=ar_i[:], pattern=[[1, T]], base=0,
                               channel_multiplier=0)
                arB = sp.tile([128, T], F32, tag="arB")
                nc.vector.tensor_copy(out=arB[:], in_=ar_i[:])
                lensf = sp.tile([128, 2], F32, tag="lensf")
                nc.vector.tensor_copy(out=lensf[:], in_=idxT[:, ni - 2:ni])
                for i in range(2):
                    m1 = sp.tile([128, T], F32, tag="m1")
                    nc.vector.tensor_scalar(
                        out=m1[:], in0=arB[:], scalar1=lensf[:, i:i + 1],
                        scalar2=1.0, op0=OP.subtract, op1=OP.add)
                    msk = sp.tile([128, T], F32, tag="msk")
                    nc.vector.tensor_scalar(
                        out=msk[:], in0=m1[:], scalar1=0.0, scalar2=-1e9,
                        op0=OP.max, op1=OP.mult)
                    if KDEBUG:
                        nc.sync.dma_start(out=dbg_msk[:, i * T:(i + 1) * T],
                                          in_=msk[:])
                    sc_t = sp.tile([128, T], F32, tag="sct")
                    nc.vector.tensor_tensor(out=sc_t[:], in0=psB[i][:],
                                            in1=msk[:], op=OP.add)
                    mx = sp.tile([128, 1], F32, tag="mx")
                    nc.vector.tensor_reduce(out=mx[:], in_=sc_t[:],
                                            axis=AX.X, op=OP.max)
                    nmx = sp.tile([128, 1], F32, tag="nmx")
                    nc.vector.tensor_scalar_mul(nmx[:], mx[:], -1.0)
                    ex = sp.tile([128, 256], F32, tag="ex")
                    nc.vector.memset(ex[:], 0.0)
                    nc.scalar.activation(out=ex[:, 0:T], in_=sc_t[:],
                                         func=AF.Exp, bias=nmx[:], scale=1.0)
                    sm = sp.tile([128, 1], F32, tag="sm")
                    nc.vector.tensor_reduce(out=sm[:], in_=ex[:, 0:T],
                                            axis=AX.X, op=OP.add)
                    rs = sp.tile([128, 1], F32, tag="rs")
                    nc.vector.reciprocal(out=rs[:], in_=sm[:])
                    nc.vector.memset(attB[i][:], 0.0)
                    nc.vector.tensor_scalar(
                        out=attB[i][:, 0:T], in0=ex[:, 0:T], scalar1=rs[:],
                        scalar2=None, op0=OP.mult)
                if KDEBUG:
                    nc.sync.dma_start(out=dbg_att[:], in_=attB[0][:])

                # transpose attB -> attT (rows = t, cols = r)
                for th in range(2):
                    tw = 128 if th == 0 else T - 128
                    for rh in range(2):
                        pat = aps.tile([128, 128], F32, tag="pAT")
                        nc.tensor.transpose(
                            out=pat[0:tw, :],
                            in_=attB[rh][:, th * 128:th * 128 + tw],
                            identity=ident[:])
                        nc.vector.tensor_copy(
                            out=attT[th][0:tw, rh * 128:(rh + 1) * 128],
                            in_=pat[0:tw, :])

            # ---------------- AUGRU scan --------------------------------
            nc.vector.memset(RH2[:], 0.0)
            with tc.tile_pool(name="aups", bufs=2, space="PSUM") as aups, \
                 tc.tile_pool(name="strp", bufs=2) as strp:
                nstrip = (tmax + 7) // 8
                for s in range(nstrip):
                    t0 = s * 8
                    t1s = min(t0 + 8, tmax)
                    rows = t1s - t0
                    strip = strp.tile([1, 8 * 256], F32, tag="strip")
                    th = t0 // 128
                    r0 = t0 - th * 128
                    nc.sync.dma_start(
                        out=strip[0:1, 0:rows * 256].rearrange(
                            "o (t r) -> o t r", t=rows),
                        in_=attT[th][r0:r0 + rows, :])
                    for t in range(t0, t1s):
                        n = int(nts[t])
                        hc = int(hcols[t])
                        nc.gpsimd.tensor_copy(out=RH2[0:64, 0:n],
                                              in_=BUF[64:128, hc:hc + n])
                        arhs = strip[0:1, (t - t0) * 256:(t - t0) * 256 + n]
                        scan_step(aups, t, RH2, 0,
                                  wrzna[:, 0:128], wrzna[:, 128:256], 3,
                                  RH2, 0, att_rhs=arhs)

            # ---------------- DNN head ----------------------------------
            with tc.tile_pool(name="mps", bufs=2, space="PSUM") as mps:
                densTt = big.tile([128, BC], F32, tag="densTt")
                nc.vector.memset(densTt[:], 0.0)
                nc.sync.dma_start(out=densTt[0:DL, :], in_=densT[:])
                nc.vector.tensor_copy(out=densTt[64:128, :],
                                      in_=RH2[64:128, :])

                groups = [spT[0], spT[1], densTt]
                gwidth = [128, 128, 128]
                stats = sp.tile([128, 6], F32, tag="stats")
                nc.vector.memset(stats[:], 0.0)
                scratch = sp.tile([128, BC], F32, tag="scr")
                for gi_, (g, wd) in enumerate(zip(groups, gwidth)):
                    nc.vector.tensor_reduce(out=stats[0:wd, gi_:gi_ + 1],
                                            in_=g[0:wd, :], axis=AX.X,
                                            op=OP.add)
                    nc.vector.scalar_tensor_tensor(
                        out=scratch[0:wd, :], in0=g[0:wd, :], scalar=0.0,
                        in1=g[0:wd, :], op0=OP.add, op1=OP.mult,
                        accum_out=stats[0:wd, 3 + gi_:4 + gi_])

                cc_in2 = dramp.tile([128, 6], F32)
                cc_out2 = dramp.tile([128, 6], F32)
                nc.sync.dma_start(out=cc_in2[:], in_=stats[:])
                nc.gpsimd.collective_compute(
                    "AllReduce", OP.add,
                    replica_groups=[list(range(NCORES))],
                    ins=[cc_in2.opt()], outs=[cc_out2.opt()])
                gstats = sp.tile([128, 6], F32, tag="gstats")
                nc.sync.dma_start(out=gstats[:], in_=cc_out2[:])

                mu = sp.tile([128, 3], F32, tag="mu")
                nc.vector.tensor_scalar_mul(mu[:], gstats[:, 0:3], 1.0 / B)
                ex2 = sp.tile([128, 3], F32, tag="ex2")
                nc.vector.tensor_scalar_mul(ex2[:], gstats[:, 3:6], 1.0 / B)
                var = sp.tile([128, 3], F32, tag="var")
                nc.vector.tensor_tensor(out=var[:], in0=mu[:], in1=mu[:],
                                        op=OP.mult)
                nc.vector.tensor_tensor(out=var[:], in0=ex2[:], in1=var[:],
                                        op=OP.subtract)
                epst = sp.tile([128, 1], F32, tag="epst")
                nc.vector.memset(epst[:], 1e-5)
                sdv = sp.tile([128, 3], F32, tag="sdv")
                nc.scalar.activation(out=sdv[:], in_=var[:], func=AF.Sqrt,
                                     bias=epst[:], scale=1.0)
                rst = sp.tile([128, 3], F32, tag="rst")
                nc.vector.reciprocal(out=rst[:], in_=sdv[:])
                scl = sp.tile([128, 3], F32, tag="scl")
                nc.vector.tensor_tensor(out=scl[:], in0=bn_gt[:, 0:3],
                                        in1=rst[:], op=OP.mult)
                shf = sp.tile([128, 3], F32, tag="shf")
                nc.vector.tensor_tensor(out=shf[:], in0=mu[:], in1=scl[:],
                                        op=OP.mult)
                nc.vector.tensor_tensor(out=shf[:], in0=bn_gt[:, 3:6],
                                        in1=shf[:], op=OP.subtract)

                for gi_, (g, wd) in enumerate(zip(groups, gwidth)):
                    nc.vector.tensor_scalar(
                        out=g[0:wd, :], in0=g[0:wd, :],
                        scalar1=scl[0:wd, gi_:gi_ + 1],
                        scalar2=shf[0:wd, gi_:gi_ + 1],
                        op0=OP.mult, op1=OP.add)

                h1d = [sp.tile([128, BC], F32, tag=f"h1d{i}", name=f"h1d{i}")
                       for i in range(2)]
                for mh in range(2):
                    pm = mps.tile([128, BC], F32, tag="pm1")
                    for gi_, (g, wd) in enumerate(zip(groups, gwidth)):
                        nc.tensor.matmul(
                            out=pm[:],
                            lhsT=w1t[gi_][0:wd, mh * 128:(mh + 1) * 128],
                            rhs=g[0:wd, :], start=(gi_ == 0), stop=(gi_ == 2))
                    nc.scalar.activation(out=h1d[mh][:], in_=pm[:],
                                         func=AF.Relu,
                                         bias=dbt[:, mh:mh + 1], scale=1.0)
                pm2 = mps.tile([128, BC], F32, tag="pm2")
                for mh in range(2):
                    nc.tensor.matmul(out=pm2[:],
                                     lhsT=w2all[:, mh * 128:(mh + 1) * 128],
                                     rhs=h1d[mh][:], start=(mh == 0),
                                     stop=(mh == 1))
                h2d = sp.tile([128, BC], F32, tag="h2d")
                nc.scalar.activation(out=h2d[:], in_=pm2[:], func=AF.Relu,
                                     bias=dbt[:, 2:3], scale=1.0)
                pmo = mps.tile([1, BC], F32, tag="pmo")
                nc.tensor.matmul(out=pmo[:], lhsT=owt[:], rhs=h2d[:],
                                 start=True, stop=True)
                res = sp.tile([1, BC], F32, tag="res")
                nc.vector.tensor_scalar(
                    out=res[:], in0=pmo[:], scalar1=obt[0:1, 0:1],
                    scalar2=None, op0=OP.add)
                nc.sync.dma_start(out=out[:], in_=res[:])

    nc.compile()
    return nc


# --------------------------------------------------------------------------
def _pack_weights(gw):
    """Pack all network weights into 8 fp16 rows of WROW elems each."""

    def stack_rz(wih, whh):
        m = np.zeros((128, 128), np.float32)
        m[0:64, 0:64] = wih[64:128].T      # z, x-side
        m[64:128, 0:64] = whh[64:128].T    # z, h-side
        m[0:64, 64:128] = wih[0:64].T      # r, x-side
        m[64:128, 64:128] = whh[0:64].T    # r, h-side
        return m

    def block_n(wih, whh):
        m = np.zeros((128, 128), np.float32)
        m[0:64, 0:64] = wih[128:192].T     # i_n (-> M 0:64)
        m[64:128, 64:128] = whh[128:192].T  # h_n (-> M 64:128)
        return m

    def vecs(bih, bhh):
        brz = np.zeros(128, np.float32)
        brz[0:64] = bih[64:128] + bhh[64:128]   # z
        brz[64:128] = bih[0:64] + bhh[0:64]     # r
        bhhn = np.zeros(128, np.float32)
        bhhn[64:128] = bhh[128:192]
        bihn = np.zeros(128, np.float32)
        bihn[64:128] = bih[128:192]
        return brz, bhhn, bihn

    gvecs = np.zeros((128, 6), np.float32)
    gvecs[:, 0], gvecs[:, 1], gvecs[:, 2] = vecs(gw["gru_bih"], gw["gru_bhh"])
    gvecs[:, 3], gvecs[:, 4], gvecs[:, 5] = vecs(gw["augru_bih"],
                                                 gw["augru_bhh"])

    w1 = gw["att_w1"]
    w_att = np.zeros((128, 192), np.float32)
    w_att[64:128, 0:64] = w1[64:128] - w1[128:192]   # k-term
    w_att[64:128, 64:128] = w1[192:256]              # q*k-term
    w_att[64:128, 128:192] = w1[0:64] + w1[128:192]  # q-term
    attb = np.zeros((64, 2), np.float32)
    attb[:, 0] = gw["att_b1"]
    attb[0:16, 1] = gw["att_b2"]

    bn_g = np.zeros((128, 6), np.float32)
    for g in range(2):
        bn_g[:, g] = gw["bn_gamma"][g * 128:(g + 1) * 128]
        bn_g[:, 3 + g] = gw["bn_beta"][g * 128:(g + 1) * 128]
    bn_g[0:DL, 2] = gw["bn_gamma"][256:272]
    bn_g[0:DL, 5] = gw["bn_beta"][256:272]
    bn_g[64:128, 2] = gw["bn_gamma"][272:336]
    bn_g[64:128, 5] = gw["bn_beta"][272:336]
    dnn_w1p = np.zeros((384, 256), np.float32)
    dnn_w1p[0:256] = gw["dnn_w1"][0:256]
    dnn_w1p[256:272] = gw["dnn_w1"][256:272]
    dnn_w1p[320:384] = gw["dnn_w1"][272:336]
    dnn_b = np.zeros((128, 3), np.float32)
    dnn_b[:, 0] = gw["dnn_b1"][0:128]
    dnn_b[:, 1] = gw["dnn_b1"][128:256]
    dnn_b[:, 2] = gw["dnn_b2"]
    w2all = np.zeros((128, 256), np.float32)
    w2all[:, 0:128] = gw["dnn_w2"][0:128]
    w2all[:, 128:256] = gw["dnn_w2"][128:256]

    rows = np.zeros((8, WROW), np.float16)
    for i in range(3):
        rows[i, 0:32768] = dnn_w1p[128 * i:128 * (i + 1)].ravel()
    rows[3, 0:32768] = w2all.ravel()
    rows[4] = np.concatenate([stack_rz(gw["gru_wih"], gw["gru_whh"]),
                              block_n(gw["gru_wih"], gw["gru_whh"])],
                             axis=1).ravel()
    rows[5] = np.concatenate([stack_rz(gw["augru_wih"], gw["augru_whh"]),
                              block_n(gw["augru_wih"], gw["augru_whh"])],
                             axis=1).ravel()
    r6 = np.zeros(WROW, np.float32)
    r6[0:24576] = w_att.ravel()
    r6[O_GV:O_GV + 768] = gvecs.ravel()
    r6[O_W2A:O_W2A + 1024] = gw["att_w2"].ravel()
    r6[O_W3A:O_W3A + 16] = gw["att_w3"].ravel()
    r6[O_AB:O_AB + 128] = attb.ravel()
    r6[O_BNG:O_BNG + 768] = bn_g.ravel()
    r6[O_DBT:O_DBT + 384] = dnn_b.ravel()
    r6[O_OWT:O_OWT + 128] = gw["out_w"].ravel()
    r6[O_OBT] = float(np.asarray(gw["out_b"]).ravel()[0])
    rows[6] = r6.astype(np.float16)
    # row 7 stays all-zero: ZOFF pad gathers read from here
    return rows


def _host_prep(inputs, sch):
    lens = np.asarray(inputs["hist_valid_lens"]).astype(np.int64)
    order = sch["order"]
    tmax, nts, xcols = sch["tmax"], sch["nts"], sch["xcols"]
    nch, ni, xspan = sch["nch"], sch["ni"], sch["xspan"]

    from concourse import mybir as _mb
    emb_f = np.asarray(inputs["emb"], np.float32)
    if EMB_WIRE == "f16":
        embw = emb_f.astype(np.float16)
    else:
        wdt = _mb.dt.np({"f8e4": F8E4, "f8e3": F8E3}[EMB_WIRE])
        embw = (emb_f * ESCALE).astype(wdt)
    hist_item = np.asarray(inputs["hist_item"]).astype(np.int64)
    tgt = np.asarray(inputs["target_item"]).astype(np.int64)
    spf = np.asarray(inputs["sparse_feature"]).astype(np.int64)
    dense = np.asarray(inputs["dense_feature"], np.float32)

    gw = {k: np.asarray(inputs[k], np.float32) for k in
          ("gru_wih", "gru_whh", "gru_bih", "gru_bhh",
           "augru_wih", "augru_whh", "augru_bih", "augru_bhh",
           "att_w1", "att_b1", "att_w2", "att_b2", "att_w3", "att_b3",
           "bn_gamma", "bn_beta", "dnn_w1", "dnn_b1", "dnn_w2", "dnn_b2",
           "out_w", "out_b")}
    wrows = _pack_weights(gw)

    def off(ids):
        # emb row index (the AllGathered blob is exactly [VOCAB, E])
        return ids.astype(np.int32)

    # schedule column -> (t, r)
    dcol_t = np.zeros(xspan, np.int64)
    dcol_r = np.zeros(xspan, np.int64)
    for t in range(tmax):
        c0, n = int(xcols[t]), int(nts[t])
        dcol_t[c0:c0 + n] = t
        dcol_r[c0:c0 + n] = np.arange(n)

    cols = np.arange(xspan)
    chs = cols // 128
    ps = cols % 128

    in_maps = []
    for c in range(NCORES):
        rows = order[c::NCORES]
        eshard = embw[VSH * c:VSH * (c + 1)].reshape(128, EPF)
        wshard = wrows[c].reshape(128, WROW // 128)
        idx = np.full((128, ni), ZOFF, np.int32)
        ids = hist_item[rows[dcol_r], dcol_t, :]          # [xspan, 2]
        offs = off(ids)
        idx[ps, 2 * chs] = offs[:, 0]
        idx[ps, 2 * chs + 1] = offs[:, 1]
        qoff = off(tgt[rows])                             # [256, 2]
        for half in range(2):
            idx[:, 2 * nch + 2 * half] = qoff[128 * half:128 * (half + 1), 0]
            idx[:, 2 * nch + 2 * half + 1] = qoff[128 * half:128 * (half + 1), 1]
        spoff = off(spf[rows])                            # [256, 8]
        for rh in range(2):
            for gf in range(2):
                for j in range(4):
                    idx[:, 2 * nch + 4 + (rh * 2 + gf) * 4 + j] = \
                        spoff[128 * rh:128 * (rh + 1), 4 * gf + j]
        idx[:, ni - 2] = lens[rows[0:128]]
        idx[:, ni - 1] = lens[rows[128:256]]

        densT = np.ascontiguousarray(dense[rows, :].T)
        in_maps.append(dict(eshard=eshard, wshard=wshard, idx=idx,
                            densT=densT))
    return in_maps, order


_CACHE = {}

# --------------------------------------------------------------------------
# run_bass_kernel_spmd re-creates a fresh jax.jit wrapper on every call,
# which costs ~0.5s/call in re-trace + executable re-load even when every
# compile cache hits.  Memoize the compiled executable per Bass module so
# repeat calls only pay transfer + execution.  Semantics are identical to
# bass2jax.run_bass_via_pjrt (same _bass_exec_p custom call, same NEFF).
_EXEC_CACHE = {}


def _cached_run_bass_via_pjrt(nc, in_maps, n_cores):
    import jax
    import numpy as _np
    from jax.sharding import Mesh, PartitionSpec
    from jax.experimental.shard_map import shard_map
    from concourse import bass2jax, mybir as _mb
    from concourse.bass2jax import (_bass_exec_p, partition_id_tensor,
                                    install_neuronx_cc_hook)

    install_neuronx_cc_hook()
    if nc.dbg_addr is not None:
        if nc.dbg_callbacks:
            raise RuntimeError("dbg_callbacks unsupported here")
        in_maps = [{**m, nc.dbg_addr.name: _np.zeros((1, 2), _np.uint32)}
                   for m in in_maps]

    key = id(nc)
    if key not in _EXEC_CACHE:
        partition_name = (nc.partition_id_tensor.name
                          if nc.partition_id_tensor else None)
        in_names, out_names, out_avals = [], [], []
        for alloc in nc.m.functions[0].allocations:
            if not isinstance(alloc, _mb.MemoryLocationSet):
                continue
            name = alloc.memorylocations[0].name
            if alloc.kind == "ExternalInput":
                if name != partition_name:
                    in_names.append(name)
            elif alloc.kind == "ExternalOutput":
                out_names.append(name)
                out_avals.append(jax.core.ShapedArray(
                    tuple(alloc.tensor_shape), _mb.dt.np(alloc.dtype)))
        n_params = len(in_names)
        n_outs = len(out_avals)
        in_names_full = in_names + out_names
        if partition_name is not None:
            in_names_full.append(partition_name)
        donate = tuple(range(n_params, n_params + n_outs))

        def _body(*args):
            operands = list(args)
            if partition_name is not None:
                operands.append(partition_id_tensor())
            outs = _bass_exec_p.bind(
                *operands, out_avals=tuple(out_avals),
                in_names=tuple(in_names_full), out_names=tuple(out_names),
                lowering_input_output_aliases=(),
                sim_require_finite=True, sim_require_nnan=True, nc=nc)
            return tuple(outs)

        devices = jax.devices()[:n_cores]
        assert len(devices) == n_cores
        mesh = Mesh(_np.asarray(devices), ("core",))
        in_specs = (PartitionSpec("core"),) * (n_params + n_outs)
        out_specs = (PartitionSpec("core"),) * n_outs
        sharded = jax.jit(
            shard_map(_body, mesh=mesh, in_specs=in_specs,
                      out_specs=out_specs, check_rep=False),
            donate_argnums=donate, keep_unused=True)
        _EXEC_CACHE[key] = (sharded, in_names, out_names, out_avals, n_params)

    sharded, in_names, out_names, out_avals, n_params = _EXEC_CACHE[key]
    per_core = [[_np.asarray(m[nm]) for nm in in_names] for m in in_maps]
    concat_in = [_np.concatenate([per_core[c][i] for c in range(n_cores)],
                                 axis=0) for i in range(n_params)]
    concat_zeros = [_np.zeros((n_cores * a.shape[0], *a.shape[1:]), a.dtype)
                    for a in out_avals]
    out_arrs = sharded(*concat_in, *concat_zeros)
    return [
        {name: _np.asarray(out_arrs[i]).reshape(n_cores,
                                                *out_avals[i].shape)[c]
         for i, name in enumerate(out_names)}
        for c in range(n_cores)
    ]


def _install_pjrt_cache():
    from concourse import bass2jax
    if getattr(bass2jax.run_bass_via_pjrt, "_dien_cached", False):
        return
    _cached_run_bass_via_pjrt._dien_cached = True
    bass2jax.run_bass_via_pjrt = _cached_run_bass_via_pjrt


def kernel(**inputs):
    _install_pjrt_cache()
    lens = np.asarray(inputs["hist_valid_lens"]).astype(np.int64)
    key = hashlib.sha1(lens.tobytes()).hexdigest()
    sch = _make_schedule(lens)
    if key not in _CACHE:
        _CACHE[key] = _build(sch)
    nc = _CACHE[key]
    in_maps, order = _host_prep(inputs, sch)
    import os, time
    trace = bool(os.environ.get("KTRACE"))
    t0 = time.perf_counter()
    res = run_bass_kernel_spmd(nc, in_maps, core_ids=list(range(NCORES)),
                               trace=trace)
    kernel.last_spmd_s = time.perf_counter() - t0
    if trace and res.exec_time_ns is not None:
        print(f"HW exec time: {res.exec_time_ns} ns")
    kernel.last_res = res
    kernel.last_sch = sch
    kernel.last_maps = in_maps
    out = np.zeros((B, 1), np.float32)
    for c in range(NCORES):
        rows = order[c::NCORES]
        out[rows, 0] = res.results[c]["out"][0]
    return out
